# revision 5
# baseline (speedup 1.0000x reference)
"""Trainium2 Bass kernel for CausalMessagePassing (B=8, N=2048, D=256, H=4).

Strategy: data-parallel across 8 NeuronCores, one graph per core.
Per-core dataflow is column-major ("transposed spine"):
  x^T -> q^T,k^T (col-major), v (row-major with ones column for softmax sums)
  scores^T[j,i] = k^T.T @ q^T per head (f32r matmuls, 1 cyc/row)
  e = exp(scores * 1/sqrt(hd)) with causal mask applied on-chip via
  affine_select (the [N,N] mask input is tril(ones) by construction and is
  never DMA'd).
  ctx'^T[65,i] = v'.T @ e^T accumulated over j-blocks; row 64 = softmax sums.
  normalize via K=1 broadcast matmul of 1/sums, fused into PSUM eviction.
  messages^T = Wo.T @ ectx^T (+bo), u^T = relu(Wu.T @ [x^T; m^T] + bu),
  PE-transpose u^T -> u, DMA out.
"""
import sys

sys.path.insert(0, "/opt/trn_rl_repo")

import numpy as np

import concourse.bass as bass  # noqa: F401
import concourse.mybir as mybir
import concourse.tile as tile
from concourse import bacc
from concourse.masks import make_identity

B, N, DM, H = 8, 2048, 256, 4
HD = DM // H  # 64
NB = N // 128  # 16 j-blocks
IT = N // 512  # 4 i-tiles
F32 = mybir.dt.float32
F32R = mybir.dt.float32r


def build_program():
    nc = bacc.Bacc("TRN2", target_bir_lowering=False, debug=False)
    x_d = nc.dram_tensor("x", [N, DM], F32, kind="ExternalInput").ap()
    wq_d = nc.dram_tensor("wq", [DM, DM], F32, kind="ExternalInput").ap()
    wk_d = nc.dram_tensor("wk", [DM, DM], F32, kind="ExternalInput").ap()
    wv_d = nc.dram_tensor("wv", [DM, DM], F32, kind="ExternalInput").ap()
    wo_d = nc.dram_tensor("wo", [DM, DM], F32, kind="ExternalInput").ap()
    wu_d = nc.dram_tensor("wu", [2 * DM, DM], F32, kind="ExternalInput").ap()
    bq_d = nc.dram_tensor("bq", [DM], F32, kind="ExternalInput").ap()
    bk_d = nc.dram_tensor("bk", [DM], F32, kind="ExternalInput").ap()
    bv_d = nc.dram_tensor("bv", [DM], F32, kind="ExternalInput").ap()
    bo_d = nc.dram_tensor("bo", [DM], F32, kind="ExternalInput").ap()
    bu_d = nc.dram_tensor("bu", [DM], F32, kind="ExternalInput").ap()
    out_d = nc.dram_tensor("out", [N, DM], F32, kind="ExternalOutput").ap()

    def r(ap):
        return ap.bitcast(F32R)

    with tile.TileContext(nc) as tc:
        with (
            tc.tile_pool(name="const", bufs=1) as cpool,
            tc.tile_pool(name="big", bufs=1) as bpool,
            tc.tile_pool(name="work", bufs=3) as wpool,
            tc.tile_pool(name="mm", bufs=3, space="PSUM") as mmp,
            tc.tile_pool(name="ctxp", bufs=1, space="PSUM") as ctxp,
        )            :
            # ---- constants / weights ----
            ident = cpool.tile([128, 128], F32R, tag="ident")
            ident_f = cpool.tile([128, 128], F32, tag="identf")
            make_identity(nc, ident_f[:])
            nc.vector.tensor_copy(ident[:], ident_f[:])
            wq_sb = [cpool.tile([128, DM], F32R, tag=f"wq{c}", name=f"wq{c}") for c in range(2)]
            wk_sb = [cpool.tile([128, DM], F32R, tag=f"wk{c}", name=f"wk{c}") for c in range(2)]
            wv_sb = [cpool.tile([128, DM], F32R, tag=f"wv{c}", name=f"wv{c}") for c in range(2)]
            wo_sb = [cpool.tile([128, DM], F32R, tag=f"wo{c}", name=f"wo{c}") for c in range(2)]
            wu_sb = [cpool.tile([128, DM], F32R, tag=f"wu{c}", name=f"wu{c}") for c in range(4)]
            for c in range(2):
                nc.sync.dma_start(wq_sb[c][:], r(wq_d[c * 128:(c + 1) * 128, :]))
                nc.sync.dma_start(wk_sb[c][:], r(wk_d[c * 128:(c + 1) * 128, :]))
                nc.sync.dma_start(wv_sb[c][:], r(wv_d[c * 128:(c + 1) * 128, :]))
                nc.sync.dma_start(wo_sb[c][:], r(wo_d[c * 128:(c + 1) * 128, :]))
            for c in range(4):
                nc.sync.dma_start(wu_sb[c][:], r(wu_d[c * 128:(c + 1) * 128, :]))
            bq_c = [cpool.tile([128, 1], F32, tag=f"bq{b}", name=f"bqc{b}") for b in range(2)]
            bk_c = [cpool.tile([128, 1], F32, tag=f"bk{b}", name=f"bkc{b}") for b in range(2)]
            bo_c = [cpool.tile([128, 1], F32, tag=f"bo{b}", name=f"boc{b}") for b in range(2)]
            bu_c = [cpool.tile([128, 1], F32, tag=f"bu{b}", name=f"buc{b}") for b in range(2)]
            for b_, (dsrc, dst) in enumerate(
                [(bq_d, bq_c), (bk_d, bk_c), (bo_d, bo_c), (bu_d, bu_c)]
            ):
                for blk in range(2):
                    nc.sync.dma_start(
                        dst[blk][:],
                        dsrc[blk * 128:(blk + 1) * 128].rearrange("(a b) -> a b", b=1),
                    )
            # bv broadcast tile [128, 256] (f32; only used by DVE add)
            bv_row = cpool.tile([1, DM], F32, tag="bvrow")
            nc.sync.dma_start(bv_row[:], bv_d.rearrange("(b a) -> b a", b=1))
            ones1 = cpool.tile([1, 128], F32, tag="ones1")
            nc.gpsimd.memset(ones1[:], 1.0)
            bv_bc = cpool.tile([128, DM], F32, tag="bvbc")
            pt = mmp.tile([128, DM], F32, tag="mm")
            nc.tensor.matmul(pt[:], ones1[:], bv_row[:], start=True, stop=True)
            nc.vector.tensor_copy(bv_bc[:], pt[:])
            ones_r = cpool.tile([1, 64], F32R, tag="onesr")
            ones_rf = cpool.tile([1, 64], F32, tag="onesrf")
            nc.gpsimd.memset(ones_rf[:], 1.0)
            nc.vector.tensor_copy(ones_r[:], ones_rf[:])
            ones_col4 = cpool.tile([128, 4], F32, tag="onescol4")
            nc.gpsimd.memset(ones_col4[:], 1.0)

            # ---- x load + transpose to x^T ----
            xT = [bpool.tile([128, N], F32R, tag=f"xT{c}", name=f"xT{c}") for c in range(2)]
            for ib in range(NB):
                xs = wpool.tile([128, DM], F32R, tag="xstage")
                nc.sync.dma_start(xs[:], r(x_d[ib * 128:(ib + 1) * 128, :]))
                for c in range(2):
                    tp = mmp.tile([128, 128], F32R, tag="mm")
                    nc.tensor.transpose(tp[:], xs[:, c * 128:(c + 1) * 128], ident[:])
                    nc.vector.tensor_copy(xT[c][:, ib * 128:(ib + 1) * 128], tp[:])

            # ---- q^T, k^T (col-major) ----
            qT = [bpool.tile([128, N], F32R, tag=f"qT{b}", name=f"qT{b}") for b in range(2)]
            kT = [bpool.tile([128, N], F32R, tag=f"kT{b}", name=f"kT{b}") for b in range(2)]
            for w_sb, b_c, dstT in ((wq_sb, bq_c, qT), (wk_sb, bk_c, kT)):
                for blk in range(2):
                    for it in range(IT):
                        pt = mmp.tile([128, 512], F32, tag="mm")
                        for c in range(2):
                            nc.tensor.matmul(
                                pt[:],
                                w_sb[c][:, blk * 128:(blk + 1) * 128],
                                xT[c][:, it * 512:(it + 1) * 512],
                                start=(c == 0),
                                stop=(c == 1),
                            )
                        nc.vector.tensor_scalar_add(
                            dstT[blk][:, it * 512:(it + 1) * 512], pt[:], b_c[blk][:]
                        )

            # ---- v (row-major, with ones col per head) ----
            # v_sb[jb]: [128, 4*65]; head h data at cols 65h..65h+63, ones at 65h+64
            v_sb = [bpool.tile([128, 4 * 65], F32R, tag=f"v{jb}", name=f"v{jb}") for jb in range(NB)]
            for jb in range(NB):
                v4 = v_sb[jb][:].rearrange("p (h e) -> p h e", e=65)
                nc.vector.tensor_copy(
                    v4[:, :, 64:65],
                    ones_col4[:].rearrange("p (h e) -> p h e", e=1),
                )
                pt = mmp.tile([128, DM], F32, tag="mm")
                for c in range(2):
                    nc.tensor.matmul(
                        pt[:],
                        xT[c][:, jb * 128:(jb + 1) * 128],
                        wv_sb[c][:],
                        start=(c == 0),
                        stop=(c == 1),
                    )
                nc.vector.tensor_tensor(
                    v4[:, :, 0:64],
                    pt[:].rearrange("p (h e) -> p h e", e=64),
                    bv_bc[:].rearrange("p (h e) -> p h e", e=64),
                    op=mybir.AluOpType.add,
                )

            # ---- attention per head ----
            ectx = [bpool.tile([128, N], F32R, tag=f"ectx{b}", name=f"ectx{b}") for b in range(2)]
            for h in range(H):
                qh = qT[h // 2][64 * (h % 2):64 * (h % 2) + 64, :]
                kh = kT[h // 2][64 * (h % 2):64 * (h % 2) + 64, :]
                ctx_ps = ctxp.tile([65, N], F32, tag="ctx")
                for jb in range(NB):
                    it0 = jb // 4
                    for it in range(it0, IT):
                        sc = mmp.tile([128, 512], F32, tag="mm")
                        nc.tensor.matmul(
                            sc[:],
                            kh[:, jb * 128:(jb + 1) * 128],
                            qh[:, it * 512:(it + 1) * 512],
                            start=True,
                            stop=True,
                        )
                        e = wpool.tile([128, 512], F32R, tag="e")
                        nc.scalar.activation(
                            e[:], sc[:], mybir.ActivationFunctionType.Exp,
                            scale=float(1.0 / np.sqrt(HD)),
                        )
                        if it == it0:
                            nc.gpsimd.affine_select(
                                e[:], e[:],
                                pattern=[[1, 512]],
                                compare_op=mybir.AluOpType.is_ge,
                                fill=0.0,
                                base=-(128 * (jb % 4)),
                                channel_multiplier=-1,
                            )
                        nc.tensor.matmul(
                            ctx_ps[0:65, it * 512:(it + 1) * 512],
                            v_sb[jb][:, 65 * h:65 * h + 65],
                            e[:],
                            start=(jb == 0),
                            stop=(jb == 4 * it + 3),
                            skip_group_check=True,
                        )
                # normalize: recip of sums row, broadcast via K=1 matmul
                recip = wpool.tile([1, N], F32R, tag="recip")
                with nc.allow_low_precision(reason="f32r rounding intentional"):
                    nc.vector.reciprocal(recip[:], ctx_ps[64:65, :])
                dst = ectx[h // 2][64 * (h % 2):64 * (h % 2) + 64, :]
                for it in range(IT):
                    bc = mmp.tile([64, 512], F32, tag="mm")
                    nc.tensor.matmul(
                        bc[:], ones_r[:, 0:64], recip[:, it * 512:(it + 1) * 512],
                        start=True, stop=True,
                    )
                    bcs = wpool.tile([64, 512], F32, tag="bcs")
                    nc.vector.tensor_copy(bcs[:], bc[:])
                    nc.vector.tensor_tensor(
                        dst[:, it * 512:(it + 1) * 512],
                        ctx_ps[0:64, it * 512:(it + 1) * 512],
                        bcs[:],
                        op=mybir.AluOpType.mult,
                    )

            # ---- messages^T = Wo.T @ ectx (+bo) ----
            m_sb = [bpool.tile([128, N], F32R, tag=f"m{b}", name=f"m{b}") for b in range(2)]
            for blk in range(2):
                for it in range(IT):
                    pt = mmp.tile([128, 512], F32, tag="mm")
                    for c in range(2):
                        nc.tensor.matmul(
                            pt[:],
                            wo_sb[c][:, blk * 128:(blk + 1) * 128],
                            ectx[c][:, it * 512:(it + 1) * 512],
                            start=(c == 0),
                            stop=(c == 1),
                        )
                    nc.vector.tensor_scalar_add(
                        m_sb[blk][:, it * 512:(it + 1) * 512], pt[:], bo_c[blk][:]
                    )

            # ---- u^T = relu(Wu.T @ [x^T; m^T] + bu) ----
            uT = [bpool.tile([128, N], F32R, tag=f"uT{b}", name=f"uT{b}") for b in range(2)]
            for blk in range(2):
                for it in range(IT):
                    pt = mmp.tile([128, 512], F32, tag="mm")
                    for c in range(4):
                        rhs = xT[c] if c < 2 else m_sb[c - 2]
                        nc.tensor.matmul(
                            pt[:],
                            wu_sb[c][:, blk * 128:(blk + 1) * 128],
                            rhs[:, it * 512:(it + 1) * 512],
                            start=(c == 0),
                            stop=(c == 3),
                        )
                    nc.scalar.activation(
                        uT[blk][:, it * 512:(it + 1) * 512], pt[:],
                        mybir.ActivationFunctionType.Relu,
                        bias=bu_c[blk][:],
                    )

            # ---- transpose u^T -> u, DMA out ----
            for ib in range(NB):
                ostage = wpool.tile([128, DM], F32, tag="ostage")
                for blk in range(2):
                    tp = mmp.tile([128, 128], F32R, tag="mm")
                    nc.tensor.transpose(
                        tp[:], uT[blk][:, ib * 128:(ib + 1) * 128], ident[:]
                    )
                    nc.vector.tensor_copy(ostage[:, blk * 128:(blk + 1) * 128], tp[:])
                nc.sync.dma_start(out_d[ib * 128:(ib + 1) * 128, :], ostage[:])

    nc.compile()
    return nc


_STATE = {}


def _get_runner():
    if "run" in _STATE:
        return _STATE["run"]
    import jax
    from concourse.bass2jax import (
        _bass_exec_p,
        install_neuronx_cc_hook,
        partition_id_tensor,
    )
    from jax.sharding import Mesh, PartitionSpec
    from jax.experimental.shard_map import shard_map

    nc = build_program()
    install_neuronx_cc_hook()
    partition_name = nc.partition_id_tensor.name if nc.partition_id_tensor else None
    in_names, out_names, out_avals, zero_outs = [], [], [], []
    for alloc in nc.m.functions[0].allocations:
        if not isinstance(alloc, mybir.MemoryLocationSet):
            continue
        name = alloc.memorylocations[0].name
        if alloc.kind == "ExternalInput":
            if name != partition_name:
                in_names.append(name)
        elif alloc.kind == "ExternalOutput":
            shape = tuple(alloc.tensor_shape)
            dtype = mybir.dt.np(alloc.dtype)
            out_names.append(name)
            out_avals.append(jax.core.ShapedArray(shape, dtype))
            zero_outs.append(np.zeros(shape, dtype))
    n_params = len(in_names)
    all_in = in_names + out_names + ([partition_name] if partition_name else [])

    def _body(*args):
        operands = list(args)
        if partition_name is not None:
            operands.append(partition_id_tensor())
        return tuple(
            _bass_exec_p.bind(
                *operands,
                out_avals=tuple(out_avals),
                in_names=tuple(all_in),
                out_names=tuple(out_names),
                lowering_input_output_aliases=(),
                sim_require_finite=True,
                sim_require_nnan=True,
                nc=nc,
            )
        )

    devices = jax.devices()[:B]
    mesh = Mesh(np.asarray(devices), ("core",))
    specs = (PartitionSpec("core"),) * (n_params + len(out_names))
    jitted = jax.jit(
        shard_map(
            _body, mesh=mesh, in_specs=specs,
            out_specs=(PartitionSpec("core"),) * len(out_names), check_rep=False,
        ),
        keep_unused=True,
    )

    def run(in_maps):
        import jax as _jax

        concat_in = [
            np.concatenate([np.asarray(m[nm]) for m in in_maps], axis=0)
            for nm in in_names
        ]
        concat_zero = [
            np.zeros((B * z.shape[0], *z.shape[1:]), z.dtype) for z in zero_outs
        ]
        outs = jitted(*concat_in, *concat_zero)
        _jax.block_until_ready(outs)
        res = []
        o = np.asarray(outs[out_names.index("out")])
        per = o.shape[0] // B
        for c in range(B):
            res.append(o[c * per:(c + 1) * per])
        return res

    _STATE["run"] = run
    return run


def make_in_maps(node_features, Wq, bq, Wk, bk, Wv, bv, Wo, bo, Wu, bu):
    in_maps = []
    for c in range(B):
        in_maps.append(
            {
                "x": np.ascontiguousarray(node_features[c], dtype=np.float32),
                "wq": np.asarray(Wq, np.float32),
                "wk": np.asarray(Wk, np.float32),
                "wv": np.asarray(Wv, np.float32),
                "wo": np.asarray(Wo, np.float32),
                "wu": np.asarray(Wu, np.float32),
                "bq": np.asarray(bq, np.float32),
                "bk": np.asarray(bk, np.float32),
                "bv": np.asarray(bv, np.float32),
                "bo": np.asarray(bo, np.float32),
                "bu": np.asarray(bu, np.float32),
            }
        )
    return in_maps


def kernel(
    node_features, causal_mask, Wq, bq, Wk, bk, Wv, bv, Wo, bo, Wu, bu
):
    """Full-input entry point: shards batch across 8 cores internally."""
    del causal_mask  # guaranteed tril(ones); mask generated on-chip
    run = _get_runner()
    in_maps = make_in_maps(node_features, Wq, bq, Wk, bk, Wv, bv, Wo, bo, Wu, bu)
    outs = run(in_maps)
    return np.stack(outs, axis=0)


# revision 8
# speedup vs baseline: 1.2261x; 1.2261x over previous
"""Trainium2 Bass kernel for CausalMessagePassing (B=8, N=2048, D=256, H=4).

Strategy: data-parallel across 8 NeuronCores, one graph per core.
Per-core dataflow is column-major ("transposed spine"):
  x^T -> q^T,k^T (col-major), v (row-major with ones column for softmax sums)
  scores^T[j,i] = k^T.T @ q^T per head (f32r matmuls, 1 cyc/row)
  e = exp(scores * 1/sqrt(hd)) with causal mask applied on-chip via
  affine_select (the [N,N] mask input is tril(ones) by construction and is
  never DMA'd).
  ctx'^T[65,i] = v'.T @ e^T accumulated over j-blocks; row 64 = softmax sums.
  normalize via K=1 broadcast matmul of 1/sums, fused into PSUM eviction.
  messages^T = Wo.T @ ectx^T (+bo), u^T = relu(Wu.T @ [x^T; m^T] + bu),
  PE-transpose u^T -> u, DMA out.
"""
import sys

sys.path.insert(0, "/opt/trn_rl_repo")

import numpy as np

import concourse.bass as bass  # noqa: F401
import concourse.mybir as mybir
import concourse.tile as tile
from concourse import bacc
from concourse.masks import make_identity

B, N, DM, H = 8, 2048, 256, 4
HD = DM // H  # 64
NB = N // 128  # 16 j-blocks
IT = N // 512  # 4 i-tiles
F32 = mybir.dt.float32
F32R = mybir.dt.float32r


def build_program():
    nc = bacc.Bacc("TRN2", target_bir_lowering=False, debug=False)
    x_d = nc.dram_tensor("x", [N, DM], F32, kind="ExternalInput").ap()
    wq_d = nc.dram_tensor("wq", [DM, DM], F32, kind="ExternalInput").ap()
    wk_d = nc.dram_tensor("wk", [DM, DM], F32, kind="ExternalInput").ap()
    wv_d = nc.dram_tensor("wv", [DM, DM], F32, kind="ExternalInput").ap()
    wo_d = nc.dram_tensor("wo", [DM, DM], F32, kind="ExternalInput").ap()
    wu_d = nc.dram_tensor("wu", [2 * DM, DM], F32, kind="ExternalInput").ap()
    bq_d = nc.dram_tensor("bq", [DM], F32, kind="ExternalInput").ap()
    bk_d = nc.dram_tensor("bk", [DM], F32, kind="ExternalInput").ap()
    bv_d = nc.dram_tensor("bv", [DM], F32, kind="ExternalInput").ap()
    bo_d = nc.dram_tensor("bo", [DM], F32, kind="ExternalInput").ap()
    bu_d = nc.dram_tensor("bu", [DM], F32, kind="ExternalInput").ap()
    out_d = nc.dram_tensor("out", [N, DM], F32, kind="ExternalOutput").ap()

    def r(ap):
        return ap.bitcast(F32R)

    with tile.TileContext(nc) as tc:
        with (
            tc.tile_pool(name="const", bufs=1) as cpool,
            tc.tile_pool(name="big", bufs=1) as bpool,
            tc.tile_pool(name="work", bufs=3) as wpool,
            tc.tile_pool(name="mm", bufs=4, space="PSUM") as mmp,
            tc.tile_pool(name="ctxp", bufs=1, space="PSUM") as ctxp,
        )            :
            # ---- constants / weights (batched DMAs) ----
            ident = cpool.tile([128, 128], F32R, tag="ident")
            ident_f = cpool.tile([128, 128], F32, tag="identf")
            make_identity(nc, ident_f[:])
            nc.vector.tensor_copy(ident[:], ident_f[:])
            # each W loaded as one DMA: [128, 2*DM], chunk c at cols [c*DM, (c+1)*DM)
            wq_a = cpool.tile([128, 2 * DM], F32R, tag="wqa")
            wk_a = cpool.tile([128, 2 * DM], F32R, tag="wka")
            wv_a = cpool.tile([128, 2 * DM], F32R, tag="wva")
            wo_a = cpool.tile([128, 2 * DM], F32R, tag="woa")
            wu_a = cpool.tile([128, 4 * DM], F32R, tag="wua")
            for t_sb, t_d in ((wq_a, wq_d), (wk_a, wk_d), (wv_a, wv_d), (wo_a, wo_d), (wu_a, wu_d)):
                nc.sync.dma_start(
                    t_sb[:].rearrange("p (c d) -> p c d", d=DM),
                    r(t_d.rearrange("(c p) d -> p c d", p=128)),
                )
            wq_sb = [wq_a[:, c * DM:(c + 1) * DM] for c in range(2)]
            wk_sb = [wk_a[:, c * DM:(c + 1) * DM] for c in range(2)]
            wv_sb = [wv_a[:, c * DM:(c + 1) * DM] for c in range(2)]
            wo_sb = [wo_a[:, c * DM:(c + 1) * DM] for c in range(2)]
            wu_sb = [wu_a[:, c * DM:(c + 1) * DM] for c in range(4)]
            bq_a = cpool.tile([128, 2], F32, tag="bqa")
            bk_a = cpool.tile([128, 2], F32, tag="bka")
            bo_a = cpool.tile([128, 2], F32, tag="boa")
            bu_a = cpool.tile([128, 2], F32, tag="bua")
            for t_sb, t_d in ((bq_a, bq_d), (bk_a, bk_d), (bo_a, bo_d), (bu_a, bu_d)):
                nc.sync.dma_start(t_sb[:], t_d.rearrange("(c p) -> p c", p=128))
            bq_c = [bq_a[:, b:b + 1] for b in range(2)]
            bk_c = [bk_a[:, b:b + 1] for b in range(2)]
            bo_c = [bo_a[:, b:b + 1] for b in range(2)]
            bu_c = [bu_a[:, b:b + 1] for b in range(2)]
            # bv broadcast tile [128, 256] (f32; only used by DVE add)
            bv_row = cpool.tile([1, DM], F32, tag="bvrow")
            nc.sync.dma_start(bv_row[:], bv_d.rearrange("(b a) -> b a", b=1))
            ones1 = cpool.tile([1, 128], F32, tag="ones1")
            nc.gpsimd.memset(ones1[:], 1.0)
            bv_bc = cpool.tile([128, DM], F32, tag="bvbc")
            pt = mmp.tile([128, DM], F32, tag="mm")
            nc.tensor.matmul(pt[:], ones1[:], bv_row[:], start=True, stop=True)
            nc.vector.tensor_copy(bv_bc[:], pt[:])
            ones_r = cpool.tile([1, 64], F32R, tag="onesr")
            ones_rf = cpool.tile([1, 64], F32, tag="onesrf")
            nc.gpsimd.memset(ones_rf[:], 1.0)
            nc.vector.tensor_copy(ones_r[:], ones_rf[:])
            ones_col4 = cpool.tile([128, 4], F32, tag="onescol4")
            nc.gpsimd.memset(ones_col4[:], 1.0)

            # ---- x load (one DMA) + transpose to x^T ----
            stage = cpool.tile([128, NB * DM], F32R, tag="stage")
            xs_all = stage
            nc.sync.dma_start(
                xs_all[:].rearrange("p (t d) -> p t d", d=DM),
                r(x_d.rearrange("(t p) d -> p t d", p=128)),
            )
            xT = [bpool.tile([128, N], F32R, tag=f"xT{c}", name=f"xT{c}") for c in range(2)]
            for ib in range(NB):
                for c in range(2):
                    tp = mmp.tile([128, 128], F32R, tag="mm")
                    nc.tensor.transpose(
                        tp[:], xs_all[:, ib * DM + c * 128:ib * DM + (c + 1) * 128], ident[:]
                    )
                    nc.vector.tensor_copy(xT[c][:, ib * 128:(ib + 1) * 128], tp[:])

            # ---- q^T, k^T (col-major) ----
            qT = [bpool.tile([128, N], F32R, tag=f"qT{b}", name=f"qT{b}") for b in range(2)]
            kT = [bpool.tile([128, N], F32R, tag=f"kT{b}", name=f"kT{b}") for b in range(2)]
            for w_sb, b_c, dstT in ((wq_sb, bq_c, qT), (wk_sb, bk_c, kT)):
                for blk in range(2):
                    for it in range(IT):
                        pt = mmp.tile([128, 512], F32, tag="mm")
                        for c in range(2):
                            nc.tensor.matmul(
                                pt[:],
                                w_sb[c][:, blk * 128:(blk + 1) * 128],
                                xT[c][:, it * 512:(it + 1) * 512],
                                start=(c == 0),
                                stop=(c == 1),
                            )
                        nc.vector.tensor_scalar_add(
                            dstT[blk][:, it * 512:(it + 1) * 512], pt[:], b_c[blk][:]
                        )

            # ---- v (row-major, with ones col per head) ----
            # v_sb[jb]: [128, 4*65]; head h data at cols 65h..65h+63, ones at 65h+64
            v_sb = [bpool.tile([128, 4 * 65], F32R, tag=f"v{jb}", name=f"v{jb}") for jb in range(NB)]
            for jb in range(NB):
                v4 = v_sb[jb][:].rearrange("p (h e) -> p h e", e=65)
                nc.vector.tensor_copy(
                    v4[:, :, 64:65],
                    ones_col4[:].rearrange("p (h e) -> p h e", e=1),
                )
                pt = mmp.tile([128, DM], F32, tag="mm")
                for c in range(2):
                    nc.tensor.matmul(
                        pt[:],
                        xT[c][:, jb * 128:(jb + 1) * 128],
                        wv_sb[c][:],
                        start=(c == 0),
                        stop=(c == 1),
                    )
                nc.vector.tensor_tensor(
                    v4[:, :, 0:64],
                    pt[:].rearrange("p (h e) -> p h e", e=64),
                    bv_bc[:].rearrange("p (h e) -> p h e", e=64),
                    op=mybir.AluOpType.add,
                )

            # ---- attention per head ----
            ectx = [bpool.tile([128, N], F32R, tag=f"ectx{b}", name=f"ectx{b}") for b in range(2)]
            for h in range(H):
                qh = qT[h // 2][64 * (h % 2):64 * (h % 2) + 64, :]
                kh = kT[h // 2][64 * (h % 2):64 * (h % 2) + 64, :]
                ctx_ps = ctxp.tile([65, N], F32, tag="ctx")
                for jb in range(NB):
                    it0 = jb // 4
                    for it in range(it0, IT):
                        # partial-width diagonal tiles (min 256 cols to stay
                        # in f32r 1-cyc/row regime)
                        if it == it0:
                            cst = it * 512 + min(128 * (jb % 4), 256)
                        else:
                            cst = it * 512
                        cend = (it + 1) * 512
                        w = cend - cst
                        sc = mmp.tile([128, 512], F32, tag="mm")
                        nc.tensor.matmul(
                            sc[:, 0:w],
                            kh[:, jb * 128:(jb + 1) * 128],
                            qh[:, cst:cend],
                            start=True,
                            stop=True,
                        )
                        e = wpool.tile([128, 512], F32R, tag="e", bufs=6)
                        nc.scalar.activation(
                            e[:, 0:w], sc[:, 0:w], mybir.ActivationFunctionType.Exp,
                            scale=float(1.0 / np.sqrt(HD)),
                        )
                        if it == it0:
                            nc.gpsimd.affine_select(
                                e[:, 0:w], e[:, 0:w],
                                pattern=[[1, w]],
                                compare_op=mybir.AluOpType.is_ge,
                                fill=0.0,
                                base=cst - 128 * jb,
                                channel_multiplier=-1,
                            )
                        nc.tensor.matmul(
                            ctx_ps[0:65, cst:cend],
                            v_sb[jb][:, 65 * h:65 * h + 65],
                            e[:, 0:w],
                            start=(jb == 0),
                            stop=(jb == 4 * it + 3),
                            skip_group_check=True,
                        )
                # normalize: recip of sums row, broadcast via K=1 matmul
                recip = wpool.tile([1, N], F32R, tag="recip", bufs=1)
                with nc.allow_low_precision(reason="f32r rounding intentional"):
                    nc.vector.reciprocal(recip[:], ctx_ps[64:65, :])
                dst = ectx[h // 2][64 * (h % 2):64 * (h % 2) + 64, :]
                for it in range(IT):
                    bc = mmp.tile([64, 512], F32, tag="mm")
                    nc.tensor.matmul(
                        bc[:], ones_r[:, 0:64], recip[:, it * 512:(it + 1) * 512],
                        start=True, stop=True,
                    )
                    bcs = wpool.tile([64, 512], F32, tag="bcs", bufs=2)
                    nc.vector.tensor_copy(bcs[:], bc[:])
                    nc.vector.tensor_tensor(
                        dst[:, it * 512:(it + 1) * 512],
                        ctx_ps[0:64, it * 512:(it + 1) * 512],
                        bcs[:],
                        op=mybir.AluOpType.mult,
                    )

            # ---- messages^T = Wo.T @ ectx (+bo) ----
            m_sb = [bpool.tile([128, N], F32R, tag=f"m{b}", name=f"m{b}") for b in range(2)]
            for blk in range(2):
                for it in range(IT):
                    pt = mmp.tile([128, 512], F32, tag="mm")
                    for c in range(2):
                        nc.tensor.matmul(
                            pt[:],
                            wo_sb[c][:, blk * 128:(blk + 1) * 128],
                            ectx[c][:, it * 512:(it + 1) * 512],
                            start=(c == 0),
                            stop=(c == 1),
                        )
                    nc.vector.tensor_scalar_add(
                        m_sb[blk][:, it * 512:(it + 1) * 512], pt[:], bo_c[blk][:]
                    )

            # ---- u^T = relu(Wu.T @ [x^T; m^T] + bu) ----
            uT = [bpool.tile([128, N], F32R, tag=f"uT{b}", name=f"uT{b}") for b in range(2)]
            for blk in range(2):
                for it in range(IT):
                    pt = mmp.tile([128, 512], F32, tag="mm")
                    for c in range(4):
                        rhs = xT[c] if c < 2 else m_sb[c - 2]
                        nc.tensor.matmul(
                            pt[:],
                            wu_sb[c][:, blk * 128:(blk + 1) * 128],
                            rhs[:, it * 512:(it + 1) * 512],
                            start=(c == 0),
                            stop=(c == 3),
                        )
                    nc.scalar.activation(
                        uT[blk][:, it * 512:(it + 1) * 512], pt[:],
                        mybir.ActivationFunctionType.Relu,
                        bias=bu_c[blk][:],
                    )

            # ---- transpose u^T -> u, DMA out (4 batched DMAs) ----
            ostage = stage
            out_r = r(out_d.rearrange("(t p) d -> p t d", p=128))
            for ib in range(NB):
                for blk in range(2):
                    tp = mmp.tile([128, 128], F32R, tag="mm")
                    nc.tensor.transpose(
                        tp[:], uT[blk][:, ib * 128:(ib + 1) * 128], ident[:]
                    )
                    nc.vector.tensor_copy(
                        ostage[:, ib * DM + blk * 128:ib * DM + (blk + 1) * 128], tp[:]
                    )
                if ib % 4 == 3:
                    nc.sync.dma_start(
                        out_r[:, ib - 3:ib + 1, :],
                        ostage[:, (ib - 3) * DM:(ib + 1) * DM].rearrange(
                            "p (t d) -> p t d", d=DM
                        ),
                    )

    nc.compile()
    return nc


_STATE = {}


def _get_runner():
    if "run" in _STATE:
        return _STATE["run"]
    import jax
    from concourse.bass2jax import (
        _bass_exec_p,
        install_neuronx_cc_hook,
        partition_id_tensor,
    )
    from jax.sharding import Mesh, PartitionSpec
    from jax.experimental.shard_map import shard_map

    nc = build_program()
    install_neuronx_cc_hook()
    partition_name = nc.partition_id_tensor.name if nc.partition_id_tensor else None
    in_names, out_names, out_avals, zero_outs = [], [], [], []
    for alloc in nc.m.functions[0].allocations:
        if not isinstance(alloc, mybir.MemoryLocationSet):
            continue
        name = alloc.memorylocations[0].name
        if alloc.kind == "ExternalInput":
            if name != partition_name:
                in_names.append(name)
        elif alloc.kind == "ExternalOutput":
            shape = tuple(alloc.tensor_shape)
            dtype = mybir.dt.np(alloc.dtype)
            out_names.append(name)
            out_avals.append(jax.core.ShapedArray(shape, dtype))
            zero_outs.append(np.zeros(shape, dtype))
    n_params = len(in_names)
    all_in = in_names + out_names + ([partition_name] if partition_name else [])

    def _body(*args):
        operands = list(args)
        if partition_name is not None:
            operands.append(partition_id_tensor())
        return tuple(
            _bass_exec_p.bind(
                *operands,
                out_avals=tuple(out_avals),
                in_names=tuple(all_in),
                out_names=tuple(out_names),
                lowering_input_output_aliases=(),
                sim_require_finite=True,
                sim_require_nnan=True,
                nc=nc,
            )
        )

    devices = jax.devices()[:B]
    mesh = Mesh(np.asarray(devices), ("core",))
    specs = (PartitionSpec("core"),) * (n_params + len(out_names))
    jitted = jax.jit(
        shard_map(
            _body, mesh=mesh, in_specs=specs,
            out_specs=(PartitionSpec("core"),) * len(out_names), check_rep=False,
        ),
        keep_unused=True,
    )

    def run(in_maps):
        import jax as _jax

        concat_in = [
            np.concatenate([np.asarray(m[nm]) for m in in_maps], axis=0)
            for nm in in_names
        ]
        concat_zero = [
            np.zeros((B * z.shape[0], *z.shape[1:]), z.dtype) for z in zero_outs
        ]
        outs = jitted(*concat_in, *concat_zero)
        _jax.block_until_ready(outs)
        res = []
        o = np.asarray(outs[out_names.index("out")])
        per = o.shape[0] // B
        for c in range(B):
            res.append(o[c * per:(c + 1) * per])
        return res

    _STATE["run"] = run
    return run


def make_in_maps(node_features, Wq, bq, Wk, bk, Wv, bv, Wo, bo, Wu, bu):
    in_maps = []
    for c in range(B):
        in_maps.append(
            {
                "x": np.ascontiguousarray(node_features[c], dtype=np.float32),
                "wq": np.asarray(Wq, np.float32),
                "wk": np.asarray(Wk, np.float32),
                "wv": np.asarray(Wv, np.float32),
                "wo": np.asarray(Wo, np.float32),
                "wu": np.asarray(Wu, np.float32),
                "bq": np.asarray(bq, np.float32),
                "bk": np.asarray(bk, np.float32),
                "bv": np.asarray(bv, np.float32),
                "bo": np.asarray(bo, np.float32),
                "bu": np.asarray(bu, np.float32),
            }
        )
    return in_maps


def kernel(
    node_features, causal_mask, Wq, bq, Wk, bk, Wv, bv, Wo, bo, Wu, bu
):
    """Full-input entry point: shards batch across 8 cores internally."""
    del causal_mask  # guaranteed tril(ones); mask generated on-chip
    run = _get_runner()
    in_maps = make_in_maps(node_features, Wq, bq, Wk, bk, Wv, bv, Wo, bo, Wu, bu)
    outs = run(in_maps)
    return np.stack(outs, axis=0)


# revision 12
# speedup vs baseline: 1.3912x; 1.1346x over previous
"""Trainium2 Bass kernel for CausalMessagePassing (B=8, N=2048, D=256, H=4).

Strategy: data-parallel across 8 NeuronCores, one graph per core.
Per-core dataflow is column-major ("transposed spine"):
  x^T -> q^T,k^T (col-major), v (row-major with ones column for softmax sums)
  scores^T[j,i] = k^T.T @ q^T per head (f32r matmuls, 1 cyc/row)
  e = exp(scores * 1/sqrt(hd)) with causal mask applied on-chip via
  affine_select (the [N,N] mask input is tril(ones) by construction and is
  never DMA'd).
  ctx'^T[65,i] = v'.T @ e^T accumulated over j-blocks; row 64 = softmax sums.
  normalize via K=1 broadcast matmul of 1/sums, fused into PSUM eviction.
  messages^T = Wo.T @ ectx^T (+bo), u^T = relu(Wu.T @ [x^T; m^T] + bu),
  PE-transpose u^T -> u, DMA out.
"""
import sys

sys.path.insert(0, "/opt/trn_rl_repo")

import numpy as np

import concourse.bass as bass  # noqa: F401
import concourse.mybir as mybir
import concourse.tile as tile
from concourse import bacc
from concourse.masks import make_identity

B, N, DM, H = 8, 2048, 256, 4
HD = DM // H  # 64
NB = N // 128  # 16 j-blocks
IT = N // 512  # 4 i-tiles
F32 = mybir.dt.float32
F32R = mybir.dt.float32r


def build_program():
    nc = bacc.Bacc("TRN2", target_bir_lowering=False, debug=False)
    x_d = nc.dram_tensor("x", [N, DM], F32, kind="ExternalInput").ap()
    wq_d = nc.dram_tensor("wq", [DM, DM], F32, kind="ExternalInput").ap()
    wk_d = nc.dram_tensor("wk", [DM, DM], F32, kind="ExternalInput").ap()
    wv_d = nc.dram_tensor("wv", [DM, DM], F32, kind="ExternalInput").ap()
    wo_d = nc.dram_tensor("wo", [DM, DM], F32, kind="ExternalInput").ap()
    wu_d = nc.dram_tensor("wu", [2 * DM, DM], F32, kind="ExternalInput").ap()
    bq_d = nc.dram_tensor("bq", [DM], F32, kind="ExternalInput").ap()
    bk_d = nc.dram_tensor("bk", [DM], F32, kind="ExternalInput").ap()
    bv_d = nc.dram_tensor("bv", [DM], F32, kind="ExternalInput").ap()
    bo_d = nc.dram_tensor("bo", [DM], F32, kind="ExternalInput").ap()
    bu_d = nc.dram_tensor("bu", [DM], F32, kind="ExternalInput").ap()
    out_d = nc.dram_tensor("out", [N, DM], F32, kind="ExternalOutput").ap()

    def r(ap):
        return ap.bitcast(F32R)

    with tile.TileContext(nc) as tc:
        with (
            tc.tile_pool(name="const", bufs=1) as cpool,
            tc.tile_pool(name="big", bufs=1) as bpool,
            tc.tile_pool(name="work", bufs=3) as wpool,
            tc.tile_pool(name="mm", bufs=2, space="PSUM") as mmp,
            tc.tile_pool(name="sc", bufs=2, space="PSUM") as scp,
            tc.tile_pool(name="ctxp", bufs=1, space="PSUM") as ctxp,
        )            :
            # ---- constants / weights (batched DMAs) ----
            ident = cpool.tile([128, 128], F32R, tag="ident")
            ident_f = cpool.tile([128, 128], F32, tag="identf")
            make_identity(nc, ident_f[:])
            nc.vector.tensor_copy(ident[:], ident_f[:])
            # each W loaded as one DMA: [128, 2*DM], chunk c at cols [c*DM, (c+1)*DM)
            wq_a = cpool.tile([128, 2 * DM], F32R, tag="wqa")
            wk_a = cpool.tile([128, 2 * DM], F32R, tag="wka")
            wv_a = cpool.tile([128, 2 * DM], F32R, tag="wva")
            wo_a = cpool.tile([128, 2 * DM], F32R, tag="woa")
            wu_a = cpool.tile([128, 4 * DM], F32R, tag="wua")
            for t_sb, t_d in ((wq_a, wq_d), (wk_a, wk_d), (wv_a, wv_d), (wo_a, wo_d), (wu_a, wu_d)):
                nc.sync.dma_start(
                    t_sb[:].rearrange("p (c d) -> p c d", d=DM),
                    r(t_d.rearrange("(c p) d -> p c d", p=128)),
                )
            wq_sb = [wq_a[:, c * DM:(c + 1) * DM] for c in range(2)]
            wk_sb = [wk_a[:, c * DM:(c + 1) * DM] for c in range(2)]
            wv_sb = [wv_a[:, c * DM:(c + 1) * DM] for c in range(2)]
            wo_sb = [wo_a[:, c * DM:(c + 1) * DM] for c in range(2)]
            wu_sb = [wu_a[:, c * DM:(c + 1) * DM] for c in range(4)]
            bq_a = cpool.tile([128, 2], F32, tag="bqa")
            bk_a = cpool.tile([128, 2], F32, tag="bka")
            bo_a = cpool.tile([128, 2], F32, tag="boa")
            bu_a = cpool.tile([128, 2], F32, tag="bua")
            for t_sb, t_d in ((bq_a, bq_d), (bk_a, bk_d), (bo_a, bo_d), (bu_a, bu_d)):
                nc.sync.dma_start(t_sb[:], t_d.rearrange("(c p) -> p c", p=128))
            bq_c = [bq_a[:, b:b + 1] for b in range(2)]
            bk_c = [bk_a[:, b:b + 1] for b in range(2)]
            bo_c = [bo_a[:, b:b + 1] for b in range(2)]
            bu_c = [bu_a[:, b:b + 1] for b in range(2)]
            # bv broadcast tile [128, 256] (f32; only used by DVE add)
            bv_row = cpool.tile([1, DM], F32, tag="bvrow")
            nc.sync.dma_start(bv_row[:], bv_d.rearrange("(b a) -> b a", b=1))
            ones1 = cpool.tile([1, 128], F32, tag="ones1")
            nc.gpsimd.memset(ones1[:], 1.0)
            bv_bc = cpool.tile([128, DM], F32, tag="bvbc")
            pt = mmp.tile([128, DM], F32, tag="mm")
            nc.tensor.matmul(pt[:], ones1[:], bv_row[:], start=True, stop=True)
            nc.vector.tensor_copy(bv_bc[:], pt[:])
            ones_r = cpool.tile([1, 64], F32R, tag="onesr")
            ones_rf = cpool.tile([1, 64], F32, tag="onesrf")
            nc.gpsimd.memset(ones_rf[:], 1.0)
            nc.vector.tensor_copy(ones_r[:], ones_rf[:])
            ones_col4 = cpool.tile([128, 4], F32, tag="onescol4")
            nc.gpsimd.memset(ones_col4[:], 1.0)

            # ---- x load (one DMA) + transpose to x^T ----
            stage = cpool.tile([128, NB * DM], F32R, tag="stage")
            xs_all = stage
            x_r = r(x_d.rearrange("(t p) d -> p t d", p=128))
            for g in range(4):
                nc.sync.dma_start(
                    xs_all[:, g * 4 * DM:(g + 1) * 4 * DM].rearrange(
                        "p (t d) -> p t d", d=DM
                    ),
                    x_r[:, g * 4:(g + 1) * 4, :],
                )
            xT = [bpool.tile([128, N], F32R, tag=f"xT{c}", name=f"xT{c}") for c in range(2)]
            for ib in range(NB):
                for c in range(2):
                    tp = mmp.tile([128, 128], F32R, tag="mm")
                    nc.tensor.transpose(
                        tp[:], xs_all[:, ib * DM + c * 128:ib * DM + (c + 1) * 128], ident[:]
                    )
                    nc.scalar.copy(xT[c][:, ib * 128:(ib + 1) * 128], tp[:])

            # ---- q^T, k^T (col-major) ----
            qT = [bpool.tile([128, N], F32R, tag=f"qT{b}", name=f"qT{b}") for b in range(2)]
            kT = [bpool.tile([128, N], F32R, tag=f"kT{b}", name=f"kT{b}") for b in range(2)]
            for w_sb, b_c, dstT in ((wq_sb, bq_c, qT), (wk_sb, bk_c, kT)):
                for blk in range(2):
                    for it in range(IT):
                        pt = mmp.tile([128, 512], F32, tag="mm")
                        for c in range(2):
                            nc.tensor.matmul(
                                pt[:],
                                w_sb[c][:, blk * 128:(blk + 1) * 128],
                                xT[c][:, it * 512:(it + 1) * 512],
                                start=(c == 0),
                                stop=(c == 1),
                            )
                        nc.vector.tensor_scalar_add(
                            dstT[blk][:, it * 512:(it + 1) * 512], pt[:], b_c[blk][:]
                        )

            # ---- v (row-major, with ones col per head) ----
            # v_sb[jb]: [128, 4*65]; head h data at cols 65h..65h+63, ones at 65h+64
            v_sb = [bpool.tile([128, 4 * 65], F32R, tag=f"v{jb}", name=f"v{jb}") for jb in range(NB)]
            for jb in range(NB):
                v4 = v_sb[jb][:].rearrange("p (h e) -> p h e", e=65)
                nc.vector.tensor_copy(
                    v4[:, :, 64:65],
                    ones_col4[:].rearrange("p (h e) -> p h e", e=1),
                )
                pt = mmp.tile([128, DM], F32, tag="mm")
                for c in range(2):
                    nc.tensor.matmul(
                        pt[:],
                        xT[c][:, jb * 128:(jb + 1) * 128],
                        wv_sb[c][:],
                        start=(c == 0),
                        stop=(c == 1),
                    )
                nc.vector.tensor_tensor(
                    v4[:, :, 0:64],
                    pt[:].rearrange("p (h e) -> p h e", e=64),
                    bv_bc[:].rearrange("p (h e) -> p h e", e=64),
                    op=mybir.AluOpType.add,
                )

            # ---- attention per head ----
            ectx = [bpool.tile([128, N], F32R, tag=f"ectx{b}", name=f"ectx{b}") for b in range(2)]
            for h in range(H):
                qh = qT[h // 2][64 * (h % 2):64 * (h % 2) + 64, :]
                kh = kT[h // 2][64 * (h % 2):64 * (h % 2) + 64, :]
                dst = ectx[h // 2][64 * (h % 2):64 * (h % 2) + 64, :]
                for half in range(2):
                    hstart, hend = 1024 * half, 1024 * (half + 1)
                    jb_max = 8 * (half + 1)
                    ctx_ps = ctxp.tile([65, 1024], F32, tag="ctx")
                    for jb in range(jb_max):
                        it0 = jb // 4

                        def col_start(it):
                            # partial-width diagonal tiles (min 256 wide to
                            # stay in the f32r 1-cyc/row regime)
                            if it == it0:
                                return it * 512 + min(128 * (jb % 4), 256)
                            return it * 512

                        its = [t for t in range(max(it0, 2 * half), 2 * half + 2)]
                        if not its:
                            continue
                        cst0 = col_start(its[0])
                        sc = scp.tile([128, 1024], F32, tag="sc")
                        for it in its:
                            cst, cend = col_start(it), (it + 1) * 512
                            nc.tensor.matmul(
                                sc[:, cst - hstart:cend - hstart],
                                kh[:, jb * 128:(jb + 1) * 128],
                                qh[:, cst:cend],
                                start=True,
                                stop=True,
                            )
                        wtot = hend - cst0
                        e = wpool.tile([128, 1024], F32R, tag="e", bufs=4)
                        nc.scalar.activation(
                            e[:, 0:wtot], sc[:, cst0 - hstart:1024],
                            mybir.ActivationFunctionType.Exp,
                            scale=float(1.0 / np.sqrt(HD)),
                        )
                        if its[0] == it0:
                            # only cols where some row can be invalid:
                            # f < p - base, base in {0, -128}
                            wd = 128 if (jb % 4) < 3 else 256
                            nc.gpsimd.affine_select(
                                e[:, 0:wd], e[:, 0:wd],
                                pattern=[[1, wd]],
                                compare_op=mybir.AluOpType.is_ge,
                                fill=0.0,
                                base=cst0 - 128 * jb,
                                channel_multiplier=-1,
                            )
                        for it in its:
                            cst, cend = col_start(it), (it + 1) * 512
                            nc.tensor.matmul(
                                ctx_ps[0:65, cst - hstart:cend - hstart],
                                v_sb[jb][:, 65 * h:65 * h + 65],
                                e[:, cst - cst0:cend - cst0],
                                start=(jb == 0),
                                stop=(jb == min(4 * it + 3, jb_max - 1)),
                                skip_group_check=True,
                            )
                    # normalize: recip of sums row, broadcast via K=1 matmul
                    recip = wpool.tile([1, 1024], F32R, tag="recip", bufs=2)
                    with nc.allow_low_precision(reason="f32r rounding intentional"):
                        nc.vector.reciprocal(recip[:], ctx_ps[64:65, :])
                    for itl in range(2):
                        bc = mmp.tile([64, 512], F32, tag="mm")
                        nc.tensor.matmul(
                            bc[:], ones_r[:, 0:64], recip[:, itl * 512:(itl + 1) * 512],
                            start=True, stop=True,
                        )
                        bcs = wpool.tile([64, 512], F32, tag="bcs", bufs=2)
                        nc.vector.tensor_copy(bcs[:], bc[:])
                        nc.vector.tensor_tensor(
                            dst[:, hstart + itl * 512:hstart + (itl + 1) * 512],
                            ctx_ps[0:64, itl * 512:(itl + 1) * 512],
                            bcs[:],
                            op=mybir.AluOpType.mult,
                        )

            # ---- tail: Wo -> Wu -> transpose -> out, interleaved per i-tile ----
            m_sb = [bpool.tile([128, N], F32R, tag=f"m{b}", name=f"m{b}") for b in range(2)]
            uT = [bpool.tile([128, N], F32R, tag=f"uT{b}", name=f"uT{b}") for b in range(2)]
            ostage = stage
            out_r = r(out_d.rearrange("(t p) d -> p t d", p=128))
            for it in range(IT):
                isl = slice(it * 512, (it + 1) * 512)
                for blk in range(2):
                    pt = mmp.tile([128, 512], F32, tag="mm")
                    for c in range(2):
                        nc.tensor.matmul(
                            pt[:],
                            wo_sb[c][:, blk * 128:(blk + 1) * 128],
                            ectx[c][:, isl],
                            start=(c == 0),
                            stop=(c == 1),
                        )
                    nc.vector.tensor_scalar_add(m_sb[blk][:, isl], pt[:], bo_c[blk][:])
                for blk in range(2):
                    pt = mmp.tile([128, 512], F32, tag="mm")
                    for c in range(4):
                        rhs = xT[c] if c < 2 else m_sb[c - 2]
                        nc.tensor.matmul(
                            pt[:],
                            wu_sb[c][:, blk * 128:(blk + 1) * 128],
                            rhs[:, isl],
                            start=(c == 0),
                            stop=(c == 3),
                        )
                    nc.vector.tensor_scalar(
                        uT[blk][:, isl], pt[:], bu_c[blk][:], 0.0,
                        op0=mybir.AluOpType.add, op1=mybir.AluOpType.max,
                    )
                for ib in range(it * 4, (it + 1) * 4):
                    for blk in range(2):
                        tp = scp.tile([128, 128], F32R, tag="sc")
                        nc.tensor.transpose(
                            tp[:], uT[blk][:, ib * 128:(ib + 1) * 128], ident[:]
                        )
                        nc.vector.tensor_copy(
                            ostage[:, ib * DM + blk * 128:ib * DM + (blk + 1) * 128],
                            tp[:],
                        )
                nc.sync.dma_start(
                    out_r[:, it * 4:(it + 1) * 4, :],
                    ostage[:, it * 4 * DM:(it + 1) * 4 * DM].rearrange(
                        "p (t d) -> p t d", d=DM
                    ),
                )

    nc.compile()
    return nc


_STATE = {}


def _get_runner():
    if "run" in _STATE:
        return _STATE["run"]
    import jax
    from concourse.bass2jax import (
        _bass_exec_p,
        install_neuronx_cc_hook,
        partition_id_tensor,
    )
    from jax.sharding import Mesh, PartitionSpec
    from jax.experimental.shard_map import shard_map

    nc = build_program()
    install_neuronx_cc_hook()
    partition_name = nc.partition_id_tensor.name if nc.partition_id_tensor else None
    in_names, out_names, out_avals, zero_outs = [], [], [], []
    for alloc in nc.m.functions[0].allocations:
        if not isinstance(alloc, mybir.MemoryLocationSet):
            continue
        name = alloc.memorylocations[0].name
        if alloc.kind == "ExternalInput":
            if name != partition_name:
                in_names.append(name)
        elif alloc.kind == "ExternalOutput":
            shape = tuple(alloc.tensor_shape)
            dtype = mybir.dt.np(alloc.dtype)
            out_names.append(name)
            out_avals.append(jax.core.ShapedArray(shape, dtype))
            zero_outs.append(np.zeros(shape, dtype))
    n_params = len(in_names)
    all_in = in_names + out_names + ([partition_name] if partition_name else [])

    def _body(*args):
        operands = list(args)
        if partition_name is not None:
            operands.append(partition_id_tensor())
        return tuple(
            _bass_exec_p.bind(
                *operands,
                out_avals=tuple(out_avals),
                in_names=tuple(all_in),
                out_names=tuple(out_names),
                lowering_input_output_aliases=(),
                sim_require_finite=True,
                sim_require_nnan=True,
                nc=nc,
            )
        )

    devices = jax.devices()[:B]
    mesh = Mesh(np.asarray(devices), ("core",))
    specs = (PartitionSpec("core"),) * (n_params + len(out_names))
    jitted = jax.jit(
        shard_map(
            _body, mesh=mesh, in_specs=specs,
            out_specs=(PartitionSpec("core"),) * len(out_names), check_rep=False,
        ),
        keep_unused=True,
    )

    def run(in_maps):
        import jax as _jax

        concat_in = [
            np.concatenate([np.asarray(m[nm]) for m in in_maps], axis=0)
            for nm in in_names
        ]
        concat_zero = [
            np.zeros((B * z.shape[0], *z.shape[1:]), z.dtype) for z in zero_outs
        ]
        outs = jitted(*concat_in, *concat_zero)
        _jax.block_until_ready(outs)
        res = []
        o = np.asarray(outs[out_names.index("out")])
        per = o.shape[0] // B
        for c in range(B):
            res.append(o[c * per:(c + 1) * per])
        return res

    _STATE["run"] = run
    return run


def make_in_maps(node_features, Wq, bq, Wk, bk, Wv, bv, Wo, bo, Wu, bu):
    in_maps = []
    for c in range(B):
        in_maps.append(
            {
                "x": np.ascontiguousarray(node_features[c], dtype=np.float32),
                "wq": np.asarray(Wq, np.float32),
                "wk": np.asarray(Wk, np.float32),
                "wv": np.asarray(Wv, np.float32),
                "wo": np.asarray(Wo, np.float32),
                "wu": np.asarray(Wu, np.float32),
                "bq": np.asarray(bq, np.float32),
                "bk": np.asarray(bk, np.float32),
                "bv": np.asarray(bv, np.float32),
                "bo": np.asarray(bo, np.float32),
                "bu": np.asarray(bu, np.float32),
            }
        )
    return in_maps


def kernel(
    node_features, causal_mask, Wq, bq, Wk, bk, Wv, bv, Wo, bo, Wu, bu
):
    """Full-input entry point: shards batch across 8 cores internally."""
    del causal_mask  # guaranteed tril(ones); mask generated on-chip
    run = _get_runner()
    in_maps = make_in_maps(node_features, Wq, bq, Wk, bk, Wv, bv, Wo, bo, Wu, bu)
    outs = run(in_maps)
    return np.stack(outs, axis=0)


# revision 22
# speedup vs baseline: 1.4278x; 1.0263x over previous
"""Trainium2 Bass kernel for CausalMessagePassing (B=8, N=2048, D=256, H=4).

Strategy: data-parallel across 8 NeuronCores, one graph per core.
Per-core dataflow is column-major ("transposed spine"):
  x^T -> q^T,k^T (col-major), v (row-major with ones column for softmax sums)
  scores^T[j,i] = k^T.T @ q^T per head (f32r matmuls, 1 cyc/row)
  e = exp(scores * 1/sqrt(hd)) with causal mask applied on-chip via
  affine_select (the [N,N] mask input is tril(ones) by construction and is
  never DMA'd).
  ctx'^T[65,i] = v'.T @ e^T accumulated over j-blocks; row 64 = softmax sums.
  normalize via K=1 broadcast matmul of 1/sums, fused into PSUM eviction.
  messages^T = Wo.T @ ectx^T (+bo), u^T = relu(Wu.T @ [x^T; m^T] + bu),
  PE-transpose u^T -> u, DMA out.
"""
import sys

sys.path.insert(0, "/opt/trn_rl_repo")

import numpy as np

import concourse.bass as bass  # noqa: F401
import concourse.mybir as mybir
import concourse.tile as tile
from concourse import bacc
from concourse.masks import make_identity

B, N, DM, H = 8, 2048, 256, 4
HD = DM // H  # 64
NB = N // 128  # 16 j-blocks
IT = N // 512  # 4 i-tiles
F32 = mybir.dt.float32
F32R = mybir.dt.float32r


def build_program():
    nc = bacc.Bacc("TRN2", target_bir_lowering=False, debug=False)
    x_d = nc.dram_tensor("x", [N, DM], F32, kind="ExternalInput").ap()
    wq_d = nc.dram_tensor("wq", [DM, DM], F32, kind="ExternalInput").ap()
    wk_d = nc.dram_tensor("wk", [DM, DM], F32, kind="ExternalInput").ap()
    wv_d = nc.dram_tensor("wv", [DM, DM], F32, kind="ExternalInput").ap()
    wo_d = nc.dram_tensor("wo", [DM, DM], F32, kind="ExternalInput").ap()
    wu_d = nc.dram_tensor("wu", [2 * DM, DM], F32, kind="ExternalInput").ap()
    bq_d = nc.dram_tensor("bq", [DM], F32, kind="ExternalInput").ap()
    bk_d = nc.dram_tensor("bk", [DM], F32, kind="ExternalInput").ap()
    bv_d = nc.dram_tensor("bv", [DM], F32, kind="ExternalInput").ap()
    bo_d = nc.dram_tensor("bo", [DM], F32, kind="ExternalInput").ap()
    bu_d = nc.dram_tensor("bu", [DM], F32, kind="ExternalInput").ap()
    out_d = nc.dram_tensor("out", [N, DM], F32, kind="ExternalOutput").ap()

    def r(ap):
        return ap.bitcast(F32R)

    with tile.TileContext(nc) as tc:
        with (
            tc.tile_pool(name="const", bufs=1) as cpool,
            tc.tile_pool(name="big", bufs=1) as bpool,
            tc.tile_pool(name="work", bufs=3) as wpool,
            tc.tile_pool(name="mm", bufs=2, space="PSUM") as mmp,
            tc.tile_pool(name="sc", bufs=2, space="PSUM") as scp,
            tc.tile_pool(name="ctxp", bufs=1, space="PSUM") as ctxp,
        )            :
            # ---- constants / weights (batched DMAs) ----
            ident = cpool.tile([128, 128], F32R, tag="ident")
            ident_f = cpool.tile([128, 128], F32, tag="identf")
            make_identity(nc, ident_f[:])
            nc.vector.tensor_copy(ident[:], ident_f[:])
            # each W loaded as one DMA: [128, 2*DM], chunk c at cols [c*DM, (c+1)*DM)
            wq_a = cpool.tile([128, 2 * DM], F32R, tag="wqa")
            wk_a = cpool.tile([128, 2 * DM], F32R, tag="wka")
            wv_a = cpool.tile([128, 2 * DM], F32R, tag="wva")
            wo_a = cpool.tile([128, 2 * DM], F32R, tag="woa")
            wu_a = cpool.tile([128, 4 * DM], F32R, tag="wua")
            for t_sb, t_d in ((wq_a, wq_d), (wk_a, wk_d), (wv_a, wv_d), (wo_a, wo_d), (wu_a, wu_d)):
                nc.sync.dma_start(
                    t_sb[:].rearrange("p (c d) -> p c d", d=DM),
                    r(t_d.rearrange("(c p) d -> p c d", p=128)),
                )
            wq_sb = [wq_a[:, c * DM:(c + 1) * DM] for c in range(2)]
            wk_sb = [wk_a[:, c * DM:(c + 1) * DM] for c in range(2)]
            wv_sb = [wv_a[:, c * DM:(c + 1) * DM] for c in range(2)]
            wo_sb = [wo_a[:, c * DM:(c + 1) * DM] for c in range(2)]
            wu_sb = [wu_a[:, c * DM:(c + 1) * DM] for c in range(4)]
            bq_a = cpool.tile([128, 2], F32, tag="bqa")
            bk_a = cpool.tile([128, 2], F32, tag="bka")
            bo_a = cpool.tile([128, 2], F32, tag="boa")
            bu_a = cpool.tile([128, 2], F32, tag="bua")
            for t_sb, t_d in ((bq_a, bq_d), (bk_a, bk_d), (bo_a, bo_d), (bu_a, bu_d)):
                nc.sync.dma_start(t_sb[:], t_d.rearrange("(c p) -> p c", p=128))
            bq_c = [bq_a[:, b:b + 1] for b in range(2)]
            bk_c = [bk_a[:, b:b + 1] for b in range(2)]
            bo_c = [bo_a[:, b:b + 1] for b in range(2)]
            bu_c = [bu_a[:, b:b + 1] for b in range(2)]
            # bv broadcast tile [128, 256] (f32; only used by DVE add)
            bv_row = cpool.tile([1, DM], F32, tag="bvrow")
            nc.sync.dma_start(bv_row[:], bv_d.rearrange("(b a) -> b a", b=1))
            ones1 = cpool.tile([1, 128], F32, tag="ones1")
            nc.gpsimd.memset(ones1[:], 1.0)
            bv_bc = cpool.tile([128, DM], F32, tag="bvbc")
            pt = mmp.tile([128, DM], F32, tag="mm")
            nc.tensor.matmul(pt[:], ones1[:], bv_row[:], start=True, stop=True)
            nc.vector.tensor_copy(bv_bc[:], pt[:])
            ones_r = cpool.tile([1, 64], F32R, tag="onesr")
            ones_rf = cpool.tile([1, 64], F32, tag="onesrf")
            nc.gpsimd.memset(ones_rf[:], 1.0)
            nc.vector.tensor_copy(ones_r[:], ones_rf[:])
            ones_col4 = cpool.tile([128, 4], F32, tag="onescol4")
            nc.gpsimd.memset(ones_col4[:], 1.0)
            # ---- x load (one DMA) + transpose to x^T ----
            stage = cpool.tile([128, NB * DM], F32R, tag="stage")
            xs_all = stage
            x_r = r(x_d.rearrange("(t p) d -> p t d", p=128))
            for g in range(4):
                nc.sync.dma_start(
                    xs_all[:, g * 4 * DM:(g + 1) * 4 * DM].rearrange(
                        "p (t d) -> p t d", d=DM
                    ),
                    x_r[:, g * 4:(g + 1) * 4, :],
                )

            xT = [bpool.tile([128, N], F32R, tag=f"xT{c}", name=f"xT{c}") for c in range(2)]
            for ib in range(NB):
                for c in range(2):
                    tp = mmp.tile([128, 128], F32R, tag="mm")
                    nc.tensor.transpose(
                        tp[:], xs_all[:, ib * DM + c * 128:ib * DM + (c + 1) * 128], ident[:]
                    )
                    nc.vector.tensor_copy(xT[c][:, ib * 128:(ib + 1) * 128], tp[:])

            # ---- q^T, k^T (col-major) ----
            qT = [bpool.tile([128, N], F32R, tag=f"qT{b}", name=f"qT{b}") for b in range(2)]
            kT = [bpool.tile([128, N], F32R, tag=f"kT{b}", name=f"kT{b}") for b in range(2)]

            def emit_qk(blk):
                for w_sb, b_c, dstT in ((wq_sb, bq_c, qT), (wk_sb, bk_c, kT)):
                    for it in range(IT):
                        pt = mmp.tile([128, 512], F32, tag="mm")
                        for c in range(2):
                            nc.tensor.matmul(
                                pt[:],
                                w_sb[c][:, blk * 128:(blk + 1) * 128],
                                xT[c][:, it * 512:(it + 1) * 512],
                                start=(c == 0),
                                stop=(c == 1),
                            )
                        nc.vector.tensor_scalar_add(
                            dstT[blk][:, it * 512:(it + 1) * 512], pt[:], b_c[blk][:]
                        )

            emit_qk(0)

            # ---- v (row-major, with ones col per head) ----
            # v_sb[jb]: [128, 4*65]; head h data at cols 65h..65h+63, ones at 65h+64
            v_sb = [bpool.tile([128, 4 * 65], F32R, tag=f"v{jb}", name=f"v{jb}") for jb in range(NB)]
            for jb in range(NB):
                v4 = v_sb[jb][:].rearrange("p (h e) -> p h e", e=65)
                nc.vector.tensor_copy(
                    v4[:, :, 64:65],
                    ones_col4[:].rearrange("p (h e) -> p h e", e=1),
                )
                pt = mmp.tile([128, DM], F32, tag="mm")
                for c in range(2):
                    nc.tensor.matmul(
                        pt[:],
                        xT[c][:, jb * 128:(jb + 1) * 128],
                        wv_sb[c][:],
                        start=(c == 0),
                        stop=(c == 1),
                    )
                nc.vector.tensor_tensor(
                    v4[:, :, 0:64],
                    pt[:].rearrange("p (h e) -> p h e", e=64),
                    bv_bc[:].rearrange("p (h e) -> p h e", e=64),
                    op=mybir.AluOpType.add,
                )

            # ---- attention per head ----
            ectx = [bpool.tile([128, N], F32R, tag=f"ectx{b}", name=f"ectx{b}") for b in range(2)]
            m_sb = [bpool.tile([128, N], F32R, tag=f"m{b}", name=f"m{b}") for b in range(2)]
            uT = [bpool.tile([128, N], F32R, tag=f"uT{b}", name=f"uT{b}") for b in range(2)]
            ostage = stage
            out_r = r(out_d.rearrange("(t p) d -> p t d", p=128))
            emit_qk(1)

            def attention(h, half):
                qh = qT[h // 2][64 * (h % 2):64 * (h % 2) + 64, :]
                kh = kT[h // 2][64 * (h % 2):64 * (h % 2) + 64, :]
                dst = ectx[h // 2][64 * (h % 2):64 * (h % 2) + 64, :]
                hstart, hend = 1024 * half, 1024 * (half + 1)
                jb_max = 8 * (half + 1)
                ctx_ps = ctxp.tile([65, 1024], F32, tag="ctx", name="ctx")
                for jb in range(jb_max):
                    it0 = jb // 4

                    def col_start(it):
                        # partial-width diagonal tiles (min 256 wide to stay
                        # in the f32r 1-cyc/row regime)
                        if it == it0:
                            return it * 512 + min(128 * (jb % 4), 256)
                        return it * 512

                    its = [t for t in range(max(it0, 2 * half), 2 * half + 2)]
                    cst0 = col_start(its[0])
                    sc = scp.tile([128, 1024], F32, tag="sc", name="sc")
                    for it in its:
                        cst, cend = col_start(it), (it + 1) * 512
                        nc.tensor.matmul(
                            sc[:, cst - hstart:cend - hstart],
                            kh[:, jb * 128:(jb + 1) * 128],
                            qh[:, cst:cend],
                            start=True,
                            stop=True,
                        )
                    wtot = hend - cst0
                    # leading cols that are entirely masked (select fills
                    # them with 0 without reading exp output)
                    skip = 128 if (its[0] == it0 and jb % 4 == 3) else 0
                    e = wpool.tile([128, 1024], F32R, tag="e", bufs=6, name="e")
                    nc.scalar.activation(
                        e[:, skip:wtot], sc[:, cst0 - hstart + skip:1024],
                        mybir.ActivationFunctionType.Exp,
                        scale=float(1.0 / np.sqrt(HD)),
                    )
                    if its[0] == it0:
                        # only cols where some row can be invalid:
                        # f < p - base, base in {0, -128}
                        wd = 128 if (jb % 4) < 3 else 256
                        nc.gpsimd.affine_select(
                            e[:, 0:wd], e[:, 0:wd],
                            pattern=[[1, wd]],
                            compare_op=mybir.AluOpType.is_ge,
                            fill=0.0,
                            base=cst0 - 128 * jb,
                            channel_multiplier=-1,
                        )
                    for it in its:
                        cst, cend = col_start(it), (it + 1) * 512
                        nc.tensor.matmul(
                            ctx_ps[0:65, cst - hstart:cend - hstart],
                            v_sb[jb][:, 65 * h:65 * h + 65],
                            e[:, cst - cst0:cend - cst0],
                            start=(jb == 0),
                            stop=(jb == min(4 * it + 3, jb_max - 1)),
                            skip_group_check=True,
                        )
                # normalize: recip of sums row, broadcast via K=1 matmul
                recip = wpool.tile([1, 1024], F32R, tag="recip", bufs=2, name="recip")
                with nc.allow_low_precision(reason="f32r rounding intentional"):
                    nc.vector.reciprocal(recip[:], ctx_ps[64:65, :])
                for itl in range(2):
                    bc = mmp.tile([64, 512], F32, tag="mm", name="bc")
                    nc.tensor.matmul(
                        bc[:], ones_r[:, 0:64], recip[:, itl * 512:(itl + 1) * 512],
                        start=True, stop=True,
                    )
                    bcs = wpool.tile([64, 512], F32, tag="bcs", bufs=2, name="bcs")
                    nc.vector.tensor_copy(bcs[:], bc[:])
                    nc.vector.tensor_tensor(
                        dst[:, hstart + itl * 512:hstart + (itl + 1) * 512],
                        ctx_ps[0:64, itl * 512:(itl + 1) * 512],
                        bcs[:],
                        op=mybir.AluOpType.mult,
                    )

            def tail(it):
                isl = slice(it * 512, (it + 1) * 512)
                for blk in range(2):
                    pt = mmp.tile([128, 512], F32, tag="mm", name="pt")
                    for c in range(2):
                        nc.tensor.matmul(
                            pt[:],
                            wo_sb[c][:, blk * 128:(blk + 1) * 128],
                            ectx[c][:, isl],
                            start=(c == 0),
                            stop=(c == 1),
                        )
                    nc.vector.tensor_scalar_add(m_sb[blk][:, isl], pt[:], bo_c[blk][:])
                for blk in range(2):
                    pt = mmp.tile([128, 512], F32, tag="mm", name="pt")
                    for c in range(4):
                        rhs = xT[c] if c < 2 else m_sb[c - 2]
                        nc.tensor.matmul(
                            pt[:],
                            wu_sb[c][:, blk * 128:(blk + 1) * 128],
                            rhs[:, isl],
                            start=(c == 0),
                            stop=(c == 3),
                        )
                    nc.vector.tensor_scalar(
                        uT[blk][:, isl], pt[:], bu_c[blk][:], 0.0,
                        op0=mybir.AluOpType.add, op1=mybir.AluOpType.max,
                    )
                for ib in range(it * 4, (it + 1) * 4):
                    for blk in range(2):
                        tp = scp.tile([128, 128], F32R, tag="sc", name="tp")
                        nc.tensor.transpose(
                            tp[:], uT[blk][:, ib * 128:(ib + 1) * 128], ident[:]
                        )
                        nc.scalar.copy(
                            ostage[:, ib * DM + blk * 128:ib * DM + (blk + 1) * 128],
                            tp[:],
                        )
                nc.sync.dma_start(
                    out_r[:, it * 4:(it + 1) * 4, :],
                    ostage[:, it * 4 * DM:(it + 1) * 4 * DM].rearrange(
                        "p (t d) -> p t d", d=DM
                    ),
                )

            for h in range(H):
                for half in range(2):
                    attention(h, half)
            for it in range(IT):
                tail(it)

    nc.compile()
    return nc


_STATE = {}


def _get_runner():
    if "run" in _STATE:
        return _STATE["run"]
    import jax
    from concourse.bass2jax import (
        _bass_exec_p,
        install_neuronx_cc_hook,
        partition_id_tensor,
    )
    from jax.sharding import Mesh, PartitionSpec
    from jax.experimental.shard_map import shard_map

    nc = build_program()
    install_neuronx_cc_hook()
    partition_name = nc.partition_id_tensor.name if nc.partition_id_tensor else None
    in_names, out_names, out_avals, zero_outs = [], [], [], []
    for alloc in nc.m.functions[0].allocations:
        if not isinstance(alloc, mybir.MemoryLocationSet):
            continue
        name = alloc.memorylocations[0].name
        if alloc.kind == "ExternalInput":
            if name != partition_name:
                in_names.append(name)
        elif alloc.kind == "ExternalOutput":
            shape = tuple(alloc.tensor_shape)
            dtype = mybir.dt.np(alloc.dtype)
            out_names.append(name)
            out_avals.append(jax.core.ShapedArray(shape, dtype))
            zero_outs.append(np.zeros(shape, dtype))
    n_params = len(in_names)
    all_in = in_names + out_names + ([partition_name] if partition_name else [])

    def _body(*args):
        operands = list(args)
        if partition_name is not None:
            operands.append(partition_id_tensor())
        return tuple(
            _bass_exec_p.bind(
                *operands,
                out_avals=tuple(out_avals),
                in_names=tuple(all_in),
                out_names=tuple(out_names),
                lowering_input_output_aliases=(),
                sim_require_finite=True,
                sim_require_nnan=True,
                nc=nc,
            )
        )

    devices = jax.devices()[:B]
    mesh = Mesh(np.asarray(devices), ("core",))
    specs = (PartitionSpec("core"),) * (n_params + len(out_names))
    jitted = jax.jit(
        shard_map(
            _body, mesh=mesh, in_specs=specs,
            out_specs=(PartitionSpec("core"),) * len(out_names), check_rep=False,
        ),
        keep_unused=True,
    )

    def run(in_maps):
        import jax as _jax

        concat_in = [
            np.concatenate([np.asarray(m[nm]) for m in in_maps], axis=0)
            for nm in in_names
        ]
        concat_zero = [
            np.zeros((B * z.shape[0], *z.shape[1:]), z.dtype) for z in zero_outs
        ]
        outs = jitted(*concat_in, *concat_zero)
        _jax.block_until_ready(outs)
        res = []
        o = np.asarray(outs[out_names.index("out")])
        per = o.shape[0] // B
        for c in range(B):
            res.append(o[c * per:(c + 1) * per])
        return res

    _STATE["run"] = run
    return run


def make_in_maps(node_features, Wq, bq, Wk, bk, Wv, bv, Wo, bo, Wu, bu):
    in_maps = []
    for c in range(B):
        in_maps.append(
            {
                "x": np.ascontiguousarray(node_features[c], dtype=np.float32),
                "wq": np.asarray(Wq, np.float32),
                "wk": np.asarray(Wk, np.float32),
                "wv": np.asarray(Wv, np.float32),
                "wo": np.asarray(Wo, np.float32),
                "wu": np.asarray(Wu, np.float32),
                "bq": np.asarray(bq, np.float32),
                "bk": np.asarray(bk, np.float32),
                "bv": np.asarray(bv, np.float32),
                "bo": np.asarray(bo, np.float32),
                "bu": np.asarray(bu, np.float32),
            }
        )
    return in_maps


def kernel(
    node_features, causal_mask, Wq, bq, Wk, bk, Wv, bv, Wo, bo, Wu, bu
):
    """Full-input entry point: shards batch across 8 cores internally."""
    del causal_mask  # guaranteed tril(ones); mask generated on-chip
    run = _get_runner()
    in_maps = make_in_maps(node_features, Wq, bq, Wk, bk, Wv, bv, Wo, bo, Wu, bu)
    outs = run(in_maps)
    return np.stack(outs, axis=0)


# revision 23
# speedup vs baseline: 1.4752x; 1.0332x over previous
"""Trainium2 Bass kernel for CausalMessagePassing (B=8, N=2048, D=256, H=4).

Strategy: data-parallel across 8 NeuronCores, one graph per core.
Per-core dataflow is column-major ("transposed spine"):
  x^T -> q^T,k^T (col-major), v (row-major with ones column for softmax sums)
  scores^T[j,i] = k^T.T @ q^T per head (f32r matmuls, 1 cyc/row)
  e = exp(scores * 1/sqrt(hd)) with causal mask applied on-chip via
  affine_select (the [N,N] mask input is tril(ones) by construction and is
  never DMA'd).
  ctx'^T[65,i] = v'.T @ e^T accumulated over j-blocks; row 64 = softmax sums.
  normalize via K=1 broadcast matmul of 1/sums, fused into PSUM eviction.
  messages^T = Wo.T @ ectx^T (+bo), u^T = relu(Wu.T @ [x^T; m^T] + bu),
  PE-transpose u^T -> u, DMA out.
"""
import sys

sys.path.insert(0, "/opt/trn_rl_repo")

import numpy as np

import concourse.bass as bass  # noqa: F401
import concourse.mybir as mybir
import concourse.tile as tile
from concourse import bacc
from concourse.masks import make_identity

B, N, DM, H = 8, 2048, 256, 4
HD = DM // H  # 64
NB = N // 128  # 16 j-blocks
IT = N // 512  # 4 i-tiles
F32 = mybir.dt.float32
F32R = mybir.dt.float32r


def build_program():
    nc = bacc.Bacc("TRN2", target_bir_lowering=False, debug=False)
    x_d = nc.dram_tensor("x", [N, DM], F32, kind="ExternalInput").ap()
    wq_d = nc.dram_tensor("wq", [DM, DM], F32, kind="ExternalInput").ap()
    wk_d = nc.dram_tensor("wk", [DM, DM], F32, kind="ExternalInput").ap()
    wv_d = nc.dram_tensor("wv", [DM, DM], F32, kind="ExternalInput").ap()
    wo_d = nc.dram_tensor("wo", [DM, DM], F32, kind="ExternalInput").ap()
    wu_d = nc.dram_tensor("wu", [2 * DM, DM], F32, kind="ExternalInput").ap()
    bq_d = nc.dram_tensor("bq", [DM], F32, kind="ExternalInput").ap()
    bk_d = nc.dram_tensor("bk", [DM], F32, kind="ExternalInput").ap()
    bv_d = nc.dram_tensor("bv", [DM], F32, kind="ExternalInput").ap()
    bo_d = nc.dram_tensor("bo", [DM], F32, kind="ExternalInput").ap()
    bu_d = nc.dram_tensor("bu", [DM], F32, kind="ExternalInput").ap()
    out_d = nc.dram_tensor("out", [N, DM], F32, kind="ExternalOutput").ap()

    def r(ap):
        return ap.bitcast(F32R)

    with tile.TileContext(nc) as tc:
        with (
            tc.tile_pool(name="const", bufs=1) as cpool,
            tc.tile_pool(name="big", bufs=1) as bpool,
            tc.tile_pool(name="work", bufs=3) as wpool,
            tc.tile_pool(name="mm", bufs=2, space="PSUM") as mmp,
            tc.tile_pool(name="sc", bufs=2, space="PSUM") as scp,
            tc.tile_pool(name="ctxp", bufs=1, space="PSUM") as ctxp,
        )            :
            # ---- constants / weights (batched DMAs) ----
            ident = cpool.tile([128, 128], F32R, tag="ident")
            ident_f = cpool.tile([128, 128], F32, tag="identf")
            make_identity(nc, ident_f[:])
            nc.vector.tensor_copy(ident[:], ident_f[:])
            # each W loaded as one DMA: [128, 2*DM], chunk c at cols [c*DM, (c+1)*DM)
            wq_a = cpool.tile([128, 2 * DM], F32R, tag="wqa")
            wk_a = cpool.tile([128, 2 * DM], F32R, tag="wka")
            wv_a = cpool.tile([128, 2 * DM], F32R, tag="wva")
            wo_a = cpool.tile([128, 2 * DM], F32R, tag="woa")
            wu_a = cpool.tile([128, 4 * DM], F32R, tag="wua")
            for t_sb, t_d in ((wq_a, wq_d), (wk_a, wk_d), (wv_a, wv_d), (wo_a, wo_d), (wu_a, wu_d)):
                nc.sync.dma_start(
                    t_sb[:].rearrange("p (c d) -> p c d", d=DM),
                    r(t_d.rearrange("(c p) d -> p c d", p=128)),
                )
            wq_sb = [wq_a[:, c * DM:(c + 1) * DM] for c in range(2)]
            wk_sb = [wk_a[:, c * DM:(c + 1) * DM] for c in range(2)]
            wv_sb = [wv_a[:, c * DM:(c + 1) * DM] for c in range(2)]
            wo_sb = [wo_a[:, c * DM:(c + 1) * DM] for c in range(2)]
            wu_sb = [wu_a[:, c * DM:(c + 1) * DM] for c in range(4)]
            bq_a = cpool.tile([128, 2], F32, tag="bqa")
            bk_a = cpool.tile([128, 2], F32, tag="bka")
            bo_a = cpool.tile([128, 2], F32, tag="boa")
            bu_a = cpool.tile([128, 2], F32, tag="bua")
            for t_sb, t_d in ((bq_a, bq_d), (bk_a, bk_d), (bo_a, bo_d), (bu_a, bu_d)):
                nc.sync.dma_start(t_sb[:], t_d.rearrange("(c p) -> p c", p=128))
            bq_c = [bq_a[:, b:b + 1] for b in range(2)]
            bk_c = [bk_a[:, b:b + 1] for b in range(2)]
            bo_c = [bo_a[:, b:b + 1] for b in range(2)]
            bu_c = [bu_a[:, b:b + 1] for b in range(2)]
            # bv broadcast tile [128, 256] (f32; only used by DVE add)
            bv_row = cpool.tile([1, DM], F32, tag="bvrow")
            nc.sync.dma_start(bv_row[:], bv_d.rearrange("(b a) -> b a", b=1))
            ones1 = cpool.tile([1, 128], F32, tag="ones1")
            nc.gpsimd.memset(ones1[:], 1.0)
            bv_bc = cpool.tile([128, DM], F32, tag="bvbc")
            pt = mmp.tile([128, DM], F32, tag="mm")
            nc.tensor.matmul(pt[:], ones1[:], bv_row[:], start=True, stop=True)
            nc.vector.tensor_copy(bv_bc[:], pt[:])
            ones_r = cpool.tile([1, 64], F32R, tag="onesr")
            ones_rf = cpool.tile([1, 64], F32, tag="onesrf")
            nc.gpsimd.memset(ones_rf[:], 1.0)
            nc.vector.tensor_copy(ones_r[:], ones_rf[:])
            ones_col4 = cpool.tile([128, 4], F32, tag="onescol4")
            nc.gpsimd.memset(ones_col4[:], 1.0)
            # ---- x load (one DMA) + transpose to x^T ----
            stage = cpool.tile([128, NB * DM], F32R, tag="stage")
            xs_all = stage
            x_r = r(x_d.rearrange("(t p) d -> p t d", p=128))
            for g in range(4):
                nc.sync.dma_start(
                    xs_all[:, g * 4 * DM:(g + 1) * 4 * DM].rearrange(
                        "p (t d) -> p t d", d=DM
                    ),
                    x_r[:, g * 4:(g + 1) * 4, :],
                )

            xT = [bpool.tile([128, N], F32R, tag=f"xT{c}", name=f"xT{c}") for c in range(2)]
            for ib in range(NB):
                for c in range(2):
                    tp = mmp.tile([128, 128], F32R, tag="mm")
                    nc.tensor.transpose(
                        tp[:], xs_all[:, ib * DM + c * 128:ib * DM + (c + 1) * 128], ident[:]
                    )
                    nc.vector.tensor_copy(xT[c][:, ib * 128:(ib + 1) * 128], tp[:])

            # ---- q^T, k^T (col-major) ----
            qT = [bpool.tile([128, N], F32R, tag=f"qT{b}", name=f"qT{b}") for b in range(2)]
            kT = [bpool.tile([128, N], F32R, tag=f"kT{b}", name=f"kT{b}") for b in range(2)]

            def emit_qk(blk):
                for w_sb, b_c, dstT in ((wq_sb, bq_c, qT), (wk_sb, bk_c, kT)):
                    for it in range(IT):
                        pt = mmp.tile([128, 512], F32, tag="mm")
                        for c in range(2):
                            nc.tensor.matmul(
                                pt[:],
                                w_sb[c][:, blk * 128:(blk + 1) * 128],
                                xT[c][:, it * 512:(it + 1) * 512],
                                start=(c == 0),
                                stop=(c == 1),
                            )
                        nc.vector.tensor_scalar_add(
                            dstT[blk][:, it * 512:(it + 1) * 512], pt[:], b_c[blk][:]
                        )

            emit_qk(0)

            # ---- v (row-major, with ones col per head) ----
            # v_sb[jb]: [128, 4*65]; head h data at cols 65h..65h+63, ones at 65h+64
            v_sb = [bpool.tile([128, 4 * 65], F32R, tag=f"v{jb}", name=f"v{jb}") for jb in range(NB)]
            for jb in range(NB):
                v4 = v_sb[jb][:].rearrange("p (h e) -> p h e", e=65)
                nc.vector.tensor_copy(
                    v4[:, :, 64:65],
                    ones_col4[:].rearrange("p (h e) -> p h e", e=1),
                )
                pt = mmp.tile([128, DM], F32, tag="mm")
                for c in range(2):
                    nc.tensor.matmul(
                        pt[:],
                        xT[c][:, jb * 128:(jb + 1) * 128],
                        wv_sb[c][:],
                        start=(c == 0),
                        stop=(c == 1),
                    )
                nc.vector.tensor_tensor(
                    v4[:, :, 0:64],
                    pt[:].rearrange("p (h e) -> p h e", e=64),
                    bv_bc[:].rearrange("p (h e) -> p h e", e=64),
                    op=mybir.AluOpType.add,
                )

            # ---- attention per head ----
            ectx = [bpool.tile([128, N], F32R, tag=f"ectx{b}", name=f"ectx{b}") for b in range(2)]
            m_sb = [bpool.tile([128, N], F32R, tag=f"m{b}", name=f"m{b}") for b in range(2)]
            uT = [bpool.tile([128, N], F32R, tag=f"uT{b}", name=f"uT{b}") for b in range(2)]
            ostage = stage
            out_r = r(out_d.rearrange("(t p) d -> p t d", p=128))
            emit_qk(1)

            def attention(h, half):
                qh = qT[h // 2][64 * (h % 2):64 * (h % 2) + 64, :]
                kh = kT[h // 2][64 * (h % 2):64 * (h % 2) + 64, :]
                dst = ectx[h // 2][64 * (h % 2):64 * (h % 2) + 64, :]
                hstart, hend = 1024 * half, 1024 * (half + 1)
                jb_max = 8 * (half + 1)
                ctx_ps = ctxp.tile([65, 1024], F32, tag="ctx", name="ctx")
                for jb in range(jb_max):
                    it0 = jb // 4

                    def col_start(it):
                        # partial-width diagonal tiles (min 256 wide to stay
                        # in the f32r 1-cyc/row regime)
                        if it == it0:
                            return it * 512 + min(128 * (jb % 4), 256)
                        return it * 512

                    its = [t for t in range(max(it0, 2 * half), 2 * half + 2)]
                    cst0 = col_start(its[0])
                    sc = scp.tile([128, 1024], F32, tag="sc", name="sc")
                    for it in its:
                        cst, cend = col_start(it), (it + 1) * 512
                        nc.tensor.matmul(
                            sc[:, cst - hstart:cend - hstart],
                            kh[:, jb * 128:(jb + 1) * 128],
                            qh[:, cst:cend],
                            start=True,
                            stop=True,
                        )
                    wtot = hend - cst0
                    # leading cols that are entirely masked (select fills
                    # them with 0 without reading exp output)
                    skip = 128 if (its[0] == it0 and jb % 4 == 3) else 0
                    e = wpool.tile([128, 1024], F32R, tag="e", bufs=6, name="e")
                    nc.scalar.activation(
                        e[:, skip:wtot], sc[:, cst0 - hstart + skip:1024],
                        mybir.ActivationFunctionType.Exp,
                        scale=float(1.0 / np.sqrt(HD)),
                    )
                    if its[0] == it0:
                        # only cols where some row can be invalid:
                        # f < p - base, base in {0, -128}
                        wd = 128 if (jb % 4) < 3 else 256
                        nc.gpsimd.affine_select(
                            e[:, 0:wd], e[:, 0:wd],
                            pattern=[[1, wd]],
                            compare_op=mybir.AluOpType.is_ge,
                            fill=0.0,
                            base=cst0 - 128 * jb,
                            channel_multiplier=-1,
                        )
                    for it in its:
                        cst, cend = col_start(it), (it + 1) * 512
                        nc.tensor.matmul(
                            ctx_ps[0:65, cst - hstart:cend - hstart],
                            v_sb[jb][:, 65 * h:65 * h + 65],
                            e[:, cst - cst0:cend - cst0],
                            start=(jb == 0),
                            stop=(jb == min(4 * it + 3, jb_max - 1)),
                            skip_group_check=True,
                        )
                # normalize: recip of sums row, partition-broadcast on Pool,
                # multiply fused into the PSUM eviction
                recip = wpool.tile([1, 1024], F32, tag="recip", bufs=2, name="recip")
                nc.vector.reciprocal(recip[:], ctx_ps[64:65, :])
                rb = wpool.tile([64, 1024], F32, tag="rb", bufs=2, name="rb")
                nc.gpsimd.partition_broadcast(rb[:], recip[:])
                nc.vector.tensor_tensor(
                    dst[:, hstart:hend],
                    ctx_ps[0:64, :],
                    rb[:],
                    op=mybir.AluOpType.mult,
                )

            def tail(it):
                isl = slice(it * 512, (it + 1) * 512)
                for blk in range(2):
                    pt = mmp.tile([128, 512], F32, tag="mm", name="pt")
                    for c in range(2):
                        nc.tensor.matmul(
                            pt[:],
                            wo_sb[c][:, blk * 128:(blk + 1) * 128],
                            ectx[c][:, isl],
                            start=(c == 0),
                            stop=(c == 1),
                        )
                    nc.vector.tensor_scalar_add(m_sb[blk][:, isl], pt[:], bo_c[blk][:])
                for blk in range(2):
                    pt = mmp.tile([128, 512], F32, tag="mm", name="pt")
                    for c in range(4):
                        rhs = xT[c] if c < 2 else m_sb[c - 2]
                        nc.tensor.matmul(
                            pt[:],
                            wu_sb[c][:, blk * 128:(blk + 1) * 128],
                            rhs[:, isl],
                            start=(c == 0),
                            stop=(c == 3),
                        )
                    nc.vector.tensor_scalar(
                        uT[blk][:, isl], pt[:], bu_c[blk][:], 0.0,
                        op0=mybir.AluOpType.add, op1=mybir.AluOpType.max,
                    )
                for ib in range(it * 4, (it + 1) * 4):
                    for blk in range(2):
                        tp = scp.tile([128, 128], F32R, tag="sc", name="tp")
                        nc.tensor.transpose(
                            tp[:], uT[blk][:, ib * 128:(ib + 1) * 128], ident[:]
                        )
                        nc.scalar.copy(
                            ostage[:, ib * DM + blk * 128:ib * DM + (blk + 1) * 128],
                            tp[:],
                        )
                nc.sync.dma_start(
                    out_r[:, it * 4:(it + 1) * 4, :],
                    ostage[:, it * 4 * DM:(it + 1) * 4 * DM].rearrange(
                        "p (t d) -> p t d", d=DM
                    ),
                )

            for h in range(H):
                for half in range(2):
                    attention(h, half)
            for it in range(IT):
                tail(it)

    nc.compile()
    return nc


_STATE = {}


def _get_runner():
    if "run" in _STATE:
        return _STATE["run"]
    import jax
    from concourse.bass2jax import (
        _bass_exec_p,
        install_neuronx_cc_hook,
        partition_id_tensor,
    )
    from jax.sharding import Mesh, PartitionSpec
    from jax.experimental.shard_map import shard_map

    nc = build_program()
    install_neuronx_cc_hook()
    partition_name = nc.partition_id_tensor.name if nc.partition_id_tensor else None
    in_names, out_names, out_avals, zero_outs = [], [], [], []
    for alloc in nc.m.functions[0].allocations:
        if not isinstance(alloc, mybir.MemoryLocationSet):
            continue
        name = alloc.memorylocations[0].name
        if alloc.kind == "ExternalInput":
            if name != partition_name:
                in_names.append(name)
        elif alloc.kind == "ExternalOutput":
            shape = tuple(alloc.tensor_shape)
            dtype = mybir.dt.np(alloc.dtype)
            out_names.append(name)
            out_avals.append(jax.core.ShapedArray(shape, dtype))
            zero_outs.append(np.zeros(shape, dtype))
    n_params = len(in_names)
    all_in = in_names + out_names + ([partition_name] if partition_name else [])

    def _body(*args):
        operands = list(args)
        if partition_name is not None:
            operands.append(partition_id_tensor())
        return tuple(
            _bass_exec_p.bind(
                *operands,
                out_avals=tuple(out_avals),
                in_names=tuple(all_in),
                out_names=tuple(out_names),
                lowering_input_output_aliases=(),
                sim_require_finite=True,
                sim_require_nnan=True,
                nc=nc,
            )
        )

    devices = jax.devices()[:B]
    mesh = Mesh(np.asarray(devices), ("core",))
    specs = (PartitionSpec("core"),) * (n_params + len(out_names))
    jitted = jax.jit(
        shard_map(
            _body, mesh=mesh, in_specs=specs,
            out_specs=(PartitionSpec("core"),) * len(out_names), check_rep=False,
        ),
        keep_unused=True,
    )

    def run(in_maps):
        import jax as _jax

        concat_in = [
            np.concatenate([np.asarray(m[nm]) for m in in_maps], axis=0)
            for nm in in_names
        ]
        concat_zero = [
            np.zeros((B * z.shape[0], *z.shape[1:]), z.dtype) for z in zero_outs
        ]
        outs = jitted(*concat_in, *concat_zero)
        _jax.block_until_ready(outs)
        res = []
        o = np.asarray(outs[out_names.index("out")])
        per = o.shape[0] // B
        for c in range(B):
            res.append(o[c * per:(c + 1) * per])
        return res

    _STATE["run"] = run
    return run


def make_in_maps(node_features, Wq, bq, Wk, bk, Wv, bv, Wo, bo, Wu, bu):
    in_maps = []
    for c in range(B):
        in_maps.append(
            {
                "x": np.ascontiguousarray(node_features[c], dtype=np.float32),
                "wq": np.asarray(Wq, np.float32),
                "wk": np.asarray(Wk, np.float32),
                "wv": np.asarray(Wv, np.float32),
                "wo": np.asarray(Wo, np.float32),
                "wu": np.asarray(Wu, np.float32),
                "bq": np.asarray(bq, np.float32),
                "bk": np.asarray(bk, np.float32),
                "bv": np.asarray(bv, np.float32),
                "bo": np.asarray(bo, np.float32),
                "bu": np.asarray(bu, np.float32),
            }
        )
    return in_maps


def kernel(
    node_features, causal_mask, Wq, bq, Wk, bk, Wv, bv, Wo, bo, Wu, bu
):
    """Full-input entry point: shards batch across 8 cores internally."""
    del causal_mask  # guaranteed tril(ones); mask generated on-chip
    run = _get_runner()
    in_maps = make_in_maps(node_features, Wq, bq, Wk, bk, Wv, bv, Wo, bo, Wu, bu)
    outs = run(in_maps)
    return np.stack(outs, axis=0)


# revision 31
# speedup vs baseline: 1.5269x; 1.0351x over previous
"""Trainium2 Bass kernel for CausalMessagePassing (B=8, N=2048, D=256, H=4).

Strategy: data-parallel across 8 NeuronCores, one graph per core.
Per-core dataflow is column-major ("transposed spine"):
  x^T -> q^T,k^T (col-major), v (row-major with ones column for softmax sums)
  scores^T[j,i] = k^T.T @ q^T per head (f32r matmuls, 1 cyc/row)
  e = exp(scores * 1/sqrt(hd)) with causal mask applied on-chip via
  affine_select (the [N,N] mask input is tril(ones) by construction and is
  never DMA'd).
  ctx'^T[65,i] = v'.T @ e^T accumulated over j-blocks; row 64 = softmax sums.
  normalize via K=1 broadcast matmul of 1/sums, fused into PSUM eviction.
  messages^T = Wo.T @ ectx^T (+bo), u^T = relu(Wu.T @ [x^T; m^T] + bu),
  PE-transpose u^T -> u, DMA out.
"""
import sys

sys.path.insert(0, "/opt/trn_rl_repo")

import numpy as np

import concourse.bass as bass  # noqa: F401
import concourse.mybir as mybir
import concourse.tile as tile
from concourse import bacc
from concourse.masks import make_identity

B, N, DM, H = 8, 2048, 256, 4
HD = DM // H  # 64
NB = N // 128  # 16 j-blocks
IT = N // 512  # 4 i-tiles
F32 = mybir.dt.float32
F32R = mybir.dt.float32r


def build_program():
    nc = bacc.Bacc("TRN2", target_bir_lowering=False, debug=False)
    x_d = nc.dram_tensor("x", [N, DM], F32, kind="ExternalInput").ap()
    wq_d = nc.dram_tensor("wq", [DM, DM], F32, kind="ExternalInput").ap()
    wk_d = nc.dram_tensor("wk", [DM, DM], F32, kind="ExternalInput").ap()
    wv_d = nc.dram_tensor("wv", [DM, DM], F32, kind="ExternalInput").ap()
    wo_d = nc.dram_tensor("wo", [DM, DM], F32, kind="ExternalInput").ap()
    wu_d = nc.dram_tensor("wu", [2 * DM, DM], F32, kind="ExternalInput").ap()
    bq_d = nc.dram_tensor("bq", [DM], F32, kind="ExternalInput").ap()
    bk_d = nc.dram_tensor("bk", [DM], F32, kind="ExternalInput").ap()
    bv_d = nc.dram_tensor("bv", [DM], F32, kind="ExternalInput").ap()
    bo_d = nc.dram_tensor("bo", [DM], F32, kind="ExternalInput").ap()
    bu_d = nc.dram_tensor("bu", [DM], F32, kind="ExternalInput").ap()
    out_d = nc.dram_tensor("out", [N, DM], F32, kind="ExternalOutput").ap()

    def r(ap):
        return ap.bitcast(F32R)

    with tile.TileContext(nc) as tc:
        with (
            tc.tile_pool(name="const", bufs=1) as cpool,
            tc.tile_pool(name="big", bufs=1) as bpool,
            tc.tile_pool(name="work", bufs=3) as wpool,
            tc.tile_pool(name="mm", bufs=2, space="PSUM") as mmp,
            tc.tile_pool(name="sc", bufs=2, space="PSUM") as scp,
            tc.tile_pool(name="ctxp", bufs=1, space="PSUM") as ctxp,
        )            :
            # ---- constants / weights (batched DMAs) ----
            ident = cpool.tile([128, 128], F32R, tag="ident")
            ident_f = cpool.tile([128, 128], F32, tag="identf")
            make_identity(nc, ident_f[:])
            nc.vector.tensor_copy(ident[:], ident_f[:])
            # each W loaded as one DMA: [128, 2*DM], chunk c at cols [c*DM, (c+1)*DM)
            wq_a = cpool.tile([128, 2 * DM], F32R, tag="wqa")
            wk_a = cpool.tile([128, 2 * DM], F32R, tag="wka")
            wv_a = cpool.tile([128, 2 * DM], F32R, tag="wva")
            wo_a = cpool.tile([128, 2 * DM], F32R, tag="woa")
            wu_a = cpool.tile([128, 4 * DM], F32R, tag="wua")

            def dma_w(t_sb, t_d):
                nc.sync.dma_start(
                    t_sb[:].rearrange("p (c d) -> p c d", d=DM),
                    r(t_d.rearrange("(c p) d -> p c d", p=128)),
                )

            stage = cpool.tile([128, NB * DM], F32R, tag="stage")
            xs_all = stage
            x_r = r(x_d.rearrange("(t p) d -> p t d", p=128))

            def dma_x(g):
                nc.sync.dma_start(
                    xs_all[:, g * 2 * DM:(g + 1) * 2 * DM].rearrange(
                        "p (t d) -> p t d", d=DM
                    ),
                    x_r[:, g * 2:(g + 1) * 2, :],
                )

            dma_x(0)
            dma_x(1)
            dma_w(wq_a, wq_d)
            dma_w(wk_a, wk_d)
            dma_x(2)
            dma_x(3)
            wq_sb = [wq_a[:, c * DM:(c + 1) * DM] for c in range(2)]
            wk_sb = [wk_a[:, c * DM:(c + 1) * DM] for c in range(2)]
            wv_sb = [wv_a[:, c * DM:(c + 1) * DM] for c in range(2)]
            wo_sb = [wo_a[:, c * DM:(c + 1) * DM] for c in range(2)]
            wu_sb = [wu_a[:, c * DM:(c + 1) * DM] for c in range(4)]
            bq_a = cpool.tile([128, 2], F32, tag="bqa")
            bk_a = cpool.tile([128, 2], F32, tag="bka")
            bo_a = cpool.tile([128, 2], F32, tag="boa")
            bu_a = cpool.tile([128, 2], F32, tag="bua")
            for t_sb, t_d in ((bq_a, bq_d), (bk_a, bk_d), (bo_a, bo_d), (bu_a, bu_d)):
                nc.sync.dma_start(t_sb[:], t_d.rearrange("(c p) -> p c", p=128))
            bq_c = [bq_a[:, b:b + 1] for b in range(2)]
            bk_c = [bk_a[:, b:b + 1] for b in range(2)]
            bo_c = [bo_a[:, b:b + 1] for b in range(2)]
            bu_c = [bu_a[:, b:b + 1] for b in range(2)]
            # bv broadcast tile [128, 256] (f32; only used by DVE add)
            bv_row = cpool.tile([1, DM], F32, tag="bvrow")
            nc.sync.dma_start(bv_row[:], bv_d.rearrange("(b a) -> b a", b=1))
            ones1 = cpool.tile([1, 128], F32, tag="ones1")
            nc.gpsimd.memset(ones1[:], 1.0)
            bv_bc = cpool.tile([128, DM], F32, tag="bvbc")
            pt = mmp.tile([128, DM], F32, tag="mm")
            nc.tensor.matmul(pt[:], ones1[:], bv_row[:], start=True, stop=True)
            nc.vector.tensor_copy(bv_bc[:], pt[:])
            ones_r = cpool.tile([1, 64], F32R, tag="onesr")
            ones_rf = cpool.tile([1, 64], F32, tag="onesrf")
            nc.gpsimd.memset(ones_rf[:], 1.0)
            nc.vector.tensor_copy(ones_r[:], ones_rf[:])
            ones_col4 = cpool.tile([128, 4], F32, tag="onescol4")
            nc.gpsimd.memset(ones_col4[:], 1.0)
            # ---- rest of x + remaining weights ----
            for g in range(4, 8):
                dma_x(g)
            dma_w(wv_a, wv_d)
            dma_w(wo_a, wo_d)
            dma_w(wu_a, wu_d)

            xT = [bpool.tile([128, N], F32R, tag=f"xT{c}", name=f"xT{c}") for c in range(2)]
            for ib in range(NB):
                for c in range(2):
                    tp = mmp.tile([128, 128], F32R, tag="mm")
                    nc.tensor.transpose(
                        tp[:], xs_all[:, ib * DM + c * 128:ib * DM + (c + 1) * 128], ident[:]
                    )
                    nc.vector.tensor_copy(xT[c][:, ib * 128:(ib + 1) * 128], tp[:])

            # ---- q^T, k^T (col-major) ----
            qT = [bpool.tile([128, N], F32R, tag=f"qT{b}", name=f"qT{b}") for b in range(2)]
            kT = [bpool.tile([128, N], F32R, tag=f"kT{b}", name=f"kT{b}") for b in range(2)]

            def emit_qk(blk):
                for w_sb, b_c, dstT in ((wq_sb, bq_c, qT), (wk_sb, bk_c, kT)):
                    for it in range(IT):
                        pt = mmp.tile([128, 512], F32, tag="mm", name="qkpt")
                        for c in range(2):
                            nc.tensor.matmul(
                                pt[:],
                                w_sb[c][:, blk * 128:(blk + 1) * 128],
                                xT[c][:, it * 512:(it + 1) * 512],
                                start=(c == 0),
                                stop=(c == 1),
                            )
                        nc.vector.tensor_scalar_add(
                            dstT[blk][:, it * 512:(it + 1) * 512], pt[:], b_c[blk][:]
                        )

            emit_qk(0)

            # ---- v (row-major, with ones col per head) ----
            # v_sb[jb]: [128, 4*65]; head h data at cols 65h..65h+63, ones at 65h+64
            v_sb = [bpool.tile([128, 4 * 65], F32R, tag=f"v{jb}", name=f"v{jb}") for jb in range(NB)]
            for jb in range(NB):
                v4 = v_sb[jb][:].rearrange("p (h e) -> p h e", e=65)
                nc.vector.tensor_copy(
                    v4[:, :, 64:65],
                    ones_col4[:].rearrange("p (h e) -> p h e", e=1),
                )
                pt = mmp.tile([128, DM], F32, tag="mm")
                for c in range(2):
                    nc.tensor.matmul(
                        pt[:],
                        xT[c][:, jb * 128:(jb + 1) * 128],
                        wv_sb[c][:],
                        start=(c == 0),
                        stop=(c == 1),
                    )
                nc.vector.tensor_tensor(
                    v4[:, :, 0:64],
                    pt[:].rearrange("p (h e) -> p h e", e=64),
                    bv_bc[:].rearrange("p (h e) -> p h e", e=64),
                    op=mybir.AluOpType.add,
                )

            # ---- attention per head ----
            ectx = [bpool.tile([128, N], F32R, tag=f"ectx{b}", name=f"ectx{b}") for b in range(2)]
            m_sb = [bpool.tile([128, N], F32R, tag=f"m{b}", name=f"m{b}") for b in range(2)]
            uT = [bpool.tile([128, N], F32R, tag=f"uT{b}", name=f"uT{b}") for b in range(2)]
            ostage = stage
            out_r = r(out_d.rearrange("(t p) d -> p t d", p=128))
            emit_qk(1)

            def attention(h, half):
                qh = qT[h // 2][64 * (h % 2):64 * (h % 2) + 64, :]
                kh = kT[h // 2][64 * (h % 2):64 * (h % 2) + 64, :]
                dst = ectx[h // 2][64 * (h % 2):64 * (h % 2) + 64, :]
                hstart, hend = 1024 * half, 1024 * (half + 1)
                jb_max = 8 * (half + 1)
                ctx_ps = ctxp.tile([65, 1024], F32, tag="ctx", name="ctx")
                for jb in range(jb_max):
                    it0 = jb // 4

                    def col_start(it):
                        # partial-width diagonal tiles (min 256 wide to stay
                        # in the f32r 1-cyc/row regime)
                        if it == it0:
                            return it * 512 + min(128 * (jb % 4), 256)
                        return it * 512

                    its = [t for t in range(max(it0, 2 * half), 2 * half + 2)]
                    cst0 = col_start(its[0])
                    sc = scp.tile([128, 1024], F32, tag="sc", name="sc")
                    for it in its:
                        cst, cend = col_start(it), (it + 1) * 512
                        nc.tensor.matmul(
                            sc[:, cst - hstart:cend - hstart],
                            kh[:, jb * 128:(jb + 1) * 128],
                            qh[:, cst:cend],
                            start=True,
                            stop=True,
                        )
                    wtot = hend - cst0
                    # leading cols that are entirely masked (select fills
                    # them with 0 without reading exp output)
                    skip = 128 if (its[0] == it0 and jb % 4 == 3) else 0
                    e = wpool.tile([128, 1024], F32R, tag="e", bufs=8, name="e")
                    nc.scalar.activation(
                        e[:, skip:wtot], sc[:, cst0 - hstart + skip:1024],
                        mybir.ActivationFunctionType.Exp,
                        scale=float(1.0 / np.sqrt(HD)),
                    )
                    if its[0] == it0:
                        # only cols where some row can be invalid:
                        # f < p - base, base in {0, -128}
                        wd = 128 if (jb % 4) < 3 else 256
                        nc.gpsimd.affine_select(
                            e[:, 0:wd], e[:, 0:wd],
                            pattern=[[1, wd]],
                            compare_op=mybir.AluOpType.is_ge,
                            fill=0.0,
                            base=cst0 - 128 * jb,
                            channel_multiplier=-1,
                        )
                    for it in its:
                        cst, cend = col_start(it), (it + 1) * 512
                        last_jb = min(4 * it + 3, jb_max - 1)
                        nc.tensor.matmul(
                            ctx_ps[0:65, cst - hstart:cend - hstart],
                            v_sb[jb][:, 65 * h:65 * h + 65],
                            e[:, cst - cst0:cend - cst0],
                            start=(jb == 0),
                            stop=(jb == last_jb),
                            skip_group_check=True,
                        )
                        if jb == last_jb:
                            # this 512-col quarter of ctx is final: normalize
                            # and evict now (its PSUM bank is done) while
                            # later jb's still accumulate the other bank
                            lsl = slice(it * 512 - hstart, it * 512 - hstart + 512)
                            recip = wpool.tile(
                                [1, 512], F32, tag="recip", bufs=2, name="recip"
                            )
                            nc.vector.reciprocal(recip[:], ctx_ps[64:65, lsl])
                            rb = wpool.tile([64, 512], F32, tag="rb", bufs=2, name="rb")
                            nc.gpsimd.partition_broadcast(rb[:], recip[:])
                            nc.vector.tensor_tensor(
                                dst[:, it * 512:(it + 1) * 512],
                                ctx_ps[0:64, lsl],
                                rb[:],
                                op=mybir.AluOpType.mult,
                            )


            def tail(it):
                isl = slice(it * 512, (it + 1) * 512)
                for blk in range(2):
                    pt = mmp.tile([128, 512], F32, tag="mm", name="pt")
                    for c in range(2):
                        nc.tensor.matmul(
                            pt[:],
                            wo_sb[c][:, blk * 128:(blk + 1) * 128],
                            ectx[c][:, isl],
                            start=(c == 0),
                            stop=(c == 1),
                        )
                    nc.vector.tensor_scalar_add(m_sb[blk][:, isl], pt[:], bo_c[blk][:])
                for blk in range(2):
                    pt = mmp.tile([128, 512], F32, tag="mm", name="pt")
                    for c in range(4):
                        rhs = xT[c] if c < 2 else m_sb[c - 2]
                        nc.tensor.matmul(
                            pt[:],
                            wu_sb[c][:, blk * 128:(blk + 1) * 128],
                            rhs[:, isl],
                            start=(c == 0),
                            stop=(c == 3),
                        )
                    nc.vector.tensor_scalar(
                        uT[blk][:, isl], pt[:], bu_c[blk][:], 0.0,
                        op0=mybir.AluOpType.add, op1=mybir.AluOpType.max,
                    )
                for ib in range(it * 4, (it + 1) * 4):
                    for blk in range(2):
                        tp = scp.tile([128, 128], F32R, tag="sc", name="tp")
                        nc.tensor.transpose(
                            tp[:], uT[blk][:, ib * 128:(ib + 1) * 128], ident[:]
                        )
                        nc.scalar.copy(
                            ostage[:, ib * DM + blk * 128:ib * DM + (blk + 1) * 128],
                            tp[:],
                        )
                nc.sync.dma_start(
                    out_r[:, it * 4:(it + 1) * 4, :],
                    ostage[:, it * 4 * DM:(it + 1) * 4 * DM].rearrange(
                        "p (t d) -> p t d", d=DM
                    ),
                )

            for h in range(H):
                for half in range(2):
                    attention(h, half)
            for it in range(IT):
                tail(it)

    nc.compile()
    return nc


_STATE = {}


def _get_runner():
    if "run" in _STATE:
        return _STATE["run"]
    import jax
    from concourse.bass2jax import (
        _bass_exec_p,
        install_neuronx_cc_hook,
        partition_id_tensor,
    )
    from jax.sharding import Mesh, PartitionSpec
    from jax.experimental.shard_map import shard_map

    nc = build_program()
    install_neuronx_cc_hook()
    partition_name = nc.partition_id_tensor.name if nc.partition_id_tensor else None
    in_names, out_names, out_avals, zero_outs = [], [], [], []
    for alloc in nc.m.functions[0].allocations:
        if not isinstance(alloc, mybir.MemoryLocationSet):
            continue
        name = alloc.memorylocations[0].name
        if alloc.kind == "ExternalInput":
            if name != partition_name:
                in_names.append(name)
        elif alloc.kind == "ExternalOutput":
            shape = tuple(alloc.tensor_shape)
            dtype = mybir.dt.np(alloc.dtype)
            out_names.append(name)
            out_avals.append(jax.core.ShapedArray(shape, dtype))
            zero_outs.append(np.zeros(shape, dtype))
    n_params = len(in_names)
    all_in = in_names + out_names + ([partition_name] if partition_name else [])

    def _body(*args):
        operands = list(args)
        if partition_name is not None:
            operands.append(partition_id_tensor())
        return tuple(
            _bass_exec_p.bind(
                *operands,
                out_avals=tuple(out_avals),
                in_names=tuple(all_in),
                out_names=tuple(out_names),
                lowering_input_output_aliases=(),
                sim_require_finite=True,
                sim_require_nnan=True,
                nc=nc,
            )
        )

    devices = jax.devices()[:B]
    mesh = Mesh(np.asarray(devices), ("core",))
    specs = (PartitionSpec("core"),) * (n_params + len(out_names))
    jitted = jax.jit(
        shard_map(
            _body, mesh=mesh, in_specs=specs,
            out_specs=(PartitionSpec("core"),) * len(out_names), check_rep=False,
        ),
        keep_unused=True,
    )

    def run(in_maps):
        import jax as _jax

        concat_in = [
            np.concatenate([np.asarray(m[nm]) for m in in_maps], axis=0)
            for nm in in_names
        ]
        concat_zero = [
            np.zeros((B * z.shape[0], *z.shape[1:]), z.dtype) for z in zero_outs
        ]
        outs = jitted(*concat_in, *concat_zero)
        _jax.block_until_ready(outs)
        res = []
        o = np.asarray(outs[out_names.index("out")])
        per = o.shape[0] // B
        for c in range(B):
            res.append(o[c * per:(c + 1) * per])
        return res

    _STATE["run"] = run
    return run


def make_in_maps(node_features, Wq, bq, Wk, bk, Wv, bv, Wo, bo, Wu, bu):
    in_maps = []
    for c in range(B):
        in_maps.append(
            {
                "x": np.ascontiguousarray(node_features[c], dtype=np.float32),
                "wq": np.asarray(Wq, np.float32),
                "wk": np.asarray(Wk, np.float32),
                "wv": np.asarray(Wv, np.float32),
                "wo": np.asarray(Wo, np.float32),
                "wu": np.asarray(Wu, np.float32),
                "bq": np.asarray(bq, np.float32),
                "bk": np.asarray(bk, np.float32),
                "bv": np.asarray(bv, np.float32),
                "bo": np.asarray(bo, np.float32),
                "bu": np.asarray(bu, np.float32),
            }
        )
    return in_maps


def kernel(
    node_features, causal_mask, Wq, bq, Wk, bk, Wv, bv, Wo, bo, Wu, bu
):
    """Full-input entry point: shards batch across 8 cores internally."""
    del causal_mask  # guaranteed tril(ones); mask generated on-chip
    run = _get_runner()
    in_maps = make_in_maps(node_features, Wq, bq, Wk, bk, Wv, bv, Wo, bo, Wu, bu)
    outs = run(in_maps)
    return np.stack(outs, axis=0)


# revision 34
# speedup vs baseline: 1.5477x; 1.0136x over previous
"""Trainium2 Bass kernel for CausalMessagePassing (B=8, N=2048, D=256, H=4).

Strategy: data-parallel across 8 NeuronCores, one graph per core.
Per-core dataflow is column-major ("transposed spine"):
  x^T -> q^T,k^T (col-major), v (row-major with ones column for softmax sums)
  scores^T[j,i] = k^T.T @ q^T per head (f32r matmuls, 1 cyc/row)
  e = exp(scores * 1/sqrt(hd)) with causal mask applied on-chip via
  affine_select (the [N,N] mask input is tril(ones) by construction and is
  never DMA'd).
  ctx'^T[65,i] = v'.T @ e^T accumulated over j-blocks; row 64 = softmax sums.
  normalize via K=1 broadcast matmul of 1/sums, fused into PSUM eviction.
  messages^T = Wo.T @ ectx^T (+bo), u^T = relu(Wu.T @ [x^T; m^T] + bu),
  PE-transpose u^T -> u, DMA out.
"""
import sys

sys.path.insert(0, "/opt/trn_rl_repo")

import numpy as np

import concourse.bass as bass  # noqa: F401
import concourse.mybir as mybir
import concourse.tile as tile
from concourse import bacc
from concourse.masks import make_identity

B, N, DM, H = 8, 2048, 256, 4
HD = DM // H  # 64
NB = N // 128  # 16 j-blocks
IT = N // 512  # 4 i-tiles
F32 = mybir.dt.float32
F32R = mybir.dt.float32r


def build_program():
    nc = bacc.Bacc("TRN2", target_bir_lowering=False, debug=False)
    x_d = nc.dram_tensor("x", [N, DM], F32, kind="ExternalInput").ap()
    wq_d = nc.dram_tensor("wq", [DM, DM], F32, kind="ExternalInput").ap()
    wk_d = nc.dram_tensor("wk", [DM, DM], F32, kind="ExternalInput").ap()
    wv_d = nc.dram_tensor("wv", [DM, DM], F32, kind="ExternalInput").ap()
    wo_d = nc.dram_tensor("wo", [DM, DM], F32, kind="ExternalInput").ap()
    wu_d = nc.dram_tensor("wu", [2 * DM, DM], F32, kind="ExternalInput").ap()
    bq_d = nc.dram_tensor("bq", [DM], F32, kind="ExternalInput").ap()
    bk_d = nc.dram_tensor("bk", [DM], F32, kind="ExternalInput").ap()
    bv_d = nc.dram_tensor("bv", [DM], F32, kind="ExternalInput").ap()
    bo_d = nc.dram_tensor("bo", [DM], F32, kind="ExternalInput").ap()
    bu_d = nc.dram_tensor("bu", [DM], F32, kind="ExternalInput").ap()
    out_d = nc.dram_tensor("out", [N, DM], F32, kind="ExternalOutput").ap()

    def r(ap):
        return ap.bitcast(F32R)

    with tile.TileContext(nc) as tc:
        with (
            tc.tile_pool(name="const", bufs=1) as cpool,
            tc.tile_pool(name="big", bufs=1) as bpool,
            tc.tile_pool(name="work", bufs=3) as wpool,
            tc.tile_pool(name="mm", bufs=2, space="PSUM") as mmp,
            tc.tile_pool(name="sc", bufs=2, space="PSUM") as scp,
            tc.tile_pool(name="ctxp", bufs=1, space="PSUM") as ctxp,
        )            :
            # ---- constants / weights (batched DMAs) ----
            ident = cpool.tile([128, 128], F32R, tag="ident")
            ident_f = cpool.tile([128, 128], F32, tag="identf")
            make_identity(nc, ident_f[:])
            nc.vector.tensor_copy(ident[:], ident_f[:])
            # each W loaded as one DMA: [128, 2*DM], chunk c at cols [c*DM, (c+1)*DM)
            wq_a = cpool.tile([128, 2 * DM], F32R, tag="wqa")
            wk_a = cpool.tile([128, 2 * DM], F32R, tag="wka")
            wv_a = cpool.tile([128, 2 * DM], F32R, tag="wva")
            wo_a = cpool.tile([128, 2 * DM], F32R, tag="woa")
            wu_a = cpool.tile([128, 4 * DM], F32R, tag="wua")

            def dma_w(t_sb, t_d):
                nc.sync.dma_start(
                    t_sb[:].rearrange("p (c d) -> p c d", d=DM),
                    r(t_d.rearrange("(c p) d -> p c d", p=128)),
                )

            stage = cpool.tile([128, NB * DM], F32R, tag="stage")
            xs_all = stage
            x_r = r(x_d.rearrange("(t p) d -> p t d", p=128))

            def dma_x(g):
                nc.sync.dma_start(
                    xs_all[:, g * 2 * DM:(g + 1) * 2 * DM].rearrange(
                        "p (t d) -> p t d", d=DM
                    ),
                    x_r[:, g * 2:(g + 1) * 2, :],
                )

            dma_x(0)
            dma_x(1)
            dma_w(wq_a, wq_d)
            dma_w(wk_a, wk_d)
            dma_x(2)
            dma_x(3)
            wq_sb = [wq_a[:, c * DM:(c + 1) * DM] for c in range(2)]
            wk_sb = [wk_a[:, c * DM:(c + 1) * DM] for c in range(2)]
            wv_sb = [wv_a[:, c * DM:(c + 1) * DM] for c in range(2)]
            wo_sb = [wo_a[:, c * DM:(c + 1) * DM] for c in range(2)]
            wu_sb = [wu_a[:, c * DM:(c + 1) * DM] for c in range(4)]
            bq_a = cpool.tile([128, 2], F32, tag="bqa")
            bk_a = cpool.tile([128, 2], F32, tag="bka")
            bo_a = cpool.tile([128, 2], F32, tag="boa")
            bu_a = cpool.tile([128, 2], F32, tag="bua")
            for t_sb, t_d in ((bq_a, bq_d), (bk_a, bk_d), (bo_a, bo_d), (bu_a, bu_d)):
                nc.sync.dma_start(t_sb[:], t_d.rearrange("(c p) -> p c", p=128))
            bq_c = [bq_a[:, b:b + 1] for b in range(2)]
            bk_c = [bk_a[:, b:b + 1] for b in range(2)]
            bo_c = [bo_a[:, b:b + 1] for b in range(2)]
            bu_c = [bu_a[:, b:b + 1] for b in range(2)]
            # bv broadcast tile [128, 256] (f32; only used by DVE add)
            bv_row = cpool.tile([1, DM], F32, tag="bvrow")
            nc.sync.dma_start(bv_row[:], bv_d.rearrange("(b a) -> b a", b=1))
            ones1 = cpool.tile([1, 128], F32, tag="ones1")
            nc.gpsimd.memset(ones1[:], 1.0)
            bv_bc = cpool.tile([128, DM], F32, tag="bvbc")
            pt = mmp.tile([128, DM], F32, tag="mm")
            nc.tensor.matmul(pt[:], ones1[:], bv_row[:], start=True, stop=True)
            nc.vector.tensor_copy(bv_bc[:], pt[:])
            ones_r = cpool.tile([1, 64], F32R, tag="onesr")
            ones_rf = cpool.tile([1, 64], F32, tag="onesrf")
            nc.gpsimd.memset(ones_rf[:], 1.0)
            nc.vector.tensor_copy(ones_r[:], ones_rf[:])
            ones_col4 = cpool.tile([128, 4], F32, tag="onescol4")
            nc.gpsimd.memset(ones_col4[:], 1.0)
            # ---- rest of x + remaining weights ----
            for g in range(4, 8):
                dma_x(g)
            dma_w(wv_a, wv_d)
            dma_w(wo_a, wo_d)
            dma_w(wu_a, wu_d)

            xT = [bpool.tile([128, N], F32R, tag=f"xT{c}", name=f"xT{c}") for c in range(2)]
            for ib in range(NB):
                for c in range(2):
                    tp = mmp.tile([128, 128], F32R, tag="mm")
                    nc.tensor.transpose(
                        tp[:], xs_all[:, ib * DM + c * 128:ib * DM + (c + 1) * 128], ident[:]
                    )
                    nc.vector.tensor_copy(xT[c][:, ib * 128:(ib + 1) * 128], tp[:])

            # ---- q^T, k^T (col-major) ----
            qT = [bpool.tile([128, N], F32R, tag=f"qT{b}", name=f"qT{b}") for b in range(2)]
            kT = [bpool.tile([128, N], F32R, tag=f"kT{b}", name=f"kT{b}") for b in range(2)]

            def emit_qk(blk):
                for w_sb, b_c, dstT in ((wq_sb, bq_c, qT), (wk_sb, bk_c, kT)):
                    for it in range(IT):
                        pt = mmp.tile([128, 512], F32, tag="mm", name="qkpt")
                        for c in range(2):
                            nc.tensor.matmul(
                                pt[:],
                                w_sb[c][:, blk * 128:(blk + 1) * 128],
                                xT[c][:, it * 512:(it + 1) * 512],
                                start=(c == 0),
                                stop=(c == 1),
                            )
                        nc.vector.tensor_scalar_add(
                            dstT[blk][:, it * 512:(it + 1) * 512], pt[:], b_c[blk][:]
                        )

            emit_qk(0)

            # ---- v (row-major, with ones col per head) ----
            # v_sb[jb]: [128, 4*65]; head h data at cols 65h..65h+63, ones at 65h+64
            v_sb = [bpool.tile([128, 4 * 65], F32R, tag=f"v{jb}", name=f"v{jb}") for jb in range(NB)]
            for jb in range(NB):
                v4 = v_sb[jb][:].rearrange("p (h e) -> p h e", e=65)
                nc.vector.tensor_copy(
                    v4[:, :, 64:65],
                    ones_col4[:].rearrange("p (h e) -> p h e", e=1),
                )
                pt = mmp.tile([128, DM], F32, tag="mm")
                for c in range(2):
                    nc.tensor.matmul(
                        pt[:],
                        xT[c][:, jb * 128:(jb + 1) * 128],
                        wv_sb[c][:],
                        start=(c == 0),
                        stop=(c == 1),
                    )
                nc.vector.tensor_tensor(
                    v4[:, :, 0:64],
                    pt[:].rearrange("p (h e) -> p h e", e=64),
                    bv_bc[:].rearrange("p (h e) -> p h e", e=64),
                    op=mybir.AluOpType.add,
                )

            # ---- attention per head ----
            ectx = [bpool.tile([128, N], F32R, tag=f"ectx{b}", name=f"ectx{b}") for b in range(2)]
            m_sb = [bpool.tile([128, N], F32R, tag=f"m{b}", name=f"m{b}") for b in range(2)]
            uT = [bpool.tile([128, N], F32R, tag=f"uT{b}", name=f"uT{b}") for b in range(2)]
            ostage = stage
            out_r = r(out_d.rearrange("(t p) d -> p t d", p=128))
            emit_qk(1)

            def attention(h, half):
                qh = qT[h // 2][64 * (h % 2):64 * (h % 2) + 64, :]
                kh = kT[h // 2][64 * (h % 2):64 * (h % 2) + 64, :]
                dst = ectx[h // 2][64 * (h % 2):64 * (h % 2) + 64, :]
                hstart, hend = 1024 * half, 1024 * (half + 1)
                jb_max = 8 * (half + 1)
                ctx_q = [
                    ctxp.tile([65, 512], F32, tag="ctx", bufs=2, name="ctxq")
                    for _ in range(2)
                ]
                for jb in range(jb_max):
                    it0 = jb // 4

                    def col_start(it):
                        # partial-width diagonal tiles (min 256 wide to stay
                        # in the f32r 1-cyc/row regime)
                        if it == it0:
                            return it * 512 + min(128 * (jb % 4), 256)
                        return it * 512

                    its = [t for t in range(max(it0, 2 * half), 2 * half + 2)]
                    cst0 = col_start(its[0])
                    sc = scp.tile([128, 1024], F32, tag="sc", name="sc")
                    for it in its:
                        cst, cend = col_start(it), (it + 1) * 512
                        nc.tensor.matmul(
                            sc[:, cst - hstart:cend - hstart],
                            kh[:, jb * 128:(jb + 1) * 128],
                            qh[:, cst:cend],
                            start=True,
                            stop=True,
                        )
                    wtot = hend - cst0
                    # leading cols that are entirely masked (select fills
                    # them with 0 without reading exp output)
                    skip = 128 if (its[0] == it0 and jb % 4 == 3) else 0
                    e = wpool.tile([128, 1024], F32R, tag="e", bufs=8, name="e")
                    nc.scalar.activation(
                        e[:, skip:wtot], sc[:, cst0 - hstart + skip:1024],
                        mybir.ActivationFunctionType.Exp,
                        scale=float(1.0 / np.sqrt(HD)),
                    )
                    if its[0] == it0:
                        # only cols where some row can be invalid:
                        # f < p - base, base in {0, -128}
                        wd = 128 if (jb % 4) < 3 else 256
                        nc.gpsimd.affine_select(
                            e[:, 0:wd], e[:, 0:wd],
                            pattern=[[1, wd]],
                            compare_op=mybir.AluOpType.is_ge,
                            fill=0.0,
                            base=cst0 - 128 * jb,
                            channel_multiplier=-1,
                        )
                    for it in its:
                        cst, cend = col_start(it), (it + 1) * 512
                        last_jb = min(4 * it + 3, jb_max - 1)
                        cq = ctx_q[it - 2 * half]
                        qoff = it * 512
                        nc.tensor.matmul(
                            cq[0:65, cst - qoff:cend - qoff],
                            v_sb[jb][:, 65 * h:65 * h + 65],
                            e[:, cst - cst0:cend - cst0],
                            start=(jb == 0),
                            stop=(jb == last_jb),
                            skip_group_check=True,
                        )
                        if jb == last_jb:
                            # this quarter's accumulation is final: normalize
                            # and evict now, freeing its PSUM bank early
                            recip = wpool.tile(
                                [1, 512], F32, tag="recip", bufs=2, name="recip"
                            )
                            nc.vector.reciprocal(recip[:], cq[64:65, :])
                            rb = wpool.tile([64, 512], F32, tag="rb", bufs=2, name="rb")
                            nc.gpsimd.partition_broadcast(rb[:], recip[:])
                            nc.vector.tensor_tensor(
                                dst[:, it * 512:(it + 1) * 512],
                                cq[0:64, :],
                                rb[:],
                                op=mybir.AluOpType.mult,
                            )


            def tail(it):
                isl = slice(it * 512, (it + 1) * 512)
                for blk in range(2):
                    pt = mmp.tile([128, 512], F32, tag="mm", name="pt")
                    for c in range(2):
                        nc.tensor.matmul(
                            pt[:],
                            wo_sb[c][:, blk * 128:(blk + 1) * 128],
                            ectx[c][:, isl],
                            start=(c == 0),
                            stop=(c == 1),
                        )
                    nc.vector.tensor_scalar_add(m_sb[blk][:, isl], pt[:], bo_c[blk][:])
                for blk in range(2):
                    pt = mmp.tile([128, 512], F32, tag="mm", name="pt")
                    for c in range(4):
                        rhs = xT[c] if c < 2 else m_sb[c - 2]
                        nc.tensor.matmul(
                            pt[:],
                            wu_sb[c][:, blk * 128:(blk + 1) * 128],
                            rhs[:, isl],
                            start=(c == 0),
                            stop=(c == 3),
                        )
                    nc.vector.tensor_scalar(
                        uT[blk][:, isl], pt[:], bu_c[blk][:], 0.0,
                        op0=mybir.AluOpType.add, op1=mybir.AluOpType.max,
                    )
                for ib in range(it * 4, (it + 1) * 4):
                    for blk in range(2):
                        tp = scp.tile([128, 128], F32R, tag="sc", name="tp")
                        nc.tensor.transpose(
                            tp[:], uT[blk][:, ib * 128:(ib + 1) * 128], ident[:]
                        )
                        nc.scalar.copy(
                            ostage[:, ib * DM + blk * 128:ib * DM + (blk + 1) * 128],
                            tp[:],
                        )
                nc.sync.dma_start(
                    out_r[:, it * 4:(it + 1) * 4, :],
                    ostage[:, it * 4 * DM:(it + 1) * 4 * DM].rearrange(
                        "p (t d) -> p t d", d=DM
                    ),
                )

            for h in range(H):
                for half in range(2):
                    attention(h, half)
            for it in range(IT):
                tail(it)

    nc.compile()
    return nc


_STATE = {}


def _get_runner():
    if "run" in _STATE:
        return _STATE["run"]
    import jax
    from concourse.bass2jax import (
        _bass_exec_p,
        install_neuronx_cc_hook,
        partition_id_tensor,
    )
    from jax.sharding import Mesh, PartitionSpec
    from jax.experimental.shard_map import shard_map

    nc = build_program()
    install_neuronx_cc_hook()
    partition_name = nc.partition_id_tensor.name if nc.partition_id_tensor else None
    in_names, out_names, out_avals, zero_outs = [], [], [], []
    for alloc in nc.m.functions[0].allocations:
        if not isinstance(alloc, mybir.MemoryLocationSet):
            continue
        name = alloc.memorylocations[0].name
        if alloc.kind == "ExternalInput":
            if name != partition_name:
                in_names.append(name)
        elif alloc.kind == "ExternalOutput":
            shape = tuple(alloc.tensor_shape)
            dtype = mybir.dt.np(alloc.dtype)
            out_names.append(name)
            out_avals.append(jax.core.ShapedArray(shape, dtype))
            zero_outs.append(np.zeros(shape, dtype))
    n_params = len(in_names)
    all_in = in_names + out_names + ([partition_name] if partition_name else [])

    def _body(*args):
        operands = list(args)
        if partition_name is not None:
            operands.append(partition_id_tensor())
        return tuple(
            _bass_exec_p.bind(
                *operands,
                out_avals=tuple(out_avals),
                in_names=tuple(all_in),
                out_names=tuple(out_names),
                lowering_input_output_aliases=(),
                sim_require_finite=True,
                sim_require_nnan=True,
                nc=nc,
            )
        )

    devices = jax.devices()[:B]
    mesh = Mesh(np.asarray(devices), ("core",))
    specs = (PartitionSpec("core"),) * (n_params + len(out_names))
    jitted = jax.jit(
        shard_map(
            _body, mesh=mesh, in_specs=specs,
            out_specs=(PartitionSpec("core"),) * len(out_names), check_rep=False,
        ),
        keep_unused=True,
    )

    def run(in_maps):
        import jax as _jax

        concat_in = [
            np.concatenate([np.asarray(m[nm]) for m in in_maps], axis=0)
            for nm in in_names
        ]
        concat_zero = [
            np.zeros((B * z.shape[0], *z.shape[1:]), z.dtype) for z in zero_outs
        ]
        outs = jitted(*concat_in, *concat_zero)
        _jax.block_until_ready(outs)
        res = []
        o = np.asarray(outs[out_names.index("out")])
        per = o.shape[0] // B
        for c in range(B):
            res.append(o[c * per:(c + 1) * per])
        return res

    _STATE["run"] = run
    return run


def make_in_maps(node_features, Wq, bq, Wk, bk, Wv, bv, Wo, bo, Wu, bu):
    in_maps = []
    for c in range(B):
        in_maps.append(
            {
                "x": np.ascontiguousarray(node_features[c], dtype=np.float32),
                "wq": np.asarray(Wq, np.float32),
                "wk": np.asarray(Wk, np.float32),
                "wv": np.asarray(Wv, np.float32),
                "wo": np.asarray(Wo, np.float32),
                "wu": np.asarray(Wu, np.float32),
                "bq": np.asarray(bq, np.float32),
                "bk": np.asarray(bk, np.float32),
                "bv": np.asarray(bv, np.float32),
                "bo": np.asarray(bo, np.float32),
                "bu": np.asarray(bu, np.float32),
            }
        )
    return in_maps


def kernel(
    node_features, causal_mask, Wq, bq, Wk, bk, Wv, bv, Wo, bo, Wu, bu
):
    """Full-input entry point: shards batch across 8 cores internally."""
    del causal_mask  # guaranteed tril(ones); mask generated on-chip
    run = _get_runner()
    in_maps = make_in_maps(node_features, Wq, bq, Wk, bk, Wv, bv, Wo, bo, Wu, bu)
    outs = run(in_maps)
    return np.stack(outs, axis=0)


# revision 38
# speedup vs baseline: 1.5971x; 1.0319x over previous
"""Trainium2 Bass kernel for CausalMessagePassing (B=8, N=2048, D=256, H=4).

Strategy: data-parallel across 8 NeuronCores, one graph per core.
Per-core dataflow is column-major ("transposed spine"):
  x^T -> q^T,k^T (col-major), v (row-major with ones column for softmax sums)
  scores^T[j,i] = k^T.T @ q^T per head (f32r matmuls, 1 cyc/row)
  e = exp(scores * 1/sqrt(hd)) with causal mask applied on-chip via
  affine_select (the [N,N] mask input is tril(ones) by construction and is
  never DMA'd).
  ctx'^T[65,i] = v'.T @ e^T accumulated over j-blocks; row 64 = softmax sums.
  normalize via K=1 broadcast matmul of 1/sums, fused into PSUM eviction.
  messages^T = Wo.T @ ectx^T (+bo), u^T = relu(Wu.T @ [x^T; m^T] + bu),
  PE-transpose u^T -> u, DMA out.
"""
import sys

sys.path.insert(0, "/opt/trn_rl_repo")

import numpy as np

import concourse.bass as bass  # noqa: F401
import concourse.mybir as mybir
import concourse.tile as tile
from concourse import bacc
from concourse.masks import make_identity

B, N, DM, H = 8, 2048, 256, 4
HD = DM // H  # 64
NB = N // 128  # 16 j-blocks
IT = N // 512  # 4 i-tiles
F32 = mybir.dt.float32
F32R = mybir.dt.float32r


def build_program():
    nc = bacc.Bacc("TRN2", target_bir_lowering=False, debug=False)
    x_d = nc.dram_tensor("x", [N, DM], F32, kind="ExternalInput").ap()
    wq_d = nc.dram_tensor("wq", [DM, DM], F32, kind="ExternalInput").ap()
    wk_d = nc.dram_tensor("wk", [DM, DM], F32, kind="ExternalInput").ap()
    wv_d = nc.dram_tensor("wv", [DM, DM], F32, kind="ExternalInput").ap()
    wo_d = nc.dram_tensor("wo", [DM, DM], F32, kind="ExternalInput").ap()
    wu_d = nc.dram_tensor("wu", [2 * DM, DM], F32, kind="ExternalInput").ap()
    bq_d = nc.dram_tensor("bq", [DM], F32, kind="ExternalInput").ap()
    bk_d = nc.dram_tensor("bk", [DM], F32, kind="ExternalInput").ap()
    bv_d = nc.dram_tensor("bv", [DM], F32, kind="ExternalInput").ap()
    bo_d = nc.dram_tensor("bo", [DM], F32, kind="ExternalInput").ap()
    bu_d = nc.dram_tensor("bu", [DM], F32, kind="ExternalInput").ap()
    out_d = nc.dram_tensor("out", [N, DM], F32, kind="ExternalOutput").ap()

    def r(ap):
        return ap.bitcast(F32R)

    with tile.TileContext(nc) as tc:
        with (
            tc.tile_pool(name="const", bufs=1) as cpool,
            tc.tile_pool(name="big", bufs=1) as bpool,
            tc.tile_pool(name="work", bufs=3) as wpool,
            tc.tile_pool(name="mm", bufs=2, space="PSUM") as mmp,
            tc.tile_pool(name="sc", bufs=2, space="PSUM") as scp,
            tc.tile_pool(name="ctxp", bufs=1, space="PSUM") as ctxp,
        )            :
            # ---- constants / weights (batched DMAs) ----
            ident = cpool.tile([128, 128], F32R, tag="ident")
            ident_f = cpool.tile([128, 128], F32, tag="identf")
            make_identity(nc, ident_f[:])
            nc.vector.tensor_copy(ident[:], ident_f[:])
            # each W loaded as one DMA: [128, 2*DM], chunk c at cols [c*DM, (c+1)*DM)
            wq_a = cpool.tile([128, 2 * DM], F32R, tag="wqa")
            wk_a = cpool.tile([128, 2 * DM], F32R, tag="wka")
            wv_a = cpool.tile([128, 2 * DM], F32R, tag="wva")
            wo_a = cpool.tile([128, 2 * DM], F32R, tag="woa")
            wu_a = cpool.tile([128, 4 * DM], F32R, tag="wua")

            def dma_w(t_sb, t_d):
                nc.sync.dma_start(
                    t_sb[:].rearrange("p (c d) -> p c d", d=DM),
                    r(t_d.rearrange("(c p) d -> p c d", p=128)),
                )

            stage = cpool.tile([128, NB * DM], F32R, tag="stage")
            xs_all = stage
            x_r = r(x_d.rearrange("(t p) d -> p t d", p=128))

            def dma_x(g):
                nc.sync.dma_start(
                    xs_all[:, g * 2 * DM:(g + 1) * 2 * DM].rearrange(
                        "p (t d) -> p t d", d=DM
                    ),
                    x_r[:, g * 2:(g + 1) * 2, :],
                )

            dma_x(0)
            dma_x(1)
            dma_w(wq_a, wq_d)
            dma_w(wk_a, wk_d)
            dma_x(2)
            dma_x(3)
            wq_sb = [wq_a[:, c * DM:(c + 1) * DM] for c in range(2)]
            wk_sb = [wk_a[:, c * DM:(c + 1) * DM] for c in range(2)]
            wv_sb = [wv_a[:, c * DM:(c + 1) * DM] for c in range(2)]
            wo_sb = [wo_a[:, c * DM:(c + 1) * DM] for c in range(2)]
            wu_sb = [wu_a[:, c * DM:(c + 1) * DM] for c in range(4)]
            bq_a = cpool.tile([128, 2], F32, tag="bqa")
            bk_a = cpool.tile([128, 2], F32, tag="bka")
            bo_a = cpool.tile([128, 2], F32, tag="boa")
            bu_a = cpool.tile([128, 2], F32, tag="bua")
            for t_sb, t_d in ((bq_a, bq_d), (bk_a, bk_d), (bo_a, bo_d), (bu_a, bu_d)):
                nc.sync.dma_start(t_sb[:], t_d.rearrange("(c p) -> p c", p=128))
            bq_c = [bq_a[:, b:b + 1] for b in range(2)]
            bk_c = [bk_a[:, b:b + 1] for b in range(2)]
            bo_c = [bo_a[:, b:b + 1] for b in range(2)]
            bu_c = [bu_a[:, b:b + 1] for b in range(2)]
            # bv broadcast tile [128, 256] (f32; only used by DVE add)
            bv_row = cpool.tile([1, DM], F32, tag="bvrow")
            nc.sync.dma_start(bv_row[:], bv_d.rearrange("(b a) -> b a", b=1))
            ones1 = cpool.tile([1, 128], F32, tag="ones1")
            nc.gpsimd.memset(ones1[:], 1.0)
            bv_bc = cpool.tile([128, DM], F32, tag="bvbc")
            pt = mmp.tile([128, DM], F32, tag="mm")
            nc.tensor.matmul(pt[:], ones1[:], bv_row[:], start=True, stop=True)
            nc.vector.tensor_copy(bv_bc[:], pt[:])
            ones_r = cpool.tile([1, 64], F32R, tag="onesr")
            ones_rf = cpool.tile([1, 64], F32, tag="onesrf")
            nc.gpsimd.memset(ones_rf[:], 1.0)
            nc.vector.tensor_copy(ones_r[:], ones_rf[:])
            ones_col4 = cpool.tile([128, 4], F32, tag="onescol4")
            nc.gpsimd.memset(ones_col4[:], 1.0)
            # ---- rest of x + remaining weights ----
            for g in range(4, 8):
                dma_x(g)
            dma_w(wv_a, wv_d)
            dma_w(wo_a, wo_d)
            dma_w(wu_a, wu_d)

            xT = [bpool.tile([128, N], F32R, tag=f"xT{c}", name=f"xT{c}") for c in range(2)]
            for ib in range(NB):
                for c in range(2):
                    tp = mmp.tile([128, 128], F32R, tag="mm")
                    nc.tensor.transpose(
                        tp[:], xs_all[:, ib * DM + c * 128:ib * DM + (c + 1) * 128], ident[:]
                    )
                    nc.vector.tensor_copy(xT[c][:, ib * 128:(ib + 1) * 128], tp[:])

            # ---- q^T, k^T (col-major) ----
            qT = [bpool.tile([128, N], F32R, tag=f"qT{b}", name=f"qT{b}") for b in range(2)]
            kT = [bpool.tile([128, N], F32R, tag=f"kT{b}", name=f"kT{b}") for b in range(2)]

            def emit_qk(blk):
                for w_sb, b_c, dstT in ((wq_sb, bq_c, qT), (wk_sb, bk_c, kT)):
                    for it in range(IT):
                        pt = mmp.tile([128, 512], F32, tag="mm", name="qkpt")
                        for c in range(2):
                            nc.tensor.matmul(
                                pt[:],
                                w_sb[c][:, blk * 128:(blk + 1) * 128],
                                xT[c][:, it * 512:(it + 1) * 512],
                                start=(c == 0),
                                stop=(c == 1),
                            )
                        nc.vector.tensor_scalar_add(
                            dstT[blk][:, it * 512:(it + 1) * 512], pt[:], b_c[blk][:]
                        )

            emit_qk(0)

            # ---- v (row-major, with ones col per head) ----
            # v_sb[jb]: [128, 4*65]; head h data at cols 65h..65h+63, ones at 65h+64
            v_sb = [bpool.tile([128, 4 * 65], F32R, tag=f"v{jb}", name=f"v{jb}") for jb in range(NB)]

            def emit_v(jb):
                v4 = v_sb[jb][:].rearrange("p (h e) -> p h e", e=65)
                nc.vector.tensor_copy(
                    v4[:, :, 64:65],
                    ones_col4[:].rearrange("p (h e) -> p h e", e=1),
                )
                pt = mmp.tile([128, DM], F32, tag="mm", name="vpt")
                for c in range(2):
                    nc.tensor.matmul(
                        pt[:],
                        xT[c][:, jb * 128:(jb + 1) * 128],
                        wv_sb[c][:],
                        start=(c == 0),
                        stop=(c == 1),
                    )
                nc.vector.tensor_tensor(
                    v4[:, :, 0:64],
                    pt[:].rearrange("p (h e) -> p h e", e=64),
                    bv_bc[:].rearrange("p (h e) -> p h e", e=64),
                    op=mybir.AluOpType.add,
                )

            # ---- attention per head ----
            ectx = [bpool.tile([128, N], F32R, tag=f"ectx{b}", name=f"ectx{b}") for b in range(2)]
            m_sb = [bpool.tile([128, N], F32R, tag=f"m{b}", name=f"m{b}") for b in range(2)]
            uT = [bpool.tile([128, N], F32R, tag=f"uT{b}", name=f"uT{b}") for b in range(2)]
            ostage = stage
            out_r = r(out_d.rearrange("(t p) d -> p t d", p=128))
            def attention(h, half, with_v=False):
                qh = qT[h // 2][64 * (h % 2):64 * (h % 2) + 64, :]
                kh = kT[h // 2][64 * (h % 2):64 * (h % 2) + 64, :]
                dst = ectx[h // 2][64 * (h % 2):64 * (h % 2) + 64, :]
                hstart, hend = 1024 * half, 1024 * (half + 1)
                jb_max = 8 * (half + 1)
                ctx_q = [
                    ctxp.tile([65, 512], F32, tag="ctx", bufs=2, name="ctxq")
                    for _ in range(2)
                ]
                for jb in range(jb_max):
                    it0 = jb // 4

                    def col_start(it):
                        # partial-width diagonal tiles (min 256 wide to stay
                        # in the f32r 1-cyc/row regime)
                        if it == it0:
                            return it * 512 + min(128 * (jb % 4), 256)
                        return it * 512

                    its = [t for t in range(max(it0, 2 * half), 2 * half + 2)]
                    if with_v and (half == 0 or jb >= 8):
                        emit_v(jb)
                    cst0 = col_start(its[0])
                    sc = scp.tile([128, 1024], F32, tag="sc", name="sc")
                    for it in its:
                        cst, cend = col_start(it), (it + 1) * 512
                        nc.tensor.matmul(
                            sc[:, cst - hstart:cend - hstart],
                            kh[:, jb * 128:(jb + 1) * 128],
                            qh[:, cst:cend],
                            start=True,
                            stop=True,
                        )
                    wtot = hend - cst0
                    # leading cols that are entirely masked (select fills
                    # them with 0 without reading exp output)
                    skip = 128 if (its[0] == it0 and jb % 4 == 3) else 0
                    e = wpool.tile([128, 1024], F32R, tag="e", bufs=8, name="e")
                    nc.scalar.activation(
                        e[:, skip:wtot], sc[:, cst0 - hstart + skip:1024],
                        mybir.ActivationFunctionType.Exp,
                        scale=float(1.0 / np.sqrt(HD)),
                    )
                    if its[0] == it0:
                        # only cols where some row can be invalid:
                        # f < p - base, base in {0, -128}
                        wd = 128 if (jb % 4) < 3 else 256
                        nc.gpsimd.affine_select(
                            e[:, 0:wd], e[:, 0:wd],
                            pattern=[[1, wd]],
                            compare_op=mybir.AluOpType.is_ge,
                            fill=0.0,
                            base=cst0 - 128 * jb,
                            channel_multiplier=-1,
                        )
                    for it in its:
                        cst, cend = col_start(it), (it + 1) * 512
                        last_jb = min(4 * it + 3, jb_max - 1)
                        cq = ctx_q[it - 2 * half]
                        qoff = it * 512
                        nc.tensor.matmul(
                            cq[0:65, cst - qoff:cend - qoff],
                            v_sb[jb][:, 65 * h:65 * h + 65],
                            e[:, cst - cst0:cend - cst0],
                            start=(jb == 0),
                            stop=(jb == last_jb),
                            skip_group_check=True,
                        )
                        if jb == last_jb:
                            # this quarter's accumulation is final: normalize
                            # and evict now, freeing its PSUM bank early
                            recip = wpool.tile(
                                [1, 512], F32, tag="recip", bufs=2, name="recip"
                            )
                            nc.vector.reciprocal(recip[:], cq[64:65, :])
                            rb = wpool.tile([64, 512], F32, tag="rb", bufs=2, name="rb")
                            nc.gpsimd.partition_broadcast(rb[:], recip[:])
                            nc.vector.tensor_tensor(
                                dst[:, it * 512:(it + 1) * 512],
                                cq[0:64, :],
                                rb[:],
                                op=mybir.AluOpType.mult,
                            )


            def tail(it):
                isl = slice(it * 512, (it + 1) * 512)
                for blk in range(2):
                    pt = mmp.tile([128, 512], F32, tag="mm", name="pt")
                    for c in range(2):
                        nc.tensor.matmul(
                            pt[:],
                            wo_sb[c][:, blk * 128:(blk + 1) * 128],
                            ectx[c][:, isl],
                            start=(c == 0),
                            stop=(c == 1),
                        )
                    nc.vector.tensor_scalar_add(m_sb[blk][:, isl], pt[:], bo_c[blk][:])
                for blk in range(2):
                    pt = mmp.tile([128, 512], F32, tag="mm", name="pt")
                    for c in range(4):
                        rhs = xT[c] if c < 2 else m_sb[c - 2]
                        nc.tensor.matmul(
                            pt[:],
                            wu_sb[c][:, blk * 128:(blk + 1) * 128],
                            rhs[:, isl],
                            start=(c == 0),
                            stop=(c == 3),
                        )
                    nc.vector.tensor_scalar(
                        uT[blk][:, isl], pt[:], bu_c[blk][:], 0.0,
                        op0=mybir.AluOpType.add, op1=mybir.AluOpType.max,
                    )
                for ib in range(it * 4, (it + 1) * 4):
                    for blk in range(2):
                        tp = scp.tile([128, 128], F32R, tag="sc", name="tp")
                        nc.tensor.transpose(
                            tp[:], uT[blk][:, ib * 128:(ib + 1) * 128], ident[:]
                        )
                        nc.scalar.copy(
                            ostage[:, ib * DM + blk * 128:ib * DM + (blk + 1) * 128],
                            tp[:],
                        )
                nc.sync.dma_start(
                    out_r[:, it * 4:(it + 1) * 4, :],
                    ostage[:, it * 4 * DM:(it + 1) * 4 * DM].rearrange(
                        "p (t d) -> p t d", d=DM
                    ),
                )

            for h in range(H):
                if h == 2:
                    emit_qk(1)
                for half in range(2):
                    attention(h, half, with_v=(h == 0))
            for it in range(IT):
                tail(it)

    nc.compile()
    return nc


_STATE = {}


def _get_runner():
    if "run" in _STATE:
        return _STATE["run"]
    import jax
    from concourse.bass2jax import (
        _bass_exec_p,
        install_neuronx_cc_hook,
        partition_id_tensor,
    )
    from jax.sharding import Mesh, PartitionSpec
    from jax.experimental.shard_map import shard_map

    nc = build_program()
    install_neuronx_cc_hook()
    partition_name = nc.partition_id_tensor.name if nc.partition_id_tensor else None
    in_names, out_names, out_avals, zero_outs = [], [], [], []
    for alloc in nc.m.functions[0].allocations:
        if not isinstance(alloc, mybir.MemoryLocationSet):
            continue
        name = alloc.memorylocations[0].name
        if alloc.kind == "ExternalInput":
            if name != partition_name:
                in_names.append(name)
        elif alloc.kind == "ExternalOutput":
            shape = tuple(alloc.tensor_shape)
            dtype = mybir.dt.np(alloc.dtype)
            out_names.append(name)
            out_avals.append(jax.core.ShapedArray(shape, dtype))
            zero_outs.append(np.zeros(shape, dtype))
    n_params = len(in_names)
    all_in = in_names + out_names + ([partition_name] if partition_name else [])

    def _body(*args):
        operands = list(args)
        if partition_name is not None:
            operands.append(partition_id_tensor())
        return tuple(
            _bass_exec_p.bind(
                *operands,
                out_avals=tuple(out_avals),
                in_names=tuple(all_in),
                out_names=tuple(out_names),
                lowering_input_output_aliases=(),
                sim_require_finite=True,
                sim_require_nnan=True,
                nc=nc,
            )
        )

    devices = jax.devices()[:B]
    mesh = Mesh(np.asarray(devices), ("core",))
    specs = (PartitionSpec("core"),) * (n_params + len(out_names))
    jitted = jax.jit(
        shard_map(
            _body, mesh=mesh, in_specs=specs,
            out_specs=(PartitionSpec("core"),) * len(out_names), check_rep=False,
        ),
        keep_unused=True,
    )

    def run(in_maps):
        import jax as _jax

        concat_in = [
            np.concatenate([np.asarray(m[nm]) for m in in_maps], axis=0)
            for nm in in_names
        ]
        concat_zero = [
            np.zeros((B * z.shape[0], *z.shape[1:]), z.dtype) for z in zero_outs
        ]
        outs = jitted(*concat_in, *concat_zero)
        _jax.block_until_ready(outs)
        res = []
        o = np.asarray(outs[out_names.index("out")])
        per = o.shape[0] // B
        for c in range(B):
            res.append(o[c * per:(c + 1) * per])
        return res

    _STATE["run"] = run
    return run


def make_in_maps(node_features, Wq, bq, Wk, bk, Wv, bv, Wo, bo, Wu, bu):
    in_maps = []
    for c in range(B):
        in_maps.append(
            {
                "x": np.ascontiguousarray(node_features[c], dtype=np.float32),
                "wq": np.asarray(Wq, np.float32),
                "wk": np.asarray(Wk, np.float32),
                "wv": np.asarray(Wv, np.float32),
                "wo": np.asarray(Wo, np.float32),
                "wu": np.asarray(Wu, np.float32),
                "bq": np.asarray(bq, np.float32),
                "bk": np.asarray(bk, np.float32),
                "bv": np.asarray(bv, np.float32),
                "bo": np.asarray(bo, np.float32),
                "bu": np.asarray(bu, np.float32),
            }
        )
    return in_maps


def kernel(
    node_features, causal_mask, Wq, bq, Wk, bk, Wv, bv, Wo, bo, Wu, bu
):
    """Full-input entry point: shards batch across 8 cores internally."""
    del causal_mask  # guaranteed tril(ones); mask generated on-chip
    run = _get_runner()
    in_maps = make_in_maps(node_features, Wq, bq, Wk, bk, Wv, bv, Wo, bo, Wu, bu)
    outs = run(in_maps)
    return np.stack(outs, axis=0)


# revision 43
# speedup vs baseline: 1.6189x; 1.0136x over previous
"""Trainium2 Bass kernel for CausalMessagePassing (B=8, N=2048, D=256, H=4).

Strategy: data-parallel across 8 NeuronCores, one graph per core.
Per-core dataflow is column-major ("transposed spine"):
  x^T -> q^T,k^T (col-major), v (row-major with ones column for softmax sums)
  scores^T[j,i] = k^T.T @ q^T per head (f32r matmuls, 1 cyc/row)
  e = exp(scores * 1/sqrt(hd)) with causal mask applied on-chip via
  affine_select (the [N,N] mask input is tril(ones) by construction and is
  never DMA'd).
  ctx'^T[65,i] = v'.T @ e^T accumulated over j-blocks; row 64 = softmax sums.
  normalize via K=1 broadcast matmul of 1/sums, fused into PSUM eviction.
  messages^T = Wo.T @ ectx^T (+bo), u^T = relu(Wu.T @ [x^T; m^T] + bu),
  PE-transpose u^T -> u, DMA out.
"""
import sys

sys.path.insert(0, "/opt/trn_rl_repo")

import numpy as np

import concourse.bass as bass  # noqa: F401
import concourse.mybir as mybir
import concourse.tile as tile
from concourse import bacc
from concourse.masks import make_identity

B, N, DM, H = 8, 2048, 256, 4
HD = DM // H  # 64
NB = N // 128  # 16 j-blocks
IT = N // 512  # 4 i-tiles
F32 = mybir.dt.float32
F32R = mybir.dt.float32r


def build_program():
    nc = bacc.Bacc("TRN2", target_bir_lowering=False, debug=False)
    x_d = nc.dram_tensor("x", [N, DM], F32, kind="ExternalInput").ap()
    wq_d = nc.dram_tensor("wq", [DM, DM], F32, kind="ExternalInput").ap()
    wk_d = nc.dram_tensor("wk", [DM, DM], F32, kind="ExternalInput").ap()
    wv_d = nc.dram_tensor("wv", [DM, DM], F32, kind="ExternalInput").ap()
    wo_d = nc.dram_tensor("wo", [DM, DM], F32, kind="ExternalInput").ap()
    wu_d = nc.dram_tensor("wu", [2 * DM, DM], F32, kind="ExternalInput").ap()
    bq_d = nc.dram_tensor("bq", [DM], F32, kind="ExternalInput").ap()
    bk_d = nc.dram_tensor("bk", [DM], F32, kind="ExternalInput").ap()
    bv_d = nc.dram_tensor("bv", [DM], F32, kind="ExternalInput").ap()
    bo_d = nc.dram_tensor("bo", [DM], F32, kind="ExternalInput").ap()
    bu_d = nc.dram_tensor("bu", [DM], F32, kind="ExternalInput").ap()
    out_d = nc.dram_tensor("out", [N, DM], F32, kind="ExternalOutput").ap()

    def r(ap):
        return ap.bitcast(F32R)

    with tile.TileContext(nc) as tc:
        with (
            tc.tile_pool(name="const", bufs=1) as cpool,
            tc.tile_pool(name="big", bufs=1) as bpool,
            tc.tile_pool(name="work", bufs=3) as wpool,
            tc.tile_pool(name="mm", bufs=2, space="PSUM") as mmp,
            tc.tile_pool(name="sc", bufs=2, space="PSUM") as scp,
            tc.tile_pool(name="ctxp", bufs=1, space="PSUM") as ctxp,
        )            :
            # ---- constants / weights (batched DMAs) ----
            ident = cpool.tile([128, 128], F32R, tag="ident")
            ident_f = cpool.tile([128, 128], F32, tag="identf")
            make_identity(nc, ident_f[:])
            nc.vector.tensor_copy(ident[:], ident_f[:])
            # PE HAM warm-up during the input-DMA window: dummy transposes
            # keep the PE busy so real matmuls start at full clock. Also
            # preload the ACT exp table set off the critical path.
            warm = scp.tile([128, 1024], F32R, tag="sc", name="warm")
            for _ in range(32):
                nc.tensor.transpose(warm[0:128, 0:128], ident[:], ident[:])
            wexp = cpool.tile([1, 8], F32, tag="wexp")
            nc.scalar.activation(
                wexp[:], ident_f[0:1, 0:8], mybir.ActivationFunctionType.Exp
            )
            # each W loaded as one DMA: [128, 2*DM], chunk c at cols [c*DM, (c+1)*DM)
            wq_a = cpool.tile([128, 2 * DM], F32R, tag="wqa")
            wk_a = cpool.tile([128, 2 * DM], F32R, tag="wka")
            wv_a = cpool.tile([128, 2 * DM], F32R, tag="wva")
            wo_a = cpool.tile([128, 2 * DM], F32R, tag="woa")
            wu_a = cpool.tile([128, 4 * DM], F32R, tag="wua")

            def dma_w(t_sb, t_d):
                nc.sync.dma_start(
                    t_sb[:].rearrange("p (c d) -> p c d", d=DM),
                    r(t_d.rearrange("(c p) d -> p c d", p=128)),
                )

            stage = cpool.tile([128, NB * DM], F32R, tag="stage")
            xs_all = stage
            x_r = r(x_d.rearrange("(t p) d -> p t d", p=128))

            def dma_x(g):
                nc.sync.dma_start(
                    xs_all[:, g * 2 * DM:(g + 1) * 2 * DM].rearrange(
                        "p (t d) -> p t d", d=DM
                    ),
                    x_r[:, g * 2:(g + 1) * 2, :],
                )

            dma_x(0)
            dma_x(1)
            dma_w(wq_a, wq_d)
            dma_w(wk_a, wk_d)
            dma_x(2)
            dma_x(3)
            wq_sb = [wq_a[:, c * DM:(c + 1) * DM] for c in range(2)]
            wk_sb = [wk_a[:, c * DM:(c + 1) * DM] for c in range(2)]
            wv_sb = [wv_a[:, c * DM:(c + 1) * DM] for c in range(2)]
            wo_sb = [wo_a[:, c * DM:(c + 1) * DM] for c in range(2)]
            wu_sb = [wu_a[:, c * DM:(c + 1) * DM] for c in range(4)]
            bq_a = cpool.tile([128, 2], F32, tag="bqa")
            bk_a = cpool.tile([128, 2], F32, tag="bka")
            bo_a = cpool.tile([128, 2], F32, tag="boa")
            bu_a = cpool.tile([128, 2], F32, tag="bua")
            for t_sb, t_d in ((bq_a, bq_d), (bk_a, bk_d), (bo_a, bo_d), (bu_a, bu_d)):
                nc.sync.dma_start(t_sb[:], t_d.rearrange("(c p) -> p c", p=128))
            bq_c = [bq_a[:, b:b + 1] for b in range(2)]
            bk_c = [bk_a[:, b:b + 1] for b in range(2)]
            bo_c = [bo_a[:, b:b + 1] for b in range(2)]
            bu_c = [bu_a[:, b:b + 1] for b in range(2)]
            # bv broadcast tile [128, 256] (f32; only used by DVE add)
            bv_row = cpool.tile([1, DM], F32, tag="bvrow")
            nc.sync.dma_start(bv_row[:], bv_d.rearrange("(b a) -> b a", b=1))
            ones1 = cpool.tile([1, 128], F32, tag="ones1")
            nc.gpsimd.memset(ones1[:], 1.0)
            bv_bc = cpool.tile([128, DM], F32, tag="bvbc")
            pt = mmp.tile([128, DM], F32, tag="mm")
            nc.tensor.matmul(pt[:], ones1[:], bv_row[:], start=True, stop=True)
            nc.vector.tensor_copy(bv_bc[:], pt[:])
            ones_r = cpool.tile([1, 64], F32R, tag="onesr")
            ones_rf = cpool.tile([1, 64], F32, tag="onesrf")
            nc.gpsimd.memset(ones_rf[:], 1.0)
            nc.vector.tensor_copy(ones_r[:], ones_rf[:])
            ones_col4 = cpool.tile([128, 4], F32, tag="onescol4")
            nc.gpsimd.memset(ones_col4[:], 1.0)
            # ---- rest of x + remaining weights ----
            for g in range(4, 8):
                dma_x(g)
            dma_w(wv_a, wv_d)
            dma_w(wo_a, wo_d)
            dma_w(wu_a, wu_d)

            xT = [bpool.tile([128, N], F32R, tag=f"xT{c}", name=f"xT{c}") for c in range(2)]
            for ib in range(NB):
                for c in range(2):
                    tp = mmp.tile([128, 128], F32R, tag="mm")
                    nc.tensor.transpose(
                        tp[:], xs_all[:, ib * DM + c * 128:ib * DM + (c + 1) * 128], ident[:]
                    )
                    nc.vector.tensor_copy(xT[c][:, ib * 128:(ib + 1) * 128], tp[:])

            # ---- q^T, k^T (col-major) ----
            qT = [bpool.tile([128, N], F32R, tag=f"qT{b}", name=f"qT{b}") for b in range(2)]
            kT = [bpool.tile([128, N], F32R, tag=f"kT{b}", name=f"kT{b}") for b in range(2)]

            def emit_qk(blk):
                for w_sb, b_c, dstT in ((wq_sb, bq_c, qT), (wk_sb, bk_c, kT)):
                    for it in range(IT):
                        pt = mmp.tile([128, 512], F32, tag="mm", name="qkpt")
                        for c in range(2):
                            nc.tensor.matmul(
                                pt[:],
                                w_sb[c][:, blk * 128:(blk + 1) * 128],
                                xT[c][:, it * 512:(it + 1) * 512],
                                start=(c == 0),
                                stop=(c == 1),
                            )
                        nc.vector.tensor_scalar_add(
                            dstT[blk][:, it * 512:(it + 1) * 512], pt[:], b_c[blk][:]
                        )

            emit_qk(0)

            # ---- v (row-major, with ones col per head) ----
            # v_sb[jb]: [128, 4*65]; head h data at cols 65h..65h+63, ones at 65h+64
            v_sb = [bpool.tile([128, 4 * 65], F32R, tag=f"v{jb}", name=f"v{jb}") for jb in range(NB)]

            def emit_v(jb):
                v4 = v_sb[jb][:].rearrange("p (h e) -> p h e", e=65)
                nc.vector.tensor_copy(
                    v4[:, :, 64:65],
                    ones_col4[:].rearrange("p (h e) -> p h e", e=1),
                )
                pt = mmp.tile([128, DM], F32, tag="mm", name="vpt")
                for c in range(2):
                    nc.tensor.matmul(
                        pt[:],
                        xT[c][:, jb * 128:(jb + 1) * 128],
                        wv_sb[c][:],
                        start=(c == 0),
                        stop=(c == 1),
                    )
                nc.vector.tensor_tensor(
                    v4[:, :, 0:64],
                    pt[:].rearrange("p (h e) -> p h e", e=64),
                    bv_bc[:].rearrange("p (h e) -> p h e", e=64),
                    op=mybir.AluOpType.add,
                )

            # ---- attention per head ----
            ectx = [bpool.tile([128, N], F32R, tag=f"ectx{b}", name=f"ectx{b}") for b in range(2)]
            m_sb = [bpool.tile([128, N], F32R, tag=f"m{b}", name=f"m{b}") for b in range(2)]
            uT = [bpool.tile([128, N], F32R, tag=f"uT{b}", name=f"uT{b}") for b in range(2)]
            ostage = stage
            out_r = r(out_d.rearrange("(t p) d -> p t d", p=128))
            def attention(h, half, with_v=False):
                qh = qT[h // 2][64 * (h % 2):64 * (h % 2) + 64, :]
                kh = kT[h // 2][64 * (h % 2):64 * (h % 2) + 64, :]
                dst = ectx[h // 2][64 * (h % 2):64 * (h % 2) + 64, :]
                hstart, hend = 1024 * half, 1024 * (half + 1)
                jb_max = 8 * (half + 1)
                ctx_q = [
                    ctxp.tile([65, 512], F32, tag="ctx", bufs=2, name="ctxq")
                    for _ in range(2)
                ]
                for jb in range(jb_max):
                    it0 = jb // 4

                    def col_start(it):
                        # partial-width diagonal tiles (min 256 wide to stay
                        # in the f32r 1-cyc/row regime)
                        if it == it0:
                            return it * 512 + min(128 * (jb % 4), 256)
                        return it * 512

                    its = [t for t in range(max(it0, 2 * half), 2 * half + 2)]
                    if with_v and (half == 0 or jb >= 8):
                        emit_v(jb)
                    cst0 = col_start(its[0])
                    sc = scp.tile([128, 1024], F32, tag="sc", name="sc")
                    for it in its:
                        cst, cend = col_start(it), (it + 1) * 512
                        nc.tensor.matmul(
                            sc[:, cst - hstart:cend - hstart],
                            kh[:, jb * 128:(jb + 1) * 128],
                            qh[:, cst:cend],
                            start=True,
                            stop=True,
                        )
                    wtot = hend - cst0
                    # leading cols that are entirely masked (select fills
                    # them with 0 without reading exp output)
                    skip = 128 if (its[0] == it0 and jb % 4 == 3) else 0
                    e = wpool.tile([128, 1024], F32R, tag="e", bufs=8, name="e")
                    nc.scalar.activation(
                        e[:, skip:wtot], sc[:, cst0 - hstart + skip:1024],
                        mybir.ActivationFunctionType.Exp,
                        scale=float(1.0 / np.sqrt(HD)),
                    )
                    if its[0] == it0:
                        # only cols where some row can be invalid:
                        # f < p - base, base in {0, -128}
                        wd = 128 if (jb % 4) < 3 else 256
                        nc.gpsimd.affine_select(
                            e[:, 0:wd], e[:, 0:wd],
                            pattern=[[1, wd]],
                            compare_op=mybir.AluOpType.is_ge,
                            fill=0.0,
                            base=cst0 - 128 * jb,
                            channel_multiplier=-1,
                        )
                    for it in its:
                        cst, cend = col_start(it), (it + 1) * 512
                        last_jb = min(4 * it + 3, jb_max - 1)
                        cq = ctx_q[it - 2 * half]
                        qoff = it * 512
                        nc.tensor.matmul(
                            cq[0:65, cst - qoff:cend - qoff],
                            v_sb[jb][:, 65 * h:65 * h + 65],
                            e[:, cst - cst0:cend - cst0],
                            start=(jb == 0),
                            stop=(jb == last_jb),
                            skip_group_check=True,
                        )
                        if jb == last_jb:
                            # this quarter's accumulation is final: normalize
                            # and evict now, freeing its PSUM bank early
                            recip = wpool.tile(
                                [1, 512], F32, tag="recip", bufs=2, name="recip"
                            )
                            nc.vector.reciprocal(recip[:], cq[64:65, :])
                            rb = wpool.tile([64, 512], F32, tag="rb", bufs=2, name="rb")
                            nc.gpsimd.partition_broadcast(rb[:], recip[:])
                            nc.vector.tensor_tensor(
                                dst[:, it * 512:(it + 1) * 512],
                                cq[0:64, :],
                                rb[:],
                                op=mybir.AluOpType.mult,
                            )


            def tail(it):
                isl = slice(it * 512, (it + 1) * 512)
                for blk in range(2):
                    pt = mmp.tile([128, 512], F32, tag="mm", name="pt")
                    for c in range(2):
                        nc.tensor.matmul(
                            pt[:],
                            wo_sb[c][:, blk * 128:(blk + 1) * 128],
                            ectx[c][:, isl],
                            start=(c == 0),
                            stop=(c == 1),
                        )
                    nc.vector.tensor_scalar_add(m_sb[blk][:, isl], pt[:], bo_c[blk][:])
                for blk in range(2):
                    pt = mmp.tile([128, 512], F32, tag="mm", name="pt")
                    for c in range(4):
                        rhs = xT[c] if c < 2 else m_sb[c - 2]
                        nc.tensor.matmul(
                            pt[:],
                            wu_sb[c][:, blk * 128:(blk + 1) * 128],
                            rhs[:, isl],
                            start=(c == 0),
                            stop=(c == 3),
                        )
                    nc.vector.tensor_scalar(
                        uT[blk][:, isl], pt[:], bu_c[blk][:], 0.0,
                        op0=mybir.AluOpType.add, op1=mybir.AluOpType.max,
                    )
                for ib in range(it * 4, (it + 1) * 4):
                    for blk in range(2):
                        tp = scp.tile([128, 128], F32R, tag="sc", name="tp")
                        nc.tensor.transpose(
                            tp[:], uT[blk][:, ib * 128:(ib + 1) * 128], ident[:]
                        )
                        nc.scalar.copy(
                            ostage[:, ib * DM + blk * 128:ib * DM + (blk + 1) * 128],
                            tp[:],
                        )
                nc.sync.dma_start(
                    out_r[:, it * 4:(it + 1) * 4, :],
                    ostage[:, it * 4 * DM:(it + 1) * 4 * DM].rearrange(
                        "p (t d) -> p t d", d=DM
                    ),
                )

            for h in range(H):
                if h == 2:
                    emit_qk(1)
                for half in range(2):
                    attention(h, half, with_v=(h == 0))
            for it in range(IT):
                tail(it)

    nc.compile()
    return nc


_STATE = {}


def _get_runner():
    if "run" in _STATE:
        return _STATE["run"]
    import jax
    from concourse.bass2jax import (
        _bass_exec_p,
        install_neuronx_cc_hook,
        partition_id_tensor,
    )
    from jax.sharding import Mesh, PartitionSpec
    from jax.experimental.shard_map import shard_map

    nc = build_program()
    install_neuronx_cc_hook()
    partition_name = nc.partition_id_tensor.name if nc.partition_id_tensor else None
    in_names, out_names, out_avals, zero_outs = [], [], [], []
    for alloc in nc.m.functions[0].allocations:
        if not isinstance(alloc, mybir.MemoryLocationSet):
            continue
        name = alloc.memorylocations[0].name
        if alloc.kind == "ExternalInput":
            if name != partition_name:
                in_names.append(name)
        elif alloc.kind == "ExternalOutput":
            shape = tuple(alloc.tensor_shape)
            dtype = mybir.dt.np(alloc.dtype)
            out_names.append(name)
            out_avals.append(jax.core.ShapedArray(shape, dtype))
            zero_outs.append(np.zeros(shape, dtype))
    n_params = len(in_names)
    all_in = in_names + out_names + ([partition_name] if partition_name else [])

    def _body(*args):
        operands = list(args)
        if partition_name is not None:
            operands.append(partition_id_tensor())
        return tuple(
            _bass_exec_p.bind(
                *operands,
                out_avals=tuple(out_avals),
                in_names=tuple(all_in),
                out_names=tuple(out_names),
                lowering_input_output_aliases=(),
                sim_require_finite=True,
                sim_require_nnan=True,
                nc=nc,
            )
        )

    devices = jax.devices()[:B]
    mesh = Mesh(np.asarray(devices), ("core",))
    specs = (PartitionSpec("core"),) * (n_params + len(out_names))
    jitted = jax.jit(
        shard_map(
            _body, mesh=mesh, in_specs=specs,
            out_specs=(PartitionSpec("core"),) * len(out_names), check_rep=False,
        ),
        keep_unused=True,
    )

    def run(in_maps):
        import jax as _jax

        concat_in = [
            np.concatenate([np.asarray(m[nm]) for m in in_maps], axis=0)
            for nm in in_names
        ]
        concat_zero = [
            np.zeros((B * z.shape[0], *z.shape[1:]), z.dtype) for z in zero_outs
        ]
        outs = jitted(*concat_in, *concat_zero)
        _jax.block_until_ready(outs)
        res = []
        o = np.asarray(outs[out_names.index("out")])
        per = o.shape[0] // B
        for c in range(B):
            res.append(o[c * per:(c + 1) * per])
        return res

    _STATE["run"] = run
    return run


def make_in_maps(node_features, Wq, bq, Wk, bk, Wv, bv, Wo, bo, Wu, bu):
    in_maps = []
    for c in range(B):
        in_maps.append(
            {
                "x": np.ascontiguousarray(node_features[c], dtype=np.float32),
                "wq": np.asarray(Wq, np.float32),
                "wk": np.asarray(Wk, np.float32),
                "wv": np.asarray(Wv, np.float32),
                "wo": np.asarray(Wo, np.float32),
                "wu": np.asarray(Wu, np.float32),
                "bq": np.asarray(bq, np.float32),
                "bk": np.asarray(bk, np.float32),
                "bv": np.asarray(bv, np.float32),
                "bo": np.asarray(bo, np.float32),
                "bu": np.asarray(bu, np.float32),
            }
        )
    return in_maps


def kernel(
    node_features, causal_mask, Wq, bq, Wk, bk, Wv, bv, Wo, bo, Wu, bu
):
    """Full-input entry point: shards batch across 8 cores internally."""
    del causal_mask  # guaranteed tril(ones); mask generated on-chip
    run = _get_runner()
    in_maps = make_in_maps(node_features, Wq, bq, Wk, bk, Wv, bv, Wo, bo, Wu, bu)
    outs = run(in_maps)
    return np.stack(outs, axis=0)


# revision 54
# speedup vs baseline: 1.6815x; 1.0387x over previous
"""Trainium2 Bass kernel for CausalMessagePassing (B=8, N=2048, D=256, H=4).

Strategy: data-parallel across 8 NeuronCores, one graph per core.
Per-core dataflow is column-major ("transposed spine"):
  x^T -> q^T,k^T (col-major), v (row-major with ones column for softmax sums)
  scores^T[j,i] = k^T.T @ q^T per head (f32r matmuls, 1 cyc/row)
  e = exp(scores * 1/sqrt(hd)) with causal mask applied on-chip via
  affine_select (the [N,N] mask input is tril(ones) by construction and is
  never DMA'd).
  ctx'^T[65,i] = v'.T @ e^T accumulated over j-blocks; row 64 = softmax sums.
  normalize via K=1 broadcast matmul of 1/sums, fused into PSUM eviction.
  messages^T = Wo.T @ ectx^T (+bo), u^T = relu(Wu.T @ [x^T; m^T] + bu),
  PE-transpose u^T -> u, DMA out.
"""
import sys

sys.path.insert(0, "/opt/trn_rl_repo")

import numpy as np

import concourse.bass as bass  # noqa: F401
import concourse.mybir as mybir
import concourse.tile as tile
from concourse import bacc
from concourse.masks import make_identity

B, N, DM, H = 8, 2048, 256, 4
HD = DM // H  # 64
NB = N // 128  # 16 j-blocks
IT = N // 512  # 4 i-tiles
F32 = mybir.dt.float32
F32R = mybir.dt.float32r


def build_program():
    nc = bacc.Bacc("TRN2", target_bir_lowering=False, debug=False)
    x_d = nc.dram_tensor("x", [N, DM], F32, kind="ExternalInput").ap()
    wq_d = nc.dram_tensor("wq", [DM, DM], F32, kind="ExternalInput").ap()
    wk_d = nc.dram_tensor("wk", [DM, DM], F32, kind="ExternalInput").ap()
    wv_d = nc.dram_tensor("wv", [DM, DM], F32, kind="ExternalInput").ap()
    wo_d = nc.dram_tensor("wo", [DM, DM], F32, kind="ExternalInput").ap()
    wu_d = nc.dram_tensor("wu", [2 * DM, DM], F32, kind="ExternalInput").ap()
    bq_d = nc.dram_tensor("bq", [DM], F32, kind="ExternalInput").ap()
    bk_d = nc.dram_tensor("bk", [DM], F32, kind="ExternalInput").ap()
    bv_d = nc.dram_tensor("bv", [DM], F32, kind="ExternalInput").ap()
    bo_d = nc.dram_tensor("bo", [DM], F32, kind="ExternalInput").ap()
    bu_d = nc.dram_tensor("bu", [DM], F32, kind="ExternalInput").ap()
    out_d = nc.dram_tensor("out", [N, DM], F32, kind="ExternalOutput").ap()

    def r(ap):
        return ap.bitcast(F32R)

    with tile.TileContext(nc) as tc:
        with (
            tc.tile_pool(name="const", bufs=1) as cpool,
            tc.tile_pool(name="big", bufs=1) as bpool,
            tc.tile_pool(name="work", bufs=3) as wpool,
            tc.tile_pool(name="mm", bufs=2, space="PSUM") as mmp,
            tc.tile_pool(name="sc", bufs=4, space="PSUM") as scp,
            tc.tile_pool(name="ctxp", bufs=1, space="PSUM") as ctxp,
        )            :
            # ---- constants / weights (batched DMAs) ----
            ident = cpool.tile([128, 128], F32R, tag="ident")
            ident_f = cpool.tile([128, 128], F32, tag="identf")
            make_identity(nc, ident_f[:])
            nc.vector.tensor_copy(ident[:], ident_f[:])
            # PE HAM warm-up during the input-DMA window: dummy transposes
            # keep the PE busy so real matmuls start at full clock. Also
            # preload the ACT exp table set off the critical path.
            warm = scp.tile([128, 512], F32R, tag="sc", name="warm")
            for _ in range(32):
                nc.tensor.transpose(warm[0:128, 0:128], ident[:], ident[:])
            wexp = cpool.tile([1, 8], F32, tag="wexp")
            nc.scalar.activation(
                wexp[:], ident_f[0:1, 0:8], mybir.ActivationFunctionType.Exp
            )
            # each W loaded as one DMA: [128, 2*DM], chunk c at cols [c*DM, (c+1)*DM)
            wq_a = cpool.tile([128, 2 * DM], F32R, tag="wqa")
            wk_a = cpool.tile([128, 2 * DM], F32R, tag="wka")
            wv_a = cpool.tile([128, 2 * DM], F32R, tag="wva")
            wo_a = cpool.tile([128, 2 * DM], F32R, tag="woa")
            wu_a = cpool.tile([128, 4 * DM], F32R, tag="wua")

            def dma_w(t_sb, t_d):
                nc.sync.dma_start(
                    t_sb[:].rearrange("p (c d) -> p c d", d=DM),
                    r(t_d.rearrange("(c p) d -> p c d", p=128)),
                )

            stage = cpool.tile([128, NB * DM], F32R, tag="stage")
            xs_all = stage
            x_r = r(x_d.rearrange("(t p) d -> p t d", p=128))

            def dma_x(g):
                nc.sync.dma_start(
                    xs_all[:, g * 2 * DM:(g + 1) * 2 * DM].rearrange(
                        "p (t d) -> p t d", d=DM
                    ),
                    x_r[:, g * 2:(g + 1) * 2, :],
                )

            dma_x(0)
            dma_x(1)
            dma_w(wq_a, wq_d)
            dma_w(wk_a, wk_d)
            dma_x(2)
            dma_x(3)
            wq_sb = [wq_a[:, c * DM:(c + 1) * DM] for c in range(2)]
            wk_sb = [wk_a[:, c * DM:(c + 1) * DM] for c in range(2)]
            wv_sb = [wv_a[:, c * DM:(c + 1) * DM] for c in range(2)]
            wo_sb = [wo_a[:, c * DM:(c + 1) * DM] for c in range(2)]
            wu_sb = [wu_a[:, c * DM:(c + 1) * DM] for c in range(4)]
            bq_a = cpool.tile([128, 2], F32, tag="bqa")
            bk_a = cpool.tile([128, 2], F32, tag="bka")
            bo_a = cpool.tile([128, 2], F32, tag="boa")
            bu_a = cpool.tile([128, 2], F32, tag="bua")
            for t_sb, t_d in ((bq_a, bq_d), (bk_a, bk_d), (bo_a, bo_d), (bu_a, bu_d)):
                nc.sync.dma_start(t_sb[:], t_d.rearrange("(c p) -> p c", p=128))
            bq_c = [bq_a[:, b:b + 1] for b in range(2)]
            bk_c = [bk_a[:, b:b + 1] for b in range(2)]
            bo_c = [bo_a[:, b:b + 1] for b in range(2)]
            bu_c = [bu_a[:, b:b + 1] for b in range(2)]
            # bv broadcast tile [128, 256] (f32; only used by DVE add)
            bv_row = cpool.tile([1, DM], F32, tag="bvrow")
            nc.sync.dma_start(bv_row[:], bv_d.rearrange("(b a) -> b a", b=1))
            ones1 = cpool.tile([1, 128], F32, tag="ones1")
            nc.gpsimd.memset(ones1[:], 1.0)
            bv_bc = cpool.tile([128, DM], F32, tag="bvbc")
            pt = mmp.tile([128, DM], F32, tag="mm")
            nc.tensor.matmul(pt[:], ones1[:], bv_row[:], start=True, stop=True)
            nc.vector.tensor_copy(bv_bc[:], pt[:])
            ones_r = cpool.tile([1, 64], F32R, tag="onesr")
            ones_rf = cpool.tile([1, 64], F32, tag="onesrf")
            nc.gpsimd.memset(ones_rf[:], 1.0)
            nc.vector.tensor_copy(ones_r[:], ones_rf[:])
            ones_col4 = cpool.tile([128, 4], F32, tag="onescol4")
            nc.gpsimd.memset(ones_col4[:], 1.0)
            # ---- rest of x + remaining weights ----
            for g in range(4, 8):
                dma_x(g)
            dma_w(wv_a, wv_d)
            dma_w(wo_a, wo_d)
            dma_w(wu_a, wu_d)

            xT = [bpool.tile([128, N], F32R, tag=f"xT{c}", name=f"xT{c}") for c in range(2)]
            for ib in range(NB):
                for c in range(2):
                    tp = mmp.tile([128, 128], F32R, tag="mm")
                    nc.tensor.transpose(
                        tp[:], xs_all[:, ib * DM + c * 128:ib * DM + (c + 1) * 128], ident[:]
                    )
                    nc.vector.tensor_copy(xT[c][:, ib * 128:(ib + 1) * 128], tp[:])

            # ---- q^T, k^T (col-major) ----
            qT = [bpool.tile([128, N], F32R, tag=f"qT{b}", name=f"qT{b}") for b in range(2)]
            kT = [bpool.tile([128, N], F32R, tag=f"kT{b}", name=f"kT{b}") for b in range(2)]

            def emit_qk(blk):
                for w_sb, b_c, dstT in ((wq_sb, bq_c, qT), (wk_sb, bk_c, kT)):
                    for it in range(IT):
                        pt = mmp.tile([128, 512], F32, tag="mm", name="qkpt")
                        for c in range(2):
                            nc.tensor.matmul(
                                pt[:],
                                w_sb[c][:, blk * 128:(blk + 1) * 128],
                                xT[c][:, it * 512:(it + 1) * 512],
                                start=(c == 0),
                                stop=(c == 1),
                            )
                        nc.vector.tensor_scalar_add(
                            dstT[blk][:, it * 512:(it + 1) * 512], pt[:], b_c[blk][:]
                        )

            emit_qk(0)

            # ---- v (row-major, with ones col per head) ----
            # v_sb[jb]: [128, 4*65]; head h data at cols 65h..65h+63, ones at 65h+64
            v_sb = [bpool.tile([128, 4 * 65], F32R, tag=f"v{jb}", name=f"v{jb}") for jb in range(NB)]

            def emit_v(jb):
                v4 = v_sb[jb][:].rearrange("p (h e) -> p h e", e=65)
                nc.vector.tensor_copy(
                    v4[:, :, 64:65],
                    ones_col4[:].rearrange("p (h e) -> p h e", e=1),
                )
                pt = mmp.tile([128, DM], F32, tag="mm", name="vpt")
                for c in range(2):
                    nc.tensor.matmul(
                        pt[:],
                        xT[c][:, jb * 128:(jb + 1) * 128],
                        wv_sb[c][:],
                        start=(c == 0),
                        stop=(c == 1),
                    )
                nc.vector.tensor_tensor(
                    v4[:, :, 0:64],
                    pt[:].rearrange("p (h e) -> p h e", e=64),
                    bv_bc[:].rearrange("p (h e) -> p h e", e=64),
                    op=mybir.AluOpType.add,
                )

            # ---- attention per head ----
            ectx = [bpool.tile([128, N], F32R, tag=f"ectx{b}", name=f"ectx{b}") for b in range(2)]
            m_sb = [bpool.tile([128, N], F32R, tag=f"m{b}", name=f"m{b}") for b in range(2)]
            uT = [bpool.tile([128, N], F32R, tag=f"uT{b}", name=f"uT{b}") for b in range(2)]
            ostage = stage
            out_r = r(out_d.rearrange("(t p) d -> p t d", p=128))
            def attention(h, half, with_v=False):
                qh = qT[h // 2][64 * (h % 2):64 * (h % 2) + 64, :]
                kh = kT[h // 2][64 * (h % 2):64 * (h % 2) + 64, :]
                dst = ectx[h // 2][64 * (h % 2):64 * (h % 2) + 64, :]
                hstart, hend = 1024 * half, 1024 * (half + 1)
                jb_max = 8 * (half + 1)
                ctx_q = [
                    ctxp.tile([65, 512], F32, tag="ctx", bufs=2, name="ctxq")
                    for _ in range(2)
                ]
                for jb in range(jb_max):
                    it0 = jb // 4

                    def col_start(it):
                        if it == it0:
                            return it * 512 + min(128 * (jb % 4), 256)
                        return it * 512

                    its = [t for t in range(max(it0, 2 * half), 2 * half + 2)]
                    if with_v and (half == 0 or jb >= 8):
                        emit_v(jb)
                    for it in its:
                        cst, cend = col_start(it), (it + 1) * 512
                        w = cend - cst
                        diag = it == it0
                        sc = scp.tile([128, 512], F32, tag="sc", name="sc")
                        nc.tensor.matmul(
                            sc[:, 0:w],
                            kh[:, jb * 128:(jb + 1) * 128],
                            qh[:, cst:cend],
                            start=True,
                            stop=True,
                        )
                        skip = 128 if (diag and jb % 4 == 3) else 0
                        e = wpool.tile([128, 512], F32R, tag="e", bufs=12, name="e")
                        nc.scalar.activation(
                            e[:, skip:w], sc[:, skip:w],
                            mybir.ActivationFunctionType.Exp,
                            scale=float(1.0 / np.sqrt(HD)),
                        )
                        if diag:
                            wd = 128 if (jb % 4) < 3 else 256
                            nc.gpsimd.affine_select(
                                e[:, 0:wd], e[:, 0:wd],
                                pattern=[[1, wd]],
                                compare_op=mybir.AluOpType.is_ge,
                                fill=0.0,
                                base=cst - 128 * jb,
                                channel_multiplier=-1,
                            )
                        last_jb = min(4 * it + 3, jb_max - 1)
                        cq = ctx_q[it - 2 * half]
                        qoff = it * 512
                        nc.tensor.matmul(
                            cq[0:65, cst - qoff:cend - qoff],
                            v_sb[jb][:, 65 * h:65 * h + 65],
                            e[:, 0:w],
                            start=(jb == 0),
                            stop=(jb == last_jb),
                            skip_group_check=True,
                        )
                        if jb == last_jb:
                            recip = wpool.tile(
                                [1, 512], F32, tag="recip", bufs=2, name="recip"
                            )
                            nc.vector.reciprocal(recip[:], cq[64:65, :])
                            rb = wpool.tile([64, 512], F32, tag="rb", bufs=2, name="rb")
                            nc.gpsimd.partition_broadcast(rb[:], recip[:])
                            nc.vector.tensor_tensor(
                                dst[:, it * 512:(it + 1) * 512],
                                cq[0:64, :],
                                rb[:],
                                op=mybir.AluOpType.mult,
                            )

            def tail(it):
                isl = slice(it * 512, (it + 1) * 512)
                for blk in range(2):
                    pt = mmp.tile([128, 512], F32, tag="mm", name="pt")
                    for c in range(2):
                        nc.tensor.matmul(
                            pt[:],
                            wo_sb[c][:, blk * 128:(blk + 1) * 128],
                            ectx[c][:, isl],
                            start=(c == 0),
                            stop=(c == 1),
                        )
                    nc.vector.tensor_scalar_add(m_sb[blk][:, isl], pt[:], bo_c[blk][:])
                for blk in range(2):
                    pt = mmp.tile([128, 512], F32, tag="mm", name="pt")
                    for c in range(4):
                        rhs = xT[c] if c < 2 else m_sb[c - 2]
                        nc.tensor.matmul(
                            pt[:],
                            wu_sb[c][:, blk * 128:(blk + 1) * 128],
                            rhs[:, isl],
                            start=(c == 0),
                            stop=(c == 3),
                        )
                    nc.vector.tensor_scalar(
                        uT[blk][:, isl], pt[:], bu_c[blk][:], 0.0,
                        op0=mybir.AluOpType.add, op1=mybir.AluOpType.max,
                    )
                for ib in range(it * 4, (it + 1) * 4):
                    for blk in range(2):
                        tp = scp.tile([128, 128], F32R, tag="sc", name="tp")
                        nc.tensor.transpose(
                            tp[:], uT[blk][:, ib * 128:(ib + 1) * 128], ident[:]
                        )
                        nc.scalar.copy(
                            ostage[:, ib * DM + blk * 128:ib * DM + (blk + 1) * 128],
                            tp[:],
                        )
                for g2 in range(4):
                    t0 = it * 4 + g2
                    nc.sync.dma_start(
                        out_r[:, t0:t0 + 1, :],
                        ostage[:, t0 * DM:(t0 + 1) * DM].rearrange(
                            "p (t d) -> p t d", d=DM
                        ),
                    )

            for h in range(H):
                if h == 2:
                    emit_qk(1)
                for half in range(2):
                    attention(h, half, with_v=(h == 0))
            for it in range(IT):
                tail(it)

    nc.compile()
    return nc


_STATE = {}


def _get_runner():
    if "run" in _STATE:
        return _STATE["run"]
    import jax
    from concourse.bass2jax import (
        _bass_exec_p,
        install_neuronx_cc_hook,
        partition_id_tensor,
    )
    from jax.sharding import Mesh, PartitionSpec
    from jax.experimental.shard_map import shard_map

    nc = build_program()
    install_neuronx_cc_hook()
    partition_name = nc.partition_id_tensor.name if nc.partition_id_tensor else None
    in_names, out_names, out_avals, zero_outs = [], [], [], []
    for alloc in nc.m.functions[0].allocations:
        if not isinstance(alloc, mybir.MemoryLocationSet):
            continue
        name = alloc.memorylocations[0].name
        if alloc.kind == "ExternalInput":
            if name != partition_name:
                in_names.append(name)
        elif alloc.kind == "ExternalOutput":
            shape = tuple(alloc.tensor_shape)
            dtype = mybir.dt.np(alloc.dtype)
            out_names.append(name)
            out_avals.append(jax.core.ShapedArray(shape, dtype))
            zero_outs.append(np.zeros(shape, dtype))
    n_params = len(in_names)
    all_in = in_names + out_names + ([partition_name] if partition_name else [])

    def _body(*args):
        operands = list(args)
        if partition_name is not None:
            operands.append(partition_id_tensor())
        return tuple(
            _bass_exec_p.bind(
                *operands,
                out_avals=tuple(out_avals),
                in_names=tuple(all_in),
                out_names=tuple(out_names),
                lowering_input_output_aliases=(),
                sim_require_finite=True,
                sim_require_nnan=True,
                nc=nc,
            )
        )

    devices = jax.devices()[:B]
    mesh = Mesh(np.asarray(devices), ("core",))
    specs = (PartitionSpec("core"),) * (n_params + len(out_names))
    jitted = jax.jit(
        shard_map(
            _body, mesh=mesh, in_specs=specs,
            out_specs=(PartitionSpec("core"),) * len(out_names), check_rep=False,
        ),
        keep_unused=True,
    )

    def run(in_maps):
        import jax as _jax

        concat_in = [
            np.concatenate([np.asarray(m[nm]) for m in in_maps], axis=0)
            for nm in in_names
        ]
        concat_zero = [
            np.zeros((B * z.shape[0], *z.shape[1:]), z.dtype) for z in zero_outs
        ]
        outs = jitted(*concat_in, *concat_zero)
        _jax.block_until_ready(outs)
        res = []
        o = np.asarray(outs[out_names.index("out")])
        per = o.shape[0] // B
        for c in range(B):
            res.append(o[c * per:(c + 1) * per])
        return res

    _STATE["run"] = run
    return run


def make_in_maps(node_features, Wq, bq, Wk, bk, Wv, bv, Wo, bo, Wu, bu):
    in_maps = []
    for c in range(B):
        in_maps.append(
            {
                "x": np.ascontiguousarray(node_features[c], dtype=np.float32),
                "wq": np.asarray(Wq, np.float32),
                "wk": np.asarray(Wk, np.float32),
                "wv": np.asarray(Wv, np.float32),
                "wo": np.asarray(Wo, np.float32),
                "wu": np.asarray(Wu, np.float32),
                "bq": np.asarray(bq, np.float32),
                "bk": np.asarray(bk, np.float32),
                "bv": np.asarray(bv, np.float32),
                "bo": np.asarray(bo, np.float32),
                "bu": np.asarray(bu, np.float32),
            }
        )
    return in_maps


def kernel(
    node_features, causal_mask, Wq, bq, Wk, bk, Wv, bv, Wo, bo, Wu, bu
):
    """Full-input entry point: shards batch across 8 cores internally."""
    del causal_mask  # guaranteed tril(ones); mask generated on-chip
    run = _get_runner()
    in_maps = make_in_maps(node_features, Wq, bq, Wk, bk, Wv, bv, Wo, bo, Wu, bu)
    outs = run(in_maps)
    return np.stack(outs, axis=0)


# revision 62
# speedup vs baseline: 1.6989x; 1.0103x over previous
"""Trainium2 Bass kernel for CausalMessagePassing (B=8, N=2048, D=256, H=4).

Strategy: data-parallel across 8 NeuronCores, one graph per core.
Per-core dataflow is column-major ("transposed spine"):
  x^T -> q^T,k^T (col-major), v (row-major with ones column for softmax sums)
  scores^T[j,i] = k^T.T @ q^T per head (f32r matmuls, 1 cyc/row)
  e = exp(scores * 1/sqrt(hd)) with causal mask applied on-chip via
  affine_select (the [N,N] mask input is tril(ones) by construction and is
  never DMA'd).
  ctx'^T[65,i] = v'.T @ e^T accumulated over j-blocks; row 64 = softmax sums.
  normalize via K=1 broadcast matmul of 1/sums, fused into PSUM eviction.
  messages^T = Wo.T @ ectx^T (+bo), u^T = relu(Wu.T @ [x^T; m^T] + bu),
  PE-transpose u^T -> u, DMA out.
"""
import sys

sys.path.insert(0, "/opt/trn_rl_repo")

import numpy as np

import concourse.bass as bass  # noqa: F401
import concourse.mybir as mybir
import concourse.tile as tile
from concourse import bacc
from concourse.masks import make_identity

B, N, DM, H = 8, 2048, 256, 4
HD = DM // H  # 64
NB = N // 128  # 16 j-blocks
IT = N // 512  # 4 i-tiles
F32 = mybir.dt.float32
F32R = mybir.dt.float32r


def build_program():
    nc = bacc.Bacc("TRN2", target_bir_lowering=False, debug=False)
    x_d = nc.dram_tensor("x", [N, DM], F32, kind="ExternalInput").ap()
    wq_d = nc.dram_tensor("wq", [DM, DM], F32, kind="ExternalInput").ap()
    wk_d = nc.dram_tensor("wk", [DM, DM], F32, kind="ExternalInput").ap()
    wv_d = nc.dram_tensor("wv", [DM, DM], F32, kind="ExternalInput").ap()
    wo_d = nc.dram_tensor("wo", [DM, DM], F32, kind="ExternalInput").ap()
    wu_d = nc.dram_tensor("wu", [2 * DM, DM], F32, kind="ExternalInput").ap()
    bq_d = nc.dram_tensor("bq", [DM], F32, kind="ExternalInput").ap()
    bk_d = nc.dram_tensor("bk", [DM], F32, kind="ExternalInput").ap()
    bv_d = nc.dram_tensor("bv", [DM], F32, kind="ExternalInput").ap()
    bo_d = nc.dram_tensor("bo", [DM], F32, kind="ExternalInput").ap()
    bu_d = nc.dram_tensor("bu", [DM], F32, kind="ExternalInput").ap()
    out_d = nc.dram_tensor("out", [N, DM], F32, kind="ExternalOutput").ap()

    def r(ap):
        return ap.bitcast(F32R)

    with tile.TileContext(nc) as tc:
        with (
            tc.tile_pool(name="const", bufs=1) as cpool,
            tc.tile_pool(name="big", bufs=1) as bpool,
            tc.tile_pool(name="work", bufs=3) as wpool,
            tc.tile_pool(name="mm", bufs=2, space="PSUM") as mmp,
            tc.tile_pool(name="sc", bufs=4, space="PSUM") as scp,
            tc.tile_pool(name="ctxp", bufs=1, space="PSUM") as ctxp,
        )            :
            # ---- constants / weights (batched DMAs) ----
            ident = cpool.tile([128, 128], F32R, tag="ident")
            ident_f = cpool.tile([128, 128], F32, tag="identf")
            make_identity(nc, ident_f[:])
            nc.vector.tensor_copy(ident[:], ident_f[:])
            # PE HAM warm-up during the input-DMA window: dummy transposes
            # keep the PE busy so real matmuls start at full clock. Also
            # preload the ACT exp table set off the critical path.
            warm = scp.tile([128, 512], F32R, tag="sc", name="warm")
            for _ in range(32):
                nc.tensor.transpose(warm[0:128, 0:128], ident[:], ident[:])
            wexp = cpool.tile([1, 8], F32, tag="wexp")
            nc.scalar.activation(
                wexp[:], ident_f[0:1, 0:8], mybir.ActivationFunctionType.Exp
            )
            # each W loaded as one DMA: [128, 2*DM], chunk c at cols [c*DM, (c+1)*DM)
            wq_a = cpool.tile([128, 2 * DM], F32R, tag="wqa")
            wk_a = cpool.tile([128, 2 * DM], F32R, tag="wka")
            wv_a = cpool.tile([128, 2 * DM], F32R, tag="wva")
            wo_a = cpool.tile([128, 2 * DM], F32R, tag="woa")
            wu_a = cpool.tile([128, 4 * DM], F32R, tag="wua")

            def dma_w(t_sb, t_d):
                nc.sync.dma_start(
                    t_sb[:].rearrange("p (c d) -> p c d", d=DM),
                    r(t_d.rearrange("(c p) d -> p c d", p=128)),
                )

            stage = cpool.tile([128, NB * DM], F32R, tag="stage")
            xs_all = stage
            x_r = r(x_d.rearrange("(t p) d -> p t d", p=128))

            def dma_x(g):
                nc.sync.dma_start(
                    xs_all[:, g * 2 * DM:(g + 1) * 2 * DM].rearrange(
                        "p (t d) -> p t d", d=DM
                    ),
                    x_r[:, g * 2:(g + 1) * 2, :],
                )

            dma_x(0)
            dma_x(1)
            dma_w(wq_a, wq_d)
            dma_w(wk_a, wk_d)
            dma_x(2)
            dma_x(3)
            wq_sb = [wq_a[:, c * DM:(c + 1) * DM] for c in range(2)]
            wk_sb = [wk_a[:, c * DM:(c + 1) * DM] for c in range(2)]
            wv_sb = [wv_a[:, c * DM:(c + 1) * DM] for c in range(2)]
            wo_sb = [wo_a[:, c * DM:(c + 1) * DM] for c in range(2)]
            wu_sb = [wu_a[:, c * DM:(c + 1) * DM] for c in range(4)]
            bq_a = cpool.tile([128, 2], F32, tag="bqa")
            bk_a = cpool.tile([128, 2], F32, tag="bka")
            bo_a = cpool.tile([128, 2], F32, tag="boa")
            bu_a = cpool.tile([128, 2], F32, tag="bua")
            for t_sb, t_d in ((bq_a, bq_d), (bk_a, bk_d), (bo_a, bo_d), (bu_a, bu_d)):
                nc.sync.dma_start(t_sb[:], t_d.rearrange("(c p) -> p c", p=128))
            bq_c = [bq_a[:, b:b + 1] for b in range(2)]
            bk_c = [bk_a[:, b:b + 1] for b in range(2)]
            bo_c = [bo_a[:, b:b + 1] for b in range(2)]
            bu_c = [bu_a[:, b:b + 1] for b in range(2)]
            # bv broadcast tile [128, 256] (f32; only used by DVE add)
            bv_row = cpool.tile([1, DM], F32, tag="bvrow")
            nc.sync.dma_start(bv_row[:], bv_d.rearrange("(b a) -> b a", b=1))
            ones1 = cpool.tile([1, 128], F32, tag="ones1")
            nc.gpsimd.memset(ones1[:], 1.0)
            bv_bc = cpool.tile([128, DM], F32, tag="bvbc")
            pt = mmp.tile([128, DM], F32, tag="mm")
            nc.tensor.matmul(pt[:], ones1[:], bv_row[:], start=True, stop=True)
            nc.vector.tensor_copy(bv_bc[:], pt[:])
            ones_r = cpool.tile([1, 64], F32R, tag="onesr")
            ones_rf = cpool.tile([1, 64], F32, tag="onesrf")
            nc.gpsimd.memset(ones_rf[:], 1.0)
            nc.vector.tensor_copy(ones_r[:], ones_rf[:])
            ones_col4 = cpool.tile([128, 4], F32, tag="onescol4")
            nc.gpsimd.memset(ones_col4[:], 1.0)
            # ---- rest of x + remaining weights ----
            for g in range(4, 8):
                dma_x(g)
            dma_w(wv_a, wv_d)
            dma_w(wo_a, wo_d)
            dma_w(wu_a, wu_d)

            xT = [bpool.tile([128, N], F32R, tag=f"xT{c}", name=f"xT{c}") for c in range(2)]
            qT = [bpool.tile([128, N], F32R, tag=f"qT{b}", name=f"qT{b}") for b in range(2)]
            kT = [bpool.tile([128, N], F32R, tag=f"kT{b}", name=f"kT{b}") for b in range(2)]

            def emit_qk_it(blk, it):
                for w_sb, b_c, dstT in ((wq_sb, bq_c, qT), (wk_sb, bk_c, kT)):
                    pt = mmp.tile([128, 512], F32, tag="mm", name="qkpt")
                    for c in range(2):
                        nc.tensor.matmul(
                            pt[:],
                            w_sb[c][:, blk * 128:(blk + 1) * 128],
                            xT[c][:, it * 512:(it + 1) * 512],
                            start=(c == 0),
                            stop=(c == 1),
                        )
                    nc.vector.tensor_scalar_add(
                        dstT[blk][:, it * 512:(it + 1) * 512], pt[:], b_c[blk][:]
                    )

            def emit_qk(blk):
                for it in range(IT):
                    emit_qk_it(blk, it)

            # interleave x transposes with q/k(blk0) per i-tile so scores can
            # start after the first quarter of the transpose stream
            for it in range(IT):
                for ib in range(it * 4, (it + 1) * 4):
                    for c in range(2):
                        tp = mmp.tile([128, 128], F32R, tag="mm")
                        nc.tensor.transpose(
                            tp[:], xs_all[:, ib * DM + c * 128:ib * DM + (c + 1) * 128], ident[:]
                        )
                        nc.vector.tensor_copy(xT[c][:, ib * 128:(ib + 1) * 128], tp[:])
                emit_qk_it(0, it)

            # ---- v (row-major, with ones col per head) ----
            # v_sb[jb]: [128, 4*65]; head h data at cols 65h..65h+63, ones at 65h+64
            v_sb = [bpool.tile([128, 4 * 65], F32R, tag=f"v{jb}", name=f"v{jb}") for jb in range(NB)]

            def emit_v(jb):
                v4 = v_sb[jb][:].rearrange("p (h e) -> p h e", e=65)
                nc.vector.tensor_copy(
                    v4[:, :, 64:65],
                    ones_col4[:].rearrange("p (h e) -> p h e", e=1),
                )
                pt = mmp.tile([128, DM], F32, tag="mm", name="vpt")
                for c in range(2):
                    nc.tensor.matmul(
                        pt[:],
                        xT[c][:, jb * 128:(jb + 1) * 128],
                        wv_sb[c][:],
                        start=(c == 0),
                        stop=(c == 1),
                    )
                nc.vector.tensor_tensor(
                    v4[:, :, 0:64],
                    pt[:].rearrange("p (h e) -> p h e", e=64),
                    bv_bc[:].rearrange("p (h e) -> p h e", e=64),
                    op=mybir.AluOpType.add,
                )

            # ---- attention per head ----
            ectx = [bpool.tile([128, N], F32R, tag=f"ectx{b}", name=f"ectx{b}") for b in range(2)]
            m_sb = [bpool.tile([128, N], F32R, tag=f"m{b}", name=f"m{b}") for b in range(2)]
            uT = [bpool.tile([128, N], F32R, tag=f"uT{b}", name=f"uT{b}") for b in range(2)]
            ostage = stage
            out_r = r(out_d.rearrange("(t p) d -> p t d", p=128))
            def attention(h, half, with_v=False):
                qh = qT[h // 2][64 * (h % 2):64 * (h % 2) + 64, :]
                kh = kT[h // 2][64 * (h % 2):64 * (h % 2) + 64, :]
                dst = ectx[h // 2][64 * (h % 2):64 * (h % 2) + 64, :]
                hstart, hend = 1024 * half, 1024 * (half + 1)
                jb_max = 8 * (half + 1)
                ctx_q = [
                    ctxp.tile([65, 512], F32, tag="ctx", bufs=2, name="ctxq")
                    for _ in range(2)
                ]
                for jb in range(jb_max):
                    it0 = jb // 4

                    def col_start(it):
                        if it == it0:
                            return it * 512 + min(128 * (jb % 4), 256)
                        return it * 512

                    its = [t for t in range(max(it0, 2 * half), 2 * half + 2)]
                    if with_v and (half == 0 or jb >= 8):
                        emit_v(jb)
                    for it in its:
                        cst, cend = col_start(it), (it + 1) * 512
                        w = cend - cst
                        diag = it == it0
                        sc = scp.tile([128, 512], F32, tag="sc", name="sc")
                        nc.tensor.matmul(
                            sc[:, 0:w],
                            kh[:, jb * 128:(jb + 1) * 128],
                            qh[:, cst:cend],
                            start=True,
                            stop=True,
                        )
                        skip = 128 if (diag and jb % 4 == 3) else 0
                        e = wpool.tile([128, 512], F32R, tag="e", bufs=8, name="e")
                        nc.scalar.activation(
                            e[:, skip:w], sc[:, skip:w],
                            mybir.ActivationFunctionType.Exp,
                            scale=float(1.0 / np.sqrt(HD)),
                        )
                        if diag:
                            wd = 128 if (jb % 4) < 3 else 256
                            nc.gpsimd.affine_select(
                                e[:, 0:wd], e[:, 0:wd],
                                pattern=[[1, wd]],
                                compare_op=mybir.AluOpType.is_ge,
                                fill=0.0,
                                base=cst - 128 * jb,
                                channel_multiplier=-1,
                            )
                        last_jb = min(4 * it + 3, jb_max - 1)
                        cq = ctx_q[it - 2 * half]
                        qoff = it * 512
                        nc.tensor.matmul(
                            cq[0:65, cst - qoff:cend - qoff],
                            v_sb[jb][:, 65 * h:65 * h + 65],
                            e[:, 0:w],
                            start=(jb == 0),
                            stop=(jb == last_jb),
                            skip_group_check=True,
                        )
                        if jb == last_jb:
                            recip = wpool.tile(
                                [1, 512], F32, tag="recip", bufs=2, name="recip"
                            )
                            nc.vector.reciprocal(recip[:], cq[64:65, :])
                            rb = wpool.tile([64, 512], F32, tag="rb", bufs=2, name="rb")
                            nc.gpsimd.partition_broadcast(rb[:], recip[:])
                            nc.vector.tensor_tensor(
                                dst[:, it * 512:(it + 1) * 512],
                                cq[0:64, :],
                                rb[:],
                                op=mybir.AluOpType.mult,
                            )

            def tail(it):
                isl = slice(it * 512, (it + 1) * 512)
                for blk in range(2):
                    pt = mmp.tile([128, 512], F32, tag="mm", name="pt")
                    for c in range(2):
                        nc.tensor.matmul(
                            pt[:],
                            wo_sb[c][:, blk * 128:(blk + 1) * 128],
                            ectx[c][:, isl],
                            start=(c == 0),
                            stop=(c == 1),
                        )
                    nc.vector.tensor_scalar_add(m_sb[blk][:, isl], pt[:], bo_c[blk][:])
                for blk in range(2):
                    pt = mmp.tile([128, 512], F32, tag="mm", name="pt")
                    for c in range(4):
                        rhs = xT[c] if c < 2 else m_sb[c - 2]
                        nc.tensor.matmul(
                            pt[:],
                            wu_sb[c][:, blk * 128:(blk + 1) * 128],
                            rhs[:, isl],
                            start=(c == 0),
                            stop=(c == 3),
                        )
                    nc.vector.tensor_scalar(
                        uT[blk][:, isl], pt[:], bu_c[blk][:], 0.0,
                        op0=mybir.AluOpType.add, op1=mybir.AluOpType.max,
                    )
                for ib in range(it * 4, (it + 1) * 4):
                    for blk in range(2):
                        tp = scp.tile([128, 128], F32R, tag="sc", name="tp")
                        nc.tensor.transpose(
                            tp[:], uT[blk][:, ib * 128:(ib + 1) * 128], ident[:]
                        )
                        nc.scalar.copy(
                            ostage[:, ib * DM + blk * 128:ib * DM + (blk + 1) * 128],
                            tp[:],
                        )
                for g2 in range(4):
                    t0 = it * 4 + g2
                    nc.sync.dma_start(
                        out_r[:, t0:t0 + 1, :],
                        ostage[:, t0 * DM:(t0 + 1) * DM].rearrange(
                            "p (t d) -> p t d", d=DM
                        ),
                    )

            for h in range(H):
                if h == 2:
                    emit_qk(1)
                for half in range(2):
                    attention(h, half, with_v=(h == 0))
            for it in range(IT):
                tail(it)

    nc.compile()
    return nc


_STATE = {}


def _get_runner():
    if "run" in _STATE:
        return _STATE["run"]
    import jax
    from concourse.bass2jax import (
        _bass_exec_p,
        install_neuronx_cc_hook,
        partition_id_tensor,
    )
    from jax.sharding import Mesh, PartitionSpec
    from jax.experimental.shard_map import shard_map

    nc = build_program()
    install_neuronx_cc_hook()
    partition_name = nc.partition_id_tensor.name if nc.partition_id_tensor else None
    in_names, out_names, out_avals, zero_outs = [], [], [], []
    for alloc in nc.m.functions[0].allocations:
        if not isinstance(alloc, mybir.MemoryLocationSet):
            continue
        name = alloc.memorylocations[0].name
        if alloc.kind == "ExternalInput":
            if name != partition_name:
                in_names.append(name)
        elif alloc.kind == "ExternalOutput":
            shape = tuple(alloc.tensor_shape)
            dtype = mybir.dt.np(alloc.dtype)
            out_names.append(name)
            out_avals.append(jax.core.ShapedArray(shape, dtype))
            zero_outs.append(np.zeros(shape, dtype))
    n_params = len(in_names)
    all_in = in_names + out_names + ([partition_name] if partition_name else [])

    def _body(*args):
        operands = list(args)
        if partition_name is not None:
            operands.append(partition_id_tensor())
        return tuple(
            _bass_exec_p.bind(
                *operands,
                out_avals=tuple(out_avals),
                in_names=tuple(all_in),
                out_names=tuple(out_names),
                lowering_input_output_aliases=(),
                sim_require_finite=True,
                sim_require_nnan=True,
                nc=nc,
            )
        )

    devices = jax.devices()[:B]
    mesh = Mesh(np.asarray(devices), ("core",))
    specs = (PartitionSpec("core"),) * (n_params + len(out_names))
    jitted = jax.jit(
        shard_map(
            _body, mesh=mesh, in_specs=specs,
            out_specs=(PartitionSpec("core"),) * len(out_names), check_rep=False,
        ),
        keep_unused=True,
    )

    def run(in_maps):
        import jax as _jax

        concat_in = [
            np.concatenate([np.asarray(m[nm]) for m in in_maps], axis=0)
            for nm in in_names
        ]
        concat_zero = [
            np.zeros((B * z.shape[0], *z.shape[1:]), z.dtype) for z in zero_outs
        ]
        outs = jitted(*concat_in, *concat_zero)
        _jax.block_until_ready(outs)
        res = []
        o = np.asarray(outs[out_names.index("out")])
        per = o.shape[0] // B
        for c in range(B):
            res.append(o[c * per:(c + 1) * per])
        return res

    _STATE["run"] = run
    return run


def make_in_maps(node_features, Wq, bq, Wk, bk, Wv, bv, Wo, bo, Wu, bu):
    in_maps = []
    for c in range(B):
        in_maps.append(
            {
                "x": np.ascontiguousarray(node_features[c], dtype=np.float32),
                "wq": np.asarray(Wq, np.float32),
                "wk": np.asarray(Wk, np.float32),
                "wv": np.asarray(Wv, np.float32),
                "wo": np.asarray(Wo, np.float32),
                "wu": np.asarray(Wu, np.float32),
                "bq": np.asarray(bq, np.float32),
                "bk": np.asarray(bk, np.float32),
                "bv": np.asarray(bv, np.float32),
                "bo": np.asarray(bo, np.float32),
                "bu": np.asarray(bu, np.float32),
            }
        )
    return in_maps


def kernel(
    node_features, causal_mask, Wq, bq, Wk, bk, Wv, bv, Wo, bo, Wu, bu
):
    """Full-input entry point: shards batch across 8 cores internally."""
    del causal_mask  # guaranteed tril(ones); mask generated on-chip
    run = _get_runner()
    in_maps = make_in_maps(node_features, Wq, bq, Wk, bk, Wv, bv, Wo, bo, Wu, bu)
    outs = run(in_maps)
    return np.stack(outs, axis=0)
